# revision 18
# baseline (speedup 1.0000x reference)
"""MoE (top-1 routing, E=8 experts) Trainium2 Bass kernel.

Sharding: expert-parallel over 8 NeuronCores. Every core runs a replicated
fp32 router over all N=8192 tokens (reads full x), builds the global
dispatch table via counting-sort (triangular-matmul cumsum + indirect DMA
scatter), then computes ONLY its own expert's capacity-limited MLP in bf16
(fp32 accumulate). Host combine is a pure index-based scatter of the
per-expert compact outputs using device-computed token indices.

Self-contained: hardcodes shapes from the problem spec.
"""

import os
import numpy as np
import ml_dtypes

B, T, D, H, E = 4, 2048, 1024, 4096, 8
N = B * T            # 8192 tokens
CAP = 1024           # capacity = max(4, ceil(N/E))
W = 2048             # table bucket width (max supported tokens/expert)
P = 128
NT = N // P          # 64 token tiles
AUX_LOSS_COEF = 0.01
Z_LOSS_COEF = 0.001

_nc_cache = None


def build_nc():
    import concourse.bacc as bacc
    import concourse.tile as tile
    import concourse.bass as bass
    import concourse.mybir as mybir
    from concourse.masks import make_identity, make_upper_triangular

    f32 = mybir.dt.float32
    bf16 = mybir.dt.bfloat16
    i32 = mybir.dt.int32
    AF = mybir.ActivationFunctionType
    OP = mybir.AluOpType
    AX = mybir.AxisListType

    nc = bacc.Bacc(None, target_bir_lowering=False)

    # ---- I/O ----
    x_pad = nc.dram_tensor("x", [N + 1, D], f32, kind="ExternalInput")
    wg_d = nc.dram_tensor("wg", [D, E], f32, kind="ExternalInput")
    w1_d = nc.dram_tensor("w1", [D, H], bf16, kind="ExternalInput")
    b1_d = nc.dram_tensor("b1", [H, 1], f32, kind="ExternalInput")
    w2_d = nc.dram_tensor("w2", [H, D], bf16, kind="ExternalInput")
    b2_d = nc.dram_tensor("b2", [D, 1], f32, kind="ExternalInput")
    ew_d = nc.dram_tensor("ew", [1, 1], f32, kind="ExternalInput")     # e*W

    o_d = nc.dram_tensor("o", [CAP, D], f32, kind="ExternalOutput")
    glist_d = nc.dram_tensor("glist", [CAP, 1], i32, kind="ExternalOutput")
    counts_d = nc.dram_tensor("counts", [1, E], f32, kind="ExternalOutput")
    aux_d = nc.dram_tensor("aux", [1, 1], f32, kind="ExternalOutput")

    table2 = nc.dram_tensor("table2", [E * W + 2, 2], f32, kind="Internal")

    with tile.TileContext(nc) as tc:
        with (
            tc.tile_pool(name="const", bufs=1) as cpool,
            tc.tile_pool(name="resident", bufs=1) as rpool,
            tc.tile_pool(name="xin", bufs=2) as xpool,
            tc.tile_pool(name="small", bufs=4) as spool,
        ):
            # ---------------- constants ----------------
            ident = cpool.tile([P, P], f32)
            make_identity(nc, ident[:])
            U128 = cpool.tile([P, P], f32)
            make_upper_triangular(nc, U128[:], val=1.0, diag=True)
            ones_r = cpool.tile([1, P], f32)   # row of ones (K=1 bcast matmuls)
            nc.gpsimd.memset(ones_r[:], 1.0)
            ones_c = cpool.tile([P, 1], f32)   # column of ones (partition reduce)
            nc.gpsimd.memset(ones_c[:], 1.0)
            iota8_i = cpool.tile([P, E], i32)
            nc.gpsimd.iota(iota8_i[:], pattern=[[1, E]], base=0, channel_multiplier=0)
            iota8f = cpool.tile([P, E], f32)
            nc.vector.tensor_copy(iota8f[:], iota8_i[:])
            tok_i = cpool.tile([P, NT], i32)   # tok_i[p, t] = t*128 + p
            nc.gpsimd.iota(tok_i[:], pattern=[[P, NT]], base=0, channel_multiplier=1)
            tokf = cpool.tile([P, NT], f32)
            nc.vector.tensor_copy(tokf[:], tok_i[:])

            wg_sb = cpool.tile([P, E * 8], f32)   # wg strips: col k*8+e
            nc.sync.dma_start(
                out=wg_sb[:].rearrange("p (k e) -> p k e", e=E),
                in_=wg_d[:].rearrange("(k p) e -> p k e", p=P),
            )
            b1_sb = cpool.tile([P, H // P], f32)  # b1[j*128+p] -> [p, j]
            nc.sync.dma_start(
                out=b1_sb[:].rearrange("p j -> p j ()"),
                in_=b1_d[:].rearrange("(j p) o -> p j o", p=P),
            )
            b2_sb = cpool.tile([1, D], f32)
            nc.sync.dma_start(out=b2_sb[:], in_=b2_d[:].rearrange("d o -> o d"))
            ew_sb = cpool.tile([1, 1], f32)
            nc.sync.dma_start(out=ew_sb[:], in_=ew_d[:])

            # w1 strips resident (8 MB bf16), no deps -> loads early
            w1_sb = [rpool.tile([P, H], bf16, tag=f"w1_{k}", name=f"w1_{k}") for k in range(8)]
            for k in range(8):
                nc.sync.dma_start(out=w1_sb[k][:], in_=w1_d[k * P:(k + 1) * P, :])

            # accumulators
            pacc = rpool.tile([P, E], f32, tag="pacc")
            nc.gpsimd.memset(pacc[:], 0.0)
            zacc = rpool.tile([P, 1], f32, tag="zacc")
            nc.gpsimd.memset(zacc[:], 0.0)
            base = rpool.tile([1, E], f32, tag="base")
            nc.gpsimd.memset(base[:], 0.0)

            # table2 init to sentinel 8192.0 (one DMA from an SBUF tile)
            sent = rpool.tile([P, E * W * 2 // P], f32, tag="sent")
            nc.gpsimd.memset(sent[:], float(N))
            nc.sync.dma_start(
                out=table2[0:E * W, :].rearrange("(a r) c -> a (r c)", a=P),
                in_=sent[:],
            )
            nc.sync.dma_start(out=table2[E * W:E * W + 2, :], in_=sent[0:2, 0:2])

            # resident MLP buffers
            xT_bf = [rpool.tile([P, CAP], bf16, tag=f"xT_{k}", name=f"xT_{k}") for k in range(8)]
            # hT layout: [128 h-partitions, (j, tok) free]: col j*CAP + tok
            # (H/128)*CAP*2B = 64KB/partition
            hT = rpool.tile([P, (H // P) * CAP], bf16, tag="hT")
            m_all = rpool.tile([P, 8], f32, tag="m_all")

            with (
                tc.tile_pool(name="psT", bufs=2, space="PSUM") as psT,
                tc.tile_pool(name="psS", bufs=4, space="PSUM") as psS,
                tc.tile_pool(name="psM1", bufs=2, space="PSUM") as psM1,
            ):
                # ================= PHASE A: replicated router =================
                for t in range(NT):
                    x_sb = xpool.tile([P, D], f32, tag="x_in")
                    nc.sync.dma_start(out=x_sb[:], in_=x_pad[t * P:(t + 1) * P, :])

                    # transpose 8 d-blocks, keep xT in fp32 SBUF for router matmul
                    xT_t = []
                    for k in range(8):
                        pst = psT.tile([P, P], f32)
                        nc.tensor.transpose(pst[:], x_sb[:, k * P:(k + 1) * P], ident[:])
                        xts = spool.tile([P, P], f32, tag="xT_f32")
                        nc.scalar.activation(xts[:], pst[:], AF.Copy)
                        xT_t.append(xts)

                    ps_l = psS.tile([P, E], f32, tag="ps_small")
                    for k in range(8):
                        nc.tensor.matmul(
                            ps_l[:], lhsT=xT_t[k][:], rhs=wg_sb[:, k * 8:k * 8 + E],
                            start=(k == 0), stop=(k == 7),
                        )

                    # softmax/argmax over E=8 (free axis), tokens on partitions
                    rowmax = spool.tile([P, 1], f32, tag="rowmax")
                    nc.vector.tensor_reduce(rowmax[:], ps_l[:], axis=AX.X, op=OP.max)
                    sh = spool.tile([P, E], f32, tag="sh")
                    nc.vector.tensor_scalar(
                        out=sh[:], in0=ps_l[:], scalar1=rowmax[:], scalar2=None,
                        op0=OP.subtract,
                    )
                    ex = spool.tile([P, E], f32, tag="ex")
                    sumex = spool.tile([P, 1], f32, tag="sumex")
                    nc.scalar.activation(ex[:], sh[:], AF.Exp, accum_out=sumex[:])
                    rec = spool.tile([P, 1], f32, tag="rec")
                    nc.vector.reciprocal(rec[:], sumex[:])
                    probs = spool.tile([P, E], f32, tag="probs")
                    nc.vector.tensor_scalar(
                        out=probs[:], in0=ex[:], scalar1=rec[:], scalar2=None,
                        op0=OP.mult,
                    )
                    nc.vector.tensor_tensor(
                        out=pacc[:], in0=pacc[:], in1=probs[:], op=OP.add
                    )
                    pmax = spool.tile([P, 1], f32, tag="pmax")
                    nc.vector.tensor_reduce(pmax[:], probs[:], axis=AX.X, op=OP.max)
                    # z-loss: lse = rowmax + ln(sumex)
                    lns = spool.tile([P, 1], f32, tag="lns")
                    nc.scalar.activation(lns[:], sumex[:], AF.Ln)
                    lse = spool.tile([P, 1], f32, tag="lse")
                    nc.vector.tensor_tensor(out=lse[:], in0=rowmax[:], in1=lns[:], op=OP.add)
                    lsq = spool.tile([P, 1], f32, tag="lsq")
                    nc.vector.tensor_tensor(out=lsq[:], in0=lse[:], in1=lse[:], op=OP.mult)
                    nc.vector.tensor_tensor(out=zacc[:], in0=zacc[:], in1=lsq[:], op=OP.add)

                    # first-argmax as min over masked expert iota
                    eq = spool.tile([P, E], f32, tag="eq")
                    nc.vector.tensor_scalar(
                        out=eq[:], in0=ps_l[:], scalar1=rowmax[:], scalar2=None,
                        op0=OP.is_equal,
                    )
                    t1 = spool.tile([P, E], f32, tag="t1")
                    nc.vector.tensor_tensor(out=t1[:], in0=iota8f[:], in1=eq[:], op=OP.mult)
                    t2 = spool.tile([P, E], f32, tag="t2")
                    nc.vector.tensor_scalar(
                        out=t2[:], in0=eq[:], scalar1=-9.0, scalar2=9.0,
                        op0=OP.mult, op1=OP.add,
                    )
                    msk = spool.tile([P, E], f32, tag="msk")
                    nc.vector.tensor_tensor(out=msk[:], in0=t1[:], in1=t2[:], op=OP.add)
                    eidx = spool.tile([P, 1], f32, tag="eidx")
                    nc.vector.tensor_reduce(eidx[:], msk[:], axis=AX.X, op=OP.min)
                    oh = spool.tile([P, E], f32, tag="oh")
                    nc.vector.tensor_scalar(
                        out=oh[:], in0=iota8f[:], scalar1=eidx[:], scalar2=None,
                        op0=OP.is_equal,
                    )

                    # in-tile inclusive cumsum over tokens: U128.T @ oh
                    ps_c = psS.tile([P, E], f32, tag="ps_small")
                    nc.tensor.matmul(ps_c[:], lhsT=U128[:], rhs=oh[:], start=True, stop=True)
                    # broadcast running base to all partitions
                    ps_b = psS.tile([P, E], f32, tag="ps_small")
                    nc.tensor.matmul(ps_b[:], lhsT=ones_r[:], rhs=base[:], start=True, stop=True)
                    sel = spool.tile([P, E], f32, tag="sel")
                    nc.vector.tensor_tensor(out=sel[:], in0=ps_c[:], in1=oh[:], op=OP.mult)
                    selc = spool.tile([P, 1], f32, tag="selc")
                    nc.vector.tensor_reduce(selc[:], sel[:], axis=AX.X, op=OP.add)
                    nc.vector.tensor_tensor(out=sel[:], in0=ps_b[:], in1=oh[:], op=OP.mult)
                    selb = spool.tile([P, 1], f32, tag="selb")
                    nc.vector.tensor_reduce(selb[:], sel[:], axis=AX.X, op=OP.add)
                    rank = spool.tile([P, 1], f32, tag="rank")
                    nc.vector.tensor_tensor(out=rank[:], in0=selc[:], in1=selb[:], op=OP.add)
                    nc.vector.tensor_scalar(
                        out=rank[:], in0=rank[:], scalar1=-1.0, scalar2=None, op0=OP.add
                    )
                    # update running base with this tile's totals (column sums of oh)
                    ps_tot = psS.tile([1, E], f32, tag="ps_small", name="ps_tot")
                    nc.tensor.matmul(ps_tot[:], lhsT=ones_c[:], rhs=oh[:], start=True, stop=True)
                    nc.vector.tensor_tensor(
                        out=base[:], in0=base[:], in1=ps_tot[0:1, :], op=OP.add
                    )

                    # scatter key = eidx*W + rank  (OOB if rank >= W)
                    keyf = spool.tile([P, 1], f32, tag="keyf")
                    nc.vector.tensor_scalar(
                        out=keyf[:], in0=eidx[:], scalar1=float(W), scalar2=None, op0=OP.mult
                    )
                    nc.vector.tensor_tensor(out=keyf[:], in0=keyf[:], in1=rank[:], op=OP.add)
                    okm = spool.tile([P, 1], f32, tag="okm")
                    nc.vector.tensor_scalar(
                        out=okm[:], in0=rank[:], scalar1=float(W), scalar2=None, op0=OP.is_lt
                    )
                    t5 = spool.tile([P, 1], f32, tag="t5")
                    nc.vector.tensor_tensor(out=t5[:], in0=keyf[:], in1=okm[:], op=OP.mult)
                    t6 = spool.tile([P, 1], f32, tag="t6")
                    nc.vector.tensor_scalar(
                        out=t6[:], in0=okm[:], scalar1=-float(E * W), scalar2=float(E * W),
                        op0=OP.mult, op1=OP.add,
                    )
                    nc.vector.tensor_tensor(out=t5[:], in0=t5[:], in1=t6[:], op=OP.add)
                    keyi = spool.tile([P, 1], i32, tag="keyi")
                    nc.vector.tensor_copy(keyi[:], t5[:])

                    combo = spool.tile([P, 2], f32, tag="combo")
                    nc.vector.tensor_copy(combo[:, 0:1], pmax[:])
                    nc.vector.tensor_copy(combo[:, 1:2], tokf[:, t:t + 1])
                    nc.gpsimd.indirect_dma_start(
                        out=table2[:],
                        out_offset=bass.IndirectOffsetOnAxis(ap=keyi[:, 0:1], axis=0),
                        in_=combo[:],
                        in_offset=None,
                    )

                # ---- counts / starts / aux loss ----
                nc.sync.dma_start(out=counts_d[:], in_=base[:])
                # exclusive prefix of counts along free axis (log shifts)
                c1 = spool.tile([1, E], f32, tag="c1")
                nc.vector.tensor_copy(c1[:], base[:])
                nc.vector.tensor_tensor(out=c1[:, 1:E], in0=base[:, 1:E], in1=base[:, 0:E - 1], op=OP.add)
                c2 = spool.tile([1, E], f32, tag="c2")
                nc.vector.tensor_copy(c2[:], c1[:])
                nc.vector.tensor_tensor(out=c2[:, 2:E], in0=c1[:, 2:E], in1=c1[:, 0:E - 2], op=OP.add)
                c3 = spool.tile([1, E], f32, tag="c3")
                nc.vector.tensor_copy(c3[:], c2[:])
                nc.vector.tensor_tensor(out=c3[:, 4:E], in0=c2[:, 4:E], in1=c2[:, 0:E - 4], op=OP.add)
                excl = spool.tile([1, E], f32, tag="excl")
                nc.vector.tensor_tensor(out=excl[:], in0=c3[:], in1=base[:], op=OP.subtract)

                # aux = 0.01*E*sum((counts/N) * probs.mean(0)) + 0.001*mean(lse^2)
                ps_z = psS.tile([1, 1], f32, tag="ps_small")
                nc.tensor.matmul(ps_z[:], lhsT=zacc[:], rhs=ones_c[:], start=True, stop=True)
                ps_p = psS.tile([1, E], f32, tag="ps_small")
                nc.tensor.matmul(ps_p[:], lhsT=ones_c[:], rhs=pacc[:], start=True, stop=True)
                fp = spool.tile([1, E], f32, tag="fp")
                nc.vector.tensor_tensor(out=fp[:], in0=ps_p[:], in1=base[:], op=OP.mult)
                auxv = spool.tile([1, 1], f32, tag="auxv")
                nc.vector.tensor_reduce(auxv[:], fp[:], axis=AX.X, op=OP.add)
                nc.vector.tensor_scalar(
                    out=auxv[:], in0=auxv[:],
                    scalar1=AUX_LOSS_COEF * E / (float(N) * float(N)), scalar2=None,
                    op0=OP.mult,
                )
                zv = spool.tile([1, 1], f32, tag="zv")
                nc.vector.tensor_scalar(
                    out=zv[:], in0=ps_z[:], scalar1=Z_LOSS_COEF / float(N), scalar2=None,
                    op0=OP.mult,
                )
                nc.vector.tensor_tensor(out=auxv[:], in0=auxv[:], in1=zv[:], op=OP.add)
                nc.sync.dma_start(out=aux_d[:], in_=auxv[:])

                # broadcasts for phase B
                ps_sb2 = psS.tile([P, E], f32, tag="ps_small")
                nc.tensor.matmul(ps_sb2[:], lhsT=ones_r[:], rhs=excl[:], start=True, stop=True)
                startsBC = rpool.tile([P, E], f32, tag="startsBC")
                nc.scalar.activation(startsBC[:], ps_sb2[:], AF.Copy)
                ps_s2 = psS.tile([P, 1], f32, tag="ps_small")
                nc.tensor.matmul(ps_s2[:], lhsT=ones_r[:], rhs=ew_sb[:], start=True, stop=True)
                ewBC = rpool.tile([P, 1], f32, tag="ewBC")
                nc.scalar.activation(ewBC[:], ps_s2[:], AF.Copy)

                # ================= PHASE B1: gather this expert's tokens =================
                for c in range(8):
                    s_f = tokf[:, c:c + 1]  # slot ids c*128+p
                    k1f = spool.tile([P, 1], f32, tag="k1f")
                    nc.vector.tensor_tensor(out=k1f[:], in0=ewBC[:], in1=s_f, op=OP.add)
                    nc.vector.tensor_scalar(
                        out=k1f[:], in0=k1f[:], scalar1=float(E * W - 1), scalar2=None,
                        op0=OP.min,
                    )
                    k1i = spool.tile([P, 1], i32, tag="k1i")
                    nc.vector.tensor_copy(k1i[:], k1f[:])
                    g1 = spool.tile([P, 2], f32, tag="g1")
                    nc.gpsimd.indirect_dma_start(
                        out=g1[:], out_offset=None,
                        in_=table2[:],
                        in_offset=bass.IndirectOffsetOnAxis(ap=k1i[:, 0:1], axis=0),
                    )
                    ti = spool.tile([P, 1], i32, tag="ti")
                    nc.vector.tensor_copy(ti[:], g1[:, 1:2])
                    nc.sync.dma_start(out=glist_d[c * P:(c + 1) * P, :], in_=ti[:])
                    xg = xpool.tile([P, D], f32, tag="xg")
                    nc.gpsimd.indirect_dma_start(
                        out=xg[:], out_offset=None,
                        in_=x_pad[:],
                        in_offset=bass.IndirectOffsetOnAxis(ap=ti[:, 0:1], axis=0),
                    )
                    # m lookup: sorted position p = token id -> bucket -> table2 col0
                    pf = spool.tile([P, 1], f32, tag="pf")
                    nc.vector.tensor_copy(pf[:], g1[:, 1:2])
                    cmp = spool.tile([P, E], f32, tag="cmp")
                    nc.vector.tensor_tensor(
                        out=cmp[:], in0=pf[:].to_broadcast([P, E]), in1=startsBC[:],
                        op=OP.is_ge,
                    )
                    ehat = spool.tile([P, 1], f32, tag="ehat")
                    nc.vector.tensor_reduce(ehat[:], cmp[:], axis=AX.X, op=OP.add)
                    nc.vector.tensor_scalar(
                        out=ehat[:], in0=ehat[:], scalar1=-1.0, scalar2=None, op0=OP.add
                    )
                    oh8 = spool.tile([P, E], f32, tag="oh8")
                    nc.vector.tensor_scalar(
                        out=oh8[:], in0=iota8f[:], scalar1=ehat[:], scalar2=None,
                        op0=OP.is_equal,
                    )
                    sts = spool.tile([P, E], f32, tag="sts")
                    nc.vector.tensor_tensor(out=sts[:], in0=startsBC[:], in1=oh8[:], op=OP.mult)
                    stsel = spool.tile([P, 1], f32, tag="stsel")
                    nc.vector.tensor_reduce(stsel[:], sts[:], axis=AX.X, op=OP.add)
                    rr = spool.tile([P, 1], f32, tag="rr")
                    nc.vector.tensor_tensor(out=rr[:], in0=pf[:], in1=stsel[:], op=OP.subtract)
                    k2f = spool.tile([P, 1], f32, tag="k2f")
                    nc.vector.tensor_scalar(
                        out=k2f[:], in0=ehat[:], scalar1=float(W), scalar2=None, op0=OP.mult
                    )
                    nc.vector.tensor_tensor(out=k2f[:], in0=k2f[:], in1=rr[:], op=OP.add)
                    nc.vector.tensor_scalar(
                        out=k2f[:], in0=k2f[:], scalar1=float(E * W + 1), scalar2=None,
                        op0=OP.min,
                    )
                    k2i = spool.tile([P, 1], i32, tag="k2i")
                    nc.vector.tensor_copy(k2i[:], k2f[:])
                    g2 = spool.tile([P, 2], f32, tag="g2")
                    nc.gpsimd.indirect_dma_start(
                        out=g2[:], out_offset=None,
                        in_=table2[:],
                        in_offset=bass.IndirectOffsetOnAxis(ap=k2i[:, 0:1], axis=0),
                    )
                    nc.vector.tensor_copy(m_all[:, c:c + 1], g2[:, 0:1])

                    # transpose gathered rows into bf16 xT strips
                    for k in range(8):
                        pst = psT.tile([P, P], f32)
                        nc.tensor.transpose(pst[:], xg[:, k * P:(k + 1) * P], ident[:])
                        nc.scalar.activation(
                            xT_bf[k][:, c * P:(c + 1) * P], pst[:], AF.Copy
                        )

                # ================= PHASE B2: mm1 + gelu =================
                # hT layout: [128 h-part, (j, n) free] with j in 32 h-tiles, n in 2
                # token halves of 512: col block j*1024 + n*512
                for j in range(H // P):
                    for n in range(2):
                        psm = psM1.tile([P, 512], f32, tag="ps_mm1")
                        for k in range(8):
                            nc.tensor.matmul(
                                psm[:],
                                lhsT=w1_sb[k][:, j * P:(j + 1) * P],
                                rhs=xT_bf[k][:, n * 512:(n + 1) * 512],
                                start=(k == 0), stop=(k == 7),
                            )
                        nc.scalar.activation(
                            hT[:, j * CAP + n * 512: j * CAP + (n + 1) * 512],
                            psm[:], AF.Gelu, bias=b1_sb[:, j:j + 1],
                        )

            # ================= PHASE B3: mm2 (+b2, *m) =================
            with (
                tc.tile_pool(name="psM2", bufs=1, space="PSUM") as psM2,
                tc.tile_pool(name="w2p", bufs=3) as w2pool,
                tc.tile_pool(name="outp", bufs=3) as opool,
            ):
                for half in range(2):
                    psos = []
                    for tb in range(4):
                        for db in range(2):
                            psos.append(psM2.tile([P, 512], f32, tag=f"ps_o{tb}{db}", name=f"ps_o{tb}{db}"))
                    for k2 in range(H // P):
                        w2s = w2pool.tile([P, D], bf16, tag="w2s")
                        nc.sync.dma_start(out=w2s[:], in_=w2_d[k2 * P:(k2 + 1) * P, :])
                        for tb in range(4):
                            tokblk = half * 4 + tb
                            for db in range(2):
                                nc.tensor.matmul(
                                    psos[tb * 2 + db][:],
                                    lhsT=hT[:, k2 * CAP + tokblk * P:
                                            k2 * CAP + (tokblk + 1) * P],
                                    rhs=w2s[:, db * 512:(db + 1) * 512],
                                    start=(k2 == 0), stop=False,
                                )
                    # b2 add closes the accumulation group
                    for tb in range(4):
                        tokblk = half * 4 + tb
                        for db in range(2):
                            nc.tensor.matmul(
                                psos[tb * 2 + db][:],
                                lhsT=ones_r[:],
                                rhs=b2_sb[:, db * 512:(db + 1) * 512],
                                start=False, stop=True,
                            )
                        o_sb = opool.tile([P, D], f32, tag="o_sb")
                        for db in range(2):
                            nc.vector.tensor_scalar(
                                out=o_sb[:, db * 512:(db + 1) * 512],
                                in0=psos[tb * 2 + db][:],
                                scalar1=m_all[:, tokblk:tokblk + 1], scalar2=None,
                                op0=OP.mult,
                            )
                        nc.sync.dma_start(
                            out=o_d[tokblk * P:(tokblk + 1) * P, :], in_=o_sb[:]
                        )
    nc.finalize()
    return nc


def _get_nc():
    global _nc_cache
    if _nc_cache is None:
        _nc_cache = build_nc()
    return _nc_cache


def run(x, Wg, W1, b1, W2, b2, trace=False):
    from concourse.bass_utils import run_bass_kernel_spmd

    nc = _get_nc()
    xf = np.ascontiguousarray(np.asarray(x, np.float32).reshape(N, D))
    x_pad = np.concatenate([xf, np.zeros((1, D), np.float32)], 0)
    Wg = np.ascontiguousarray(np.asarray(Wg, np.float32))
    W1b = np.asarray(W1, np.float32).astype(ml_dtypes.bfloat16)
    W2b = np.asarray(W2, np.float32).astype(ml_dtypes.bfloat16)
    b1f = np.asarray(b1, np.float32)
    b2f = np.asarray(b2, np.float32)

    in_maps = []
    for e in range(E):
        in_maps.append({
            "x": x_pad,
            "wg": Wg,
            "w1": np.ascontiguousarray(W1b[e]),
            "b1": np.ascontiguousarray(b1f[e].reshape(H, 1)),
            "w2": np.ascontiguousarray(W2b[e]),
            "b2": np.ascontiguousarray(b2f[e].reshape(D, 1)),
            "ew": np.full((1, 1), float(e * W), np.float32),
        })
    res = run_bass_kernel_spmd(nc, in_maps, core_ids=list(range(E)), trace=trace)

    out = np.zeros((N, D), np.float32)
    for e in range(E):
        r = res.results[e]
        cnt = min(int(round(float(r["counts"][0, e]))), CAP)
        idx = r["glist"][:cnt, 0].astype(np.int64)
        out[idx] = r["o"][:cnt]
    aux = np.float32(res.results[0]["aux"][0, 0])
    return (out.reshape(B, T, D), aux), res


def kernel(x, Wg, W1, b1, W2, b2):
    (out, aux), _ = run(x, Wg, W1, b1, W2, b2, trace=False)
    return out, aux


# revision 25
# speedup vs baseline: 1.0957x; 1.0957x over previous
"""MoE (top-1 routing, E=8 experts) Trainium2 Bass kernel.

Sharding: expert-parallel over 8 NeuronCores. Every core runs a replicated
fp32 router over all N=8192 tokens (reads host-pretransposed xT), builds the
global dispatch table via counting-sort (one triangular-matmul cumsum per
4-tile block + indirect DMA scatter), then computes ONLY its own expert's
capacity-limited MLP in bf16 (fp32 accumulate). Host combine is a pure
index-based scatter of the per-expert compact outputs using device-computed
token indices.

Self-contained: hardcodes shapes from the problem spec.
"""

import numpy as np
import ml_dtypes

B, T, D, H, E = 4, 2048, 1024, 4096, 8
N = B * T            # 8192 tokens
CAP = 1024           # capacity = max(4, ceil(N/E))
W = 2048             # table bucket width (max supported tokens/expert)
P = 128
NT = N // P          # 64 token tiles
NB = NT // 4         # 16 blocks of 4 tiles (512 tokens)
AUX_LOSS_COEF = 0.01
Z_LOSS_COEF = 0.001

_nc_cache = None


def build_nc():
    import concourse.bacc as bacc
    import concourse.tile as tile
    import concourse.bass as bass
    import concourse.mybir as mybir
    from concourse.masks import make_identity, make_upper_triangular

    f32 = mybir.dt.float32
    bf16 = mybir.dt.bfloat16
    i32 = mybir.dt.int32
    AF = mybir.ActivationFunctionType
    OP = mybir.AluOpType
    AX = mybir.AxisListType

    nc = bacc.Bacc(None, target_bir_lowering=False, num_swdge_queues=2)

    # ---- I/O ----
    x_pad = nc.dram_tensor("x", [N + 1, D], f32, kind="ExternalInput")
    xT_d = nc.dram_tensor("xt", [D, N], f32, kind="ExternalInput")
    wg_d = nc.dram_tensor("wg", [D, E], f32, kind="ExternalInput")
    w1_d = nc.dram_tensor("w1", [D, H], bf16, kind="ExternalInput")
    b1_d = nc.dram_tensor("b1", [H, 1], f32, kind="ExternalInput")
    w2_d = nc.dram_tensor("w2", [H, D], bf16, kind="ExternalInput")
    b2_d = nc.dram_tensor("b2", [D, 1], f32, kind="ExternalInput")
    ew_d = nc.dram_tensor("ew", [1, 1], f32, kind="ExternalInput")     # e*W

    o_d = nc.dram_tensor("o", [CAP, D], f32, kind="ExternalOutput")
    glist_d = nc.dram_tensor("glist", [CAP, 1], i32, kind="ExternalOutput")
    counts_d = nc.dram_tensor("counts", [1, E], f32, kind="ExternalOutput")
    aux_d = nc.dram_tensor("aux", [1, 1], f32, kind="ExternalOutput")

    table2 = nc.dram_tensor("table2", [E * W + 2, 2], f32, kind="Internal")

    with tile.TileContext(nc) as tc:
        with (
            tc.tile_pool(name="const", bufs=1) as cpool,
            tc.tile_pool(name="resident", bufs=1) as rpool,
            tc.tile_pool(name="xin", bufs=2) as xpool,
            tc.tile_pool(name="small", bufs=4) as spool,
        ):
            # ---------------- constants ----------------
            ident = cpool.tile([P, P], f32)
            make_identity(nc, ident[:])
            ident8 = cpool.tile([8, 8], f32)
            make_identity(nc, ident8[:])
            U128 = cpool.tile([P, P], f32)
            make_upper_triangular(nc, U128[:], val=1.0, diag=True)
            ones_r = cpool.tile([1, P], f32)   # row of ones (K=1 bcast matmuls)
            nc.gpsimd.memset(ones_r[:], 1.0)
            ones_c = cpool.tile([P, 1], f32)   # column of ones (partition reduce)
            nc.gpsimd.memset(ones_c[:], 1.0)
            iota8_i = cpool.tile([P, E], i32)
            nc.gpsimd.iota(iota8_i[:], pattern=[[1, E]], base=0, channel_multiplier=0)
            iota8f = cpool.tile([P, E], f32)
            nc.vector.tensor_copy(iota8f[:], iota8_i[:])
            tok_i = cpool.tile([P, NT], i32)   # tok_i[p, t] = t*128 + p
            nc.gpsimd.iota(tok_i[:], pattern=[[P, NT]], base=0, channel_multiplier=1)
            tokf = cpool.tile([P, NT], f32)
            nc.vector.tensor_copy(tokf[:], tok_i[:])

            wg_sb = cpool.tile([P, E * 8], f32)   # wg strips: col k*8+e
            nc.sync.dma_start(
                out=wg_sb[:].rearrange("p (k e) -> p k e", e=E),
                in_=wg_d[:].rearrange("(k p) e -> p k e", p=P),
            )
            b1_sb = cpool.tile([P, H // P], f32)  # b1[j*128+p] -> [p, j]
            nc.sync.dma_start(
                out=b1_sb[:].rearrange("p j -> p j ()"),
                in_=b1_d[:].rearrange("(j p) o -> p j o", p=P),
            )
            b2_sb = cpool.tile([1, D], f32)
            nc.sync.dma_start(out=b2_sb[:], in_=b2_d[:].rearrange("d o -> o d"))
            ew_sb = cpool.tile([1, 1], f32)
            nc.sync.dma_start(out=ew_sb[:], in_=ew_d[:])

            # w1 strips resident (8 MB bf16), no deps -> loads early
            w1_sb = [rpool.tile([P, H], bf16, tag=f"w1_{k}", name=f"w1_{k}") for k in range(8)]
            for k in range(8):
                nc.sync.dma_start(out=w1_sb[k][:], in_=w1_d[k * P:(k + 1) * P, :])

            # accumulators
            pacc4 = rpool.tile([P, 4 * E], f32, tag="pacc4")
            nc.gpsimd.memset(pacc4[:], 0.0)
            base = rpool.tile([1, E], f32, tag="base")
            nc.gpsimd.memset(base[:], 0.0)
            rowmax_all = rpool.tile([P, NT], f32, tag="rowmax_all")
            sumex_all = rpool.tile([P, NT], f32, tag="sumex_all")
            oh_all = rpool.tile([P, NT * E], f32, tag="oh_all")
            pmax_all = rpool.tile([P, NT], f32, tag="pmax_all")
            eidx_all = rpool.tile([P, NT], f32, tag="eidx_all")

            # table2 init to sentinel 8192.0 (one DMA from an SBUF tile)
            sent = rpool.tile([P, E * W * 2 // P], f32, tag="sent")
            nc.gpsimd.memset(sent[:], float(N))
            nc.sync.dma_start(
                out=table2[0:E * W, :].rearrange("(a r) c -> a (r c)", a=P),
                in_=sent[:],
            )
            nc.sync.dma_start(out=table2[E * W:E * W + 2, :], in_=sent[0:2, 0:2])

            # resident MLP buffers
            xT_bf = [rpool.tile([P, CAP], bf16, tag=f"xT_{k}", name=f"xT_{k}") for k in range(8)]
            # hT layout: [128 h-partitions, (j, tok) free]: col j*CAP + tok
            hT = rpool.tile([P, (H // P) * CAP], bf16, tag="hT")
            m_all = rpool.tile([P, 8], f32, tag="m_all")
            startsBC = rpool.tile([P, E], f32, tag="startsBC")
            ewBC = rpool.tile([P, 1], f32, tag="ewBC")

            # ================= PHASE A: replicated router =================
            with (
                tc.tile_pool(name="psLT", bufs=2, space="PSUM") as psLT,
                tc.tile_pool(name="psL", bufs=4, space="PSUM") as psL,
                tc.tile_pool(name="psS", bufs=2, space="PSUM") as psS,
            ):
                for b in range(NB):
                    # load xT strip-block [1024 d, 512 tok] as 8 strips
                    xs = []
                    for k in range(8):
                        s = xpool.tile([P, 512], f32, tag="xs", name=f"xs_{b}_{k}")
                        nc.sync.dma_start(
                            out=s[:], in_=xT_d[k * P:(k + 1) * P, b * 512:(b + 1) * 512]
                        )
                        xs.append(s)
                    # logitsT [8, 512] accumulated over k (wg stationary)
                    ps_lt = psLT.tile([8, 512], f32, tag="ps_lt")
                    for k in range(8):
                        nc.tensor.matmul(
                            ps_lt[:], lhsT=wg_sb[:, k * 8:k * 8 + E], rhs=xs[k][:],
                            start=(k == 0), stop=(k == 7),
                        )
                    lt_sb = spool.tile([8, 512], f32, tag="lt_sb")
                    nc.vector.tensor_copy(lt_sb[:], ps_lt[:])
                    # transpose back into [128 tok, (t,8)] layout
                    logits4 = spool.tile([P, 4 * E], f32, tag="logits4")
                    for t in range(4):
                        ps_l = psL.tile([P, E], f32, tag="ps_l")
                        nc.tensor.transpose(ps_l[:], lt_sb[:, t * P:(t + 1) * P], ident8[:])
                        nc.vector.tensor_copy(logits4[:, t * E:(t + 1) * E], ps_l[:])

                    l3 = logits4[:].rearrange("p (t e) -> p t e", e=E)
                    rm4 = rowmax_all[:, b * 4:(b + 1) * 4]
                    nc.vector.tensor_reduce(rm4, l3, axis=AX.X, op=OP.max)
                    rm4b = rowmax_all[:].rearrange("p t -> p t ()")[
                        :, b * 4:(b + 1) * 4, :
                    ].to_broadcast([P, 4, E])
                    sh4 = spool.tile([P, 4 * E], f32, tag="sh4")
                    sh43 = sh4[:].rearrange("p (t e) -> p t e", e=E)
                    nc.vector.tensor_tensor(out=sh43, in0=l3, in1=rm4b, op=OP.subtract)
                    ex4 = spool.tile([P, 4 * E], f32, tag="ex4")
                    nc.scalar.activation(ex4[:], sh4[:], AF.Exp)
                    ex43 = ex4[:].rearrange("p (t e) -> p t e", e=E)
                    se4 = sumex_all[:, b * 4:(b + 1) * 4]
                    nc.vector.tensor_reduce(se4, ex43, axis=AX.X, op=OP.add)
                    rec4 = spool.tile([P, 4], f32, tag="rec4")
                    nc.vector.reciprocal(rec4[:], se4)
                    rec4b = rec4[:].rearrange("p t -> p t ()").to_broadcast([P, 4, E])
                    probs4 = spool.tile([P, 4 * E], f32, tag="probs4")
                    p43 = probs4[:].rearrange("p (t e) -> p t e", e=E)
                    nc.vector.tensor_tensor(out=p43, in0=ex43, in1=rec4b, op=OP.mult)
                    nc.vector.tensor_tensor(out=pacc4[:], in0=pacc4[:], in1=probs4[:], op=OP.add)
                    nc.vector.tensor_reduce(pmax_all[:, b * 4:(b + 1) * 4], p43, axis=AX.X, op=OP.max)

                    # first-argmax per tile: min over masked expert iota
                    iota48 = iota8f[:].rearrange("p e -> p () e").to_broadcast([P, 4, E])
                    eq4 = spool.tile([P, 4 * E], f32, tag="eq4")
                    eq43 = eq4[:].rearrange("p (t e) -> p t e", e=E)
                    nc.vector.tensor_tensor(out=eq43, in0=l3, in1=rm4b, op=OP.is_equal)
                    m14 = spool.tile([P, 4 * E], f32, tag="m14")
                    m143 = m14[:].rearrange("p (t e) -> p t e", e=E)
                    nc.vector.tensor_tensor(out=m143, in0=iota48, in1=eq43, op=OP.mult)
                    m24 = spool.tile([P, 4 * E], f32, tag="m24")
                    nc.vector.tensor_scalar(
                        out=m24[:], in0=eq4[:], scalar1=-9.0, scalar2=9.0,
                        op0=OP.mult, op1=OP.add,
                    )
                    nc.vector.tensor_tensor(out=m14[:], in0=m14[:], in1=m24[:], op=OP.add)
                    eidx4 = eidx_all[:, b * 4:(b + 1) * 4]
                    nc.vector.tensor_reduce(eidx4, m143, axis=AX.X, op=OP.min)
                    eidx4b = eidx_all[:].rearrange("p t -> p t ()")[
                        :, b * 4:(b + 1) * 4, :
                    ].to_broadcast([P, 4, E])
                    oh43 = oh_all[:].rearrange("p (t e) -> p t e", e=E)[
                        :, b * 4:(b + 1) * 4, :
                    ]
                    nc.vector.tensor_tensor(out=oh43, in0=iota48, in1=eidx4b, op=OP.is_equal)

                    # pmax into resident store (for scatter payload later)
                    # (pmax4 already written via pmax_all slice above)

                    if b % (NB // 4) == (NB // 4) - 1:
                        hb = b // (NB // 4)          # which quarter just finished
                        hw = NT // 4                 # 16 tiles per quarter
                        ht0 = hb * hw                # first tile of half
                        c0 = ht0 * E                 # first oh column
                        cw = hw * E                  # 256 columns
                        # tile totals for this half: [1, (t,e)]
                        ps_tt = psS.tile([1, cw], f32, tag="ps_small", name=f"ps_tt{hb}")
                        nc.tensor.matmul(
                            ps_tt[:], lhsT=ones_c[:], rhs=oh_all[:, c0:c0 + cw],
                            start=True, stop=True,
                        )
                        tots = spool.tile([1, cw], f32, tag="tots", name=f"tots{hb}", bufs=2)
                        nc.vector.tensor_copy(tots[:], ps_tt[:])
                        # inclusive prefix over tiles (shift-adds), then exclusive
                        for sh in [1, 2, 4, 8]:
                            nc.vector.tensor_tensor(
                                out=tots[:, sh * E:cw], in0=tots[:, sh * E:cw],
                                in1=tots[:, 0:cw - sh * E], op=OP.add,
                            )
                        # exclusive prefix = inclusive shifted right one tile
                        exclp = spool.tile([1, cw], f32, tag="exclp", name=f"exclp{hb}", bufs=2)
                        nc.vector.memset(exclp[:, 0:E], 0.0)
                        nc.vector.tensor_copy(exclp[:, E:cw], tots[:, 0:cw - E])
                        # baseb[t] = carry base (prev halves) + exclusive prefix
                        baseb = spool.tile([1, cw], f32, tag="baseb", name=f"baseb{hb}", bufs=2)
                        bb3 = baseb[:].rearrange("o (t e) -> o t e", e=E)
                        nc.vector.tensor_tensor(
                            out=bb3,
                            in0=exclp[:].rearrange("o (t e) -> o t e", e=E),
                            in1=base[:].rearrange("o e -> o () e").to_broadcast([1, hw, E]),
                            op=OP.add,
                        )
                        # update global base with this half's grand total (last inclusive)
                        nc.vector.tensor_tensor(
                            out=base[:], in0=base[:], in1=tots[:, cw - E:cw], op=OP.add,
                        )
                        # rank psum = per-tile cumsum + base broadcast (one accum group)
                        ps_rk = psLT.tile([P, cw], f32, tag="ps_lt", name=f"ps_rk{hb}")
                        nc.tensor.matmul(
                            ps_rk[:], lhsT=U128[:], rhs=oh_all[:, c0:c0 + cw],
                            start=True, stop=False,
                        )
                        nc.tensor.matmul(
                            ps_rk[:], lhsT=ones_r[:], rhs=baseb[:],
                            start=False, stop=True,
                        )
                        sel = spool.tile([P, cw], f32, tag="selh", name=f"selh{hb}", bufs=2)
                        nc.vector.tensor_tensor(
                            out=sel[:], in0=ps_rk[:], in1=oh_all[:, c0:c0 + cw], op=OP.mult,
                        )
                        rankh = spool.tile([P, hw], f32, tag="rankh", name=f"rankh{hb}", bufs=2)
                        nc.vector.tensor_reduce(
                            rankh[:], sel[:].rearrange("p (t e) -> p t e", e=E),
                            axis=AX.X, op=OP.add,
                        )
                        nc.vector.tensor_scalar(
                            out=rankh[:], in0=rankh[:], scalar1=-1.0, scalar2=None, op0=OP.add
                        )
                        # keys = eidx*W + rank, clamped to trash row if rank >= W
                        keyh = spool.tile([P, hw], f32, tag="keyh", name=f"keyh{hb}", bufs=2)
                        nc.vector.tensor_scalar(
                            out=keyh[:], in0=eidx_all[:, ht0:ht0 + hw],
                            scalar1=float(W), scalar2=None, op0=OP.mult,
                        )
                        nc.vector.tensor_tensor(out=keyh[:], in0=keyh[:], in1=rankh[:], op=OP.add)
                        okh = spool.tile([P, hw], f32, tag="okh", name=f"okh{hb}", bufs=2)
                        nc.vector.tensor_scalar(
                            out=okh[:], in0=rankh[:], scalar1=float(W), scalar2=None, op0=OP.is_lt
                        )
                        nc.vector.tensor_tensor(out=keyh[:], in0=keyh[:], in1=okh[:], op=OP.mult)
                        nc.vector.tensor_scalar(
                            out=okh[:], in0=okh[:], scalar1=-float(E * W), scalar2=float(E * W),
                            op0=OP.mult, op1=OP.add,
                        )
                        nc.vector.tensor_tensor(out=keyh[:], in0=keyh[:], in1=okh[:], op=OP.add)
                        keyih = spool.tile([P, hw], i32, tag="keyih", name=f"keyih{hb}", bufs=2)
                        nc.vector.tensor_copy(keyih[:], keyh[:])
                        comboh = spool.tile([P, hw * 2], f32, tag="comboh", name=f"comboh{hb}", bufs=2)
                        ch3 = comboh[:].rearrange("p (t c) -> p t c", c=2)
                        nc.vector.tensor_copy(
                            ch3[:, :, 0:1],
                            pmax_all[:, ht0:ht0 + hw].rearrange("p t -> p t ()"),
                        )
                        nc.vector.tensor_copy(
                            ch3[:, :, 1:2],
                            tokf[:, ht0:ht0 + hw].rearrange("p t -> p t ()"),
                        )
                        for tt in range(hw):
                            nc.gpsimd.indirect_dma_start(
                                out=table2[:],
                                out_offset=bass.IndirectOffsetOnAxis(
                                    ap=keyih[:, tt:tt + 1], axis=0),
                                in_=comboh[:, 2 * tt:2 * tt + 2],
                                in_offset=None,
                            )

                # ---- counts / starts / z-loss tail / aux ----
                nc.sync.dma_start(out=counts_d[:], in_=base[:])
                c1 = spool.tile([1, E], f32, tag="c1")
                nc.vector.tensor_copy(c1[:], base[:])
                nc.vector.tensor_tensor(out=c1[:, 1:E], in0=base[:, 1:E], in1=base[:, 0:E - 1], op=OP.add)
                c2 = spool.tile([1, E], f32, tag="c2")
                nc.vector.tensor_copy(c2[:], c1[:])
                nc.vector.tensor_tensor(out=c2[:, 2:E], in0=c1[:, 2:E], in1=c1[:, 0:E - 2], op=OP.add)
                c3 = spool.tile([1, E], f32, tag="c3")
                nc.vector.tensor_copy(c3[:], c2[:])
                nc.vector.tensor_tensor(out=c3[:, 4:E], in0=c2[:, 4:E], in1=c2[:, 0:E - 4], op=OP.add)
                excl = spool.tile([1, E], f32, tag="excl")
                nc.vector.tensor_tensor(out=excl[:], in0=c3[:], in1=base[:], op=OP.subtract)

                # z-loss: lse = rowmax + ln(sumex), batched over all 64 tiles
                lns = spool.tile([P, NT], f32, tag="lns")
                nc.scalar.activation(lns[:], sumex_all[:], AF.Ln)
                nc.vector.tensor_tensor(out=lns[:], in0=lns[:], in1=rowmax_all[:], op=OP.add)
                nc.vector.tensor_tensor(out=lns[:], in0=lns[:], in1=lns[:], op=OP.mult)
                zrow = spool.tile([P, 1], f32, tag="zrow")
                nc.vector.tensor_reduce(zrow[:], lns[:], axis=AX.X, op=OP.add)
                # fold pacc4 -> [P, E]
                pacc = spool.tile([P, E], f32, tag="pacc")
                nc.vector.tensor_tensor(out=pacc[:], in0=pacc4[:, 0:E], in1=pacc4[:, E:2 * E], op=OP.add)
                nc.vector.tensor_tensor(out=pacc[:], in0=pacc[:], in1=pacc4[:, 2 * E:3 * E], op=OP.add)
                nc.vector.tensor_tensor(out=pacc[:], in0=pacc[:], in1=pacc4[:, 3 * E:4 * E], op=OP.add)

                ps_z = psS.tile([1, 1], f32, tag="ps_small", name="ps_z")
                nc.tensor.matmul(ps_z[:], lhsT=zrow[:], rhs=ones_c[:], start=True, stop=True)
                ps_p = psS.tile([1, E], f32, tag="ps_small", name="ps_p")
                nc.tensor.matmul(ps_p[:], lhsT=ones_c[:], rhs=pacc[:], start=True, stop=True)
                fp = spool.tile([1, E], f32, tag="fp")
                nc.vector.tensor_tensor(out=fp[:], in0=ps_p[:], in1=base[:], op=OP.mult)
                auxv = spool.tile([1, 1], f32, tag="auxv")
                nc.vector.tensor_reduce(auxv[:], fp[:], axis=AX.X, op=OP.add)
                nc.vector.tensor_scalar(
                    out=auxv[:], in0=auxv[:],
                    scalar1=AUX_LOSS_COEF * E / (float(N) * float(N)), scalar2=None,
                    op0=OP.mult,
                )
                zv = spool.tile([1, 1], f32, tag="zv")
                nc.vector.tensor_scalar(
                    out=zv[:], in0=ps_z[:], scalar1=Z_LOSS_COEF / float(N), scalar2=None,
                    op0=OP.mult,
                )
                nc.vector.tensor_tensor(out=auxv[:], in0=auxv[:], in1=zv[:], op=OP.add)
                nc.sync.dma_start(out=aux_d[:], in_=auxv[:])

                # broadcasts for phase B
                ps_sb2 = psS.tile([P, E], f32, tag="ps_small", name="ps_sb2")
                nc.tensor.matmul(ps_sb2[:], lhsT=ones_r[:], rhs=excl[:], start=True, stop=True)
                nc.vector.tensor_copy(startsBC[:], ps_sb2[:])
                ps_s2 = psS.tile([P, 1], f32, tag="ps_small", name="ps_s2")
                nc.tensor.matmul(ps_s2[:], lhsT=ones_r[:], rhs=ew_sb[:], start=True, stop=True)
                nc.vector.tensor_copy(ewBC[:], ps_s2[:])

            # ================= PHASE B1 + mm1 =================
            with (
                tc.tile_pool(name="psT", bufs=2, space="PSUM") as psT,
                tc.tile_pool(name="psM1", bufs=4, space="PSUM") as psM1,
            ):
                for c in range(8):
                    s_f = tokf[:, c:c + 1]  # slot ids c*128+p
                    k1f = spool.tile([P, 1], f32, tag="k1f")
                    nc.vector.tensor_tensor(out=k1f[:], in0=ewBC[:], in1=s_f, op=OP.add)
                    nc.vector.tensor_scalar(
                        out=k1f[:], in0=k1f[:], scalar1=float(E * W - 1), scalar2=None,
                        op0=OP.min,
                    )
                    k1i = spool.tile([P, 1], i32, tag="k1i")
                    nc.vector.tensor_copy(k1i[:], k1f[:])
                    g1 = spool.tile([P, 2], f32, tag="g1")
                    nc.gpsimd.indirect_dma_start(
                        out=g1[:], out_offset=None,
                        in_=table2[:],
                        in_offset=bass.IndirectOffsetOnAxis(ap=k1i[:, 0:1], axis=0),
                    )
                    ti = spool.tile([P, 1], i32, tag="ti")
                    nc.vector.tensor_copy(ti[:], g1[:, 1:2])
                    nc.sync.dma_start(out=glist_d[c * P:(c + 1) * P, :], in_=ti[:])
                    xg = xpool.tile([P, D], f32, tag="xg")
                    nc.gpsimd.indirect_dma_start(
                        out=xg[:], out_offset=None,
                        in_=x_pad[:],
                        in_offset=bass.IndirectOffsetOnAxis(ap=ti[:, 0:1], axis=0),
                    )
                    # m lookup: sorted position p = token id -> bucket -> table2 col0
                    pf = spool.tile([P, 1], f32, tag="pf")
                    nc.vector.tensor_copy(pf[:], g1[:, 1:2])
                    cmp = spool.tile([P, E], f32, tag="cmp")
                    nc.vector.tensor_tensor(
                        out=cmp[:], in0=pf[:].to_broadcast([P, E]), in1=startsBC[:],
                        op=OP.is_ge,
                    )
                    ehat = spool.tile([P, 1], f32, tag="ehat")
                    nc.vector.tensor_reduce(ehat[:], cmp[:], axis=AX.X, op=OP.add)
                    nc.vector.tensor_scalar(
                        out=ehat[:], in0=ehat[:], scalar1=-1.0, scalar2=None, op0=OP.add
                    )
                    oh8 = spool.tile([P, E], f32, tag="oh8")
                    nc.vector.tensor_scalar(
                        out=oh8[:], in0=iota8f[:], scalar1=ehat[:], scalar2=None,
                        op0=OP.is_equal,
                    )
                    sts = spool.tile([P, E], f32, tag="sts")
                    nc.vector.tensor_tensor(out=sts[:], in0=startsBC[:], in1=oh8[:], op=OP.mult)
                    stsel = spool.tile([P, 1], f32, tag="stsel")
                    nc.vector.tensor_reduce(stsel[:], sts[:], axis=AX.X, op=OP.add)
                    rr = spool.tile([P, 1], f32, tag="rr")
                    nc.vector.tensor_tensor(out=rr[:], in0=pf[:], in1=stsel[:], op=OP.subtract)
                    k2f = spool.tile([P, 1], f32, tag="k2f")
                    nc.vector.tensor_scalar(
                        out=k2f[:], in0=ehat[:], scalar1=float(W), scalar2=None, op0=OP.mult
                    )
                    nc.vector.tensor_tensor(out=k2f[:], in0=k2f[:], in1=rr[:], op=OP.add)
                    nc.vector.tensor_scalar(
                        out=k2f[:], in0=k2f[:], scalar1=float(E * W + 1), scalar2=None,
                        op0=OP.min,
                    )
                    k2i = spool.tile([P, 1], i32, tag="k2i")
                    nc.vector.tensor_copy(k2i[:], k2f[:])
                    g2 = spool.tile([P, 2], f32, tag="g2")
                    nc.gpsimd.indirect_dma_start(
                        out=g2[:], out_offset=None,
                        in_=table2[:],
                        in_offset=bass.IndirectOffsetOnAxis(ap=k2i[:, 0:1], axis=0),
                    )
                    nc.vector.tensor_copy(m_all[:, c:c + 1], g2[:, 0:1])

                    # transpose gathered rows into bf16 xT strips
                    for k in range(8):
                        pst = psT.tile([P, P], f32)
                        nc.tensor.transpose(pst[:], xg[:, k * P:(k + 1) * P], ident[:])
                        nc.vector.tensor_copy(xT_bf[k][:, c * P:(c + 1) * P], pst[:])

                # ---- mm1 + gelu: hT[j] = gelu(x @ W1 + b1), stationary reused over n
                for n in range(2):
                    for j in range(H // P):
                        psm = psM1.tile([P, 512], f32, tag="ps_mm1", name=f"psm_{j}_{n}")
                        for k in range(8):
                            nc.tensor.matmul(
                                psm[:],
                                lhsT=w1_sb[k][:, j * P:(j + 1) * P],
                                rhs=xT_bf[k][:, n * 512:(n + 1) * 512],
                                start=(k == 0), stop=(k == 7),
                            )
                        nc.scalar.activation(
                            hT[:, j * CAP + n * 512: j * CAP + (n + 1) * 512],
                            psm[:], AF.Gelu, bias=b1_sb[:, j:j + 1],
                        )

            # ================= PHASE B3: mm2 (+b2, *m) =================
            with (
                tc.tile_pool(name="psM2", bufs=1, space="PSUM") as psM2,
                tc.tile_pool(name="w2p", bufs=3) as w2pool,
                tc.tile_pool(name="outp", bufs=2) as opool,
            ):
                for half in range(2):
                    psos = []
                    for tb in range(4):
                        for db in range(2):
                            psos.append(psM2.tile([P, 512], f32, tag=f"ps_o{tb}{db}", name=f"ps_o{half}{tb}{db}"))
                    for k2 in range(H // P):
                        w2s = w2pool.tile([P, D], bf16, tag="w2s")
                        nc.sync.dma_start(out=w2s[:], in_=w2_d[k2 * P:(k2 + 1) * P, :])
                        for tb in range(4):
                            tokblk = half * 4 + tb
                            for db in range(2):
                                nc.tensor.matmul(
                                    psos[tb * 2 + db][:],
                                    lhsT=hT[:, k2 * CAP + tokblk * P:
                                            k2 * CAP + (tokblk + 1) * P],
                                    rhs=w2s[:, db * 512:(db + 1) * 512],
                                    start=(k2 == 0), stop=False,
                                )
                    # b2 add closes the accumulation group
                    for tb in range(4):
                        tokblk = half * 4 + tb
                        for db in range(2):
                            nc.tensor.matmul(
                                psos[tb * 2 + db][:],
                                lhsT=ones_r[:],
                                rhs=b2_sb[:, db * 512:(db + 1) * 512],
                                start=False, stop=True,
                            )
                        o_sb = opool.tile([P, D], f32, tag="o_sb")
                        for db in range(2):
                            nc.vector.tensor_scalar(
                                out=o_sb[:, db * 512:(db + 1) * 512],
                                in0=psos[tb * 2 + db][:],
                                scalar1=m_all[:, tokblk:tokblk + 1], scalar2=None,
                                op0=OP.mult,
                            )
                        nc.sync.dma_start(
                            out=o_d[tokblk * P:(tokblk + 1) * P, :], in_=o_sb[:]
                        )
    nc.finalize()
    return nc


def _get_nc():
    global _nc_cache
    if _nc_cache is None:
        _nc_cache = build_nc()
    return _nc_cache


def run(x, Wg, W1, b1, W2, b2, trace=False):
    from concourse.bass_utils import run_bass_kernel_spmd

    nc = _get_nc()
    xf = np.ascontiguousarray(np.asarray(x, np.float32).reshape(N, D))
    x_pad = np.concatenate([xf, np.zeros((1, D), np.float32)], 0)
    xT = np.ascontiguousarray(xf.T)
    Wg = np.ascontiguousarray(np.asarray(Wg, np.float32))
    W1b = np.asarray(W1, np.float32).astype(ml_dtypes.bfloat16)
    W2b = np.asarray(W2, np.float32).astype(ml_dtypes.bfloat16)
    b1f = np.asarray(b1, np.float32)
    b2f = np.asarray(b2, np.float32)

    in_maps = []
    for e in range(E):
        in_maps.append({
            "x": x_pad,
            "xt": xT,
            "wg": Wg,
            "w1": np.ascontiguousarray(W1b[e]),
            "b1": np.ascontiguousarray(b1f[e].reshape(H, 1)),
            "w2": np.ascontiguousarray(W2b[e]),
            "b2": np.ascontiguousarray(b2f[e].reshape(D, 1)),
            "ew": np.full((1, 1), float(e * W), np.float32),
        })
    res = run_bass_kernel_spmd(nc, in_maps, core_ids=list(range(E)), trace=trace)

    out = np.zeros((N, D), np.float32)
    for e in range(E):
        r = res.results[e]
        cnt = min(int(round(float(r["counts"][0, e]))), CAP)
        idx = r["glist"][:cnt, 0].astype(np.int64)
        out[idx] = r["o"][:cnt]
    aux = np.float32(res.results[0]["aux"][0, 0])
    return (out.reshape(B, T, D), aux), res


def kernel(x, Wg, W1, b1, W2, b2):
    (out, aux), _ = run(x, Wg, W1, b1, W2, b2, trace=False)
    return out, aux


# revision 26
# speedup vs baseline: 1.3138x; 1.1991x over previous
"""MoE (top-1 routing, E=8 experts) Trainium2 Bass kernel.

Sharding: expert-parallel over 8 NeuronCores. Every core runs a replicated
fp32 router over all N=8192 tokens (reads host-pretransposed xT), builds the
global dispatch table via counting-sort (one triangular-matmul cumsum per
4-tile block + indirect DMA scatter), then computes ONLY its own expert's
capacity-limited MLP in bf16 (fp32 accumulate). Host combine is a pure
index-based scatter of the per-expert compact outputs using device-computed
token indices.

Self-contained: hardcodes shapes from the problem spec.
"""

import numpy as np
import ml_dtypes

B, T, D, H, E = 4, 2048, 1024, 4096, 8
N = B * T            # 8192 tokens
CAP = 1024           # capacity = max(4, ceil(N/E))
W = 2048             # table bucket width (max supported tokens/expert)
P = 128
NT = N // P          # 64 token tiles
NB = NT // 4         # 16 blocks of 4 tiles (512 tokens)
AUX_LOSS_COEF = 0.01
Z_LOSS_COEF = 0.001

_nc_cache = None


def build_nc():
    import concourse.bacc as bacc
    import concourse.tile as tile
    import concourse.bass as bass
    import concourse.mybir as mybir
    from concourse.masks import make_identity, make_upper_triangular

    f32 = mybir.dt.float32
    bf16 = mybir.dt.bfloat16
    i32 = mybir.dt.int32
    AF = mybir.ActivationFunctionType
    OP = mybir.AluOpType
    AX = mybir.AxisListType

    nc = bacc.Bacc(None, target_bir_lowering=False, num_swdge_queues=2)

    # ---- I/O ----
    x_pad = nc.dram_tensor("x", [N + 1, D], f32, kind="ExternalInput")
    xT_d = nc.dram_tensor("xt", [D, N], f32, kind="ExternalInput")
    wg_d = nc.dram_tensor("wg", [D, E], f32, kind="ExternalInput")
    w1_d = nc.dram_tensor("w1", [D, H], bf16, kind="ExternalInput")
    b1_d = nc.dram_tensor("b1", [H, 1], f32, kind="ExternalInput")
    w2_d = nc.dram_tensor("w2", [H, D], bf16, kind="ExternalInput")
    b2_d = nc.dram_tensor("b2", [D, 1], f32, kind="ExternalInput")
    ew_d = nc.dram_tensor("ew", [1, 1], f32, kind="ExternalInput")     # e*W

    o_d = nc.dram_tensor("o", [CAP, D], f32, kind="ExternalOutput")
    glist_d = nc.dram_tensor("glist", [CAP, 1], i32, kind="ExternalOutput")
    counts_d = nc.dram_tensor("counts", [1, E], f32, kind="ExternalOutput")
    aux_d = nc.dram_tensor("aux", [1, 1], f32, kind="ExternalOutput")

    table2 = nc.dram_tensor("table2", [E * W + 2, 2], f32, kind="Internal")

    with tile.TileContext(nc) as tc:
        with (
            tc.tile_pool(name="const", bufs=1) as cpool,
            tc.tile_pool(name="resident", bufs=1) as rpool,
            tc.tile_pool(name="xin", bufs=2) as xpool,
            tc.tile_pool(name="small", bufs=4) as spool,
        ):
            # ---------------- constants ----------------
            ident = cpool.tile([P, P], f32)
            make_identity(nc, ident[:])
            ident8 = cpool.tile([8, 8], f32)
            make_identity(nc, ident8[:])
            U128 = cpool.tile([P, P], f32)
            make_upper_triangular(nc, U128[:], val=1.0, diag=True)
            ones_r = cpool.tile([1, P], f32)   # row of ones (K=1 bcast matmuls)
            nc.gpsimd.memset(ones_r[:], 1.0)
            ones_c = cpool.tile([P, 1], f32)   # column of ones (partition reduce)
            nc.gpsimd.memset(ones_c[:], 1.0)
            iota8_i = cpool.tile([P, E], i32)
            nc.gpsimd.iota(iota8_i[:], pattern=[[1, E]], base=0, channel_multiplier=0)
            iota8f = cpool.tile([P, E], f32)
            nc.vector.tensor_copy(iota8f[:], iota8_i[:])
            tok_i = cpool.tile([P, NT], i32)   # tok_i[p, t] = t*128 + p
            nc.gpsimd.iota(tok_i[:], pattern=[[P, NT]], base=0, channel_multiplier=1)
            tokf = cpool.tile([P, NT], f32)
            nc.vector.tensor_copy(tokf[:], tok_i[:])

            wg_sb = cpool.tile([P, E * 8], f32)   # wg strips: col k*8+e
            nc.sync.dma_start(
                out=wg_sb[:].rearrange("p (k e) -> p k e", e=E),
                in_=wg_d[:].rearrange("(k p) e -> p k e", p=P),
            )
            b1_sb = cpool.tile([P, H // P], f32)  # b1[j*128+p] -> [p, j]
            nc.sync.dma_start(
                out=b1_sb[:].rearrange("p j -> p j ()"),
                in_=b1_d[:].rearrange("(j p) o -> p j o", p=P),
            )
            b2_sb = cpool.tile([1, D], f32)
            nc.sync.dma_start(out=b2_sb[:], in_=b2_d[:].rearrange("d o -> o d"))
            ew_sb = cpool.tile([1, 1], f32)
            nc.sync.dma_start(out=ew_sb[:], in_=ew_d[:])

            # accumulators
            pacc4 = rpool.tile([P, 4 * E], f32, tag="pacc4")
            nc.gpsimd.memset(pacc4[:], 0.0)
            base = rpool.tile([1, E], f32, tag="base")
            nc.gpsimd.memset(base[:], 0.0)
            rowmax_all = rpool.tile([P, NT], f32, tag="rowmax_all")
            sumex_all = rpool.tile([P, NT], f32, tag="sumex_all")
            oh_all = rpool.tile([P, NT * E], f32, tag="oh_all")
            pmax_all = rpool.tile([P, NT], f32, tag="pmax_all")
            eidx_all = rpool.tile([P, NT], f32, tag="eidx_all")

            # table2 init to sentinel 8192.0 (one DMA from an SBUF tile)
            sent = rpool.tile([P, E * W * 2 // P], f32, tag="sent")
            nc.gpsimd.memset(sent[:], float(N))
            nc.sync.dma_start(
                out=table2[0:E * W, :].rearrange("(a r) c -> a (r c)", a=P),
                in_=sent[:],
            )
            nc.sync.dma_start(out=table2[E * W:E * W + 2, :], in_=sent[0:2, 0:2])

            # resident MLP buffers
            xT_bf = [rpool.tile([P, CAP], bf16, tag=f"xT_{k}", name=f"xT_{k}") for k in range(8)]
            # hT layout: [128 h-partitions, (j, tok) free]: col j*CAP + tok
            hT = rpool.tile([P, (H // P) * CAP], bf16, tag="hT")
            m_all = rpool.tile([P, 8], f32, tag="m_all")
            startsBC = rpool.tile([P, E], f32, tag="startsBC")
            ewBC = rpool.tile([P, 1], f32, tag="ewBC")

            # ================= PHASE A: replicated router =================
            with (
                tc.tile_pool(name="psLT", bufs=2, space="PSUM") as psLT,
                tc.tile_pool(name="psL", bufs=4, space="PSUM") as psL,
                tc.tile_pool(name="psS", bufs=2, space="PSUM") as psS,
                tc.tile_pool(name="xsp", bufs=2) as xsp,
            ):
                for b in range(NB):
                    # load xT strip-block [1024 d, 512 tok] as 8 strips
                    xs = []
                    for k in range(8):
                        s = xsp.tile([P, 512], f32, tag=f"xs{k}", name=f"xs_{b}_{k}")
                        nc.sync.dma_start(
                            out=s[:], in_=xT_d[k * P:(k + 1) * P, b * 512:(b + 1) * 512]
                        )
                        xs.append(s)
                    # logitsT [8, 512] accumulated over k (wg stationary)
                    ps_lt = psLT.tile([8, 512], f32, tag="ps_lt")
                    for k in range(8):
                        nc.tensor.matmul(
                            ps_lt[:], lhsT=wg_sb[:, k * 8:k * 8 + E], rhs=xs[k][:],
                            start=(k == 0), stop=(k == 7),
                        )
                    lt_sb = spool.tile([8, 512], f32, tag="lt_sb")
                    nc.vector.tensor_copy(lt_sb[:], ps_lt[:])
                    # transpose back into [128 tok, (t,8)] layout
                    logits4 = spool.tile([P, 4 * E], f32, tag="logits4")
                    for t in range(4):
                        ps_l = psL.tile([P, E], f32, tag="ps_l")
                        nc.tensor.transpose(ps_l[:], lt_sb[:, t * P:(t + 1) * P], ident8[:])
                        nc.vector.tensor_copy(logits4[:, t * E:(t + 1) * E], ps_l[:])

                    l3 = logits4[:].rearrange("p (t e) -> p t e", e=E)
                    rm4 = rowmax_all[:, b * 4:(b + 1) * 4]
                    nc.vector.tensor_reduce(rm4, l3, axis=AX.X, op=OP.max)
                    rm4b = rowmax_all[:].rearrange("p t -> p t ()")[
                        :, b * 4:(b + 1) * 4, :
                    ].to_broadcast([P, 4, E])
                    sh4 = spool.tile([P, 4 * E], f32, tag="sh4")
                    sh43 = sh4[:].rearrange("p (t e) -> p t e", e=E)
                    nc.vector.tensor_tensor(out=sh43, in0=l3, in1=rm4b, op=OP.subtract)
                    ex4 = spool.tile([P, 4 * E], f32, tag="ex4")
                    nc.scalar.activation(ex4[:], sh4[:], AF.Exp)
                    ex43 = ex4[:].rearrange("p (t e) -> p t e", e=E)
                    se4 = sumex_all[:, b * 4:(b + 1) * 4]
                    nc.vector.tensor_reduce(se4, ex43, axis=AX.X, op=OP.add)
                    rec4 = spool.tile([P, 4], f32, tag="rec4")
                    nc.vector.reciprocal(rec4[:], se4)
                    rec4b = rec4[:].rearrange("p t -> p t ()").to_broadcast([P, 4, E])
                    probs4 = spool.tile([P, 4 * E], f32, tag="probs4")
                    p43 = probs4[:].rearrange("p (t e) -> p t e", e=E)
                    nc.vector.tensor_tensor(out=p43, in0=ex43, in1=rec4b, op=OP.mult)
                    nc.vector.tensor_tensor(out=pacc4[:], in0=pacc4[:], in1=probs4[:], op=OP.add)
                    nc.vector.tensor_reduce(pmax_all[:, b * 4:(b + 1) * 4], p43, axis=AX.X, op=OP.max)

                    # first-argmax per tile: min over masked expert iota
                    iota48 = iota8f[:].rearrange("p e -> p () e").to_broadcast([P, 4, E])
                    eq4 = spool.tile([P, 4 * E], f32, tag="eq4")
                    eq43 = eq4[:].rearrange("p (t e) -> p t e", e=E)
                    nc.vector.tensor_tensor(out=eq43, in0=l3, in1=rm4b, op=OP.is_equal)
                    m14 = spool.tile([P, 4 * E], f32, tag="m14")
                    m143 = m14[:].rearrange("p (t e) -> p t e", e=E)
                    nc.vector.tensor_tensor(out=m143, in0=iota48, in1=eq43, op=OP.mult)
                    m24 = spool.tile([P, 4 * E], f32, tag="m24")
                    nc.vector.tensor_scalar(
                        out=m24[:], in0=eq4[:], scalar1=-9.0, scalar2=9.0,
                        op0=OP.mult, op1=OP.add,
                    )
                    nc.vector.tensor_tensor(out=m14[:], in0=m14[:], in1=m24[:], op=OP.add)
                    eidx4 = eidx_all[:, b * 4:(b + 1) * 4]
                    nc.vector.tensor_reduce(eidx4, m143, axis=AX.X, op=OP.min)
                    eidx4b = eidx_all[:].rearrange("p t -> p t ()")[
                        :, b * 4:(b + 1) * 4, :
                    ].to_broadcast([P, 4, E])
                    oh43 = oh_all[:].rearrange("p (t e) -> p t e", e=E)[
                        :, b * 4:(b + 1) * 4, :
                    ]
                    nc.vector.tensor_tensor(out=oh43, in0=iota48, in1=eidx4b, op=OP.is_equal)

                    # pmax into resident store (for scatter payload later)
                    # (pmax4 already written via pmax_all slice above)

                    if b % (NB // 4) == (NB // 4) - 1:
                        hb = b // (NB // 4)          # which quarter just finished
                        hw = NT // 4                 # 16 tiles per quarter
                        ht0 = hb * hw                # first tile of half
                        c0 = ht0 * E                 # first oh column
                        cw = hw * E                  # 256 columns
                        # tile totals for this half: [1, (t,e)]
                        ps_tt = psS.tile([1, cw], f32, tag="ps_small", name=f"ps_tt{hb}")
                        nc.tensor.matmul(
                            ps_tt[:], lhsT=ones_c[:], rhs=oh_all[:, c0:c0 + cw],
                            start=True, stop=True,
                        )
                        tots = spool.tile([1, cw], f32, tag="tots", name=f"tots{hb}", bufs=2)
                        nc.vector.tensor_copy(tots[:], ps_tt[:])
                        # inclusive prefix over tiles (shift-adds), then exclusive
                        for sh in [1, 2, 4, 8]:
                            nc.vector.tensor_tensor(
                                out=tots[:, sh * E:cw], in0=tots[:, sh * E:cw],
                                in1=tots[:, 0:cw - sh * E], op=OP.add,
                            )
                        # exclusive prefix = inclusive shifted right one tile
                        exclp = spool.tile([1, cw], f32, tag="exclp", name=f"exclp{hb}", bufs=2)
                        nc.vector.memset(exclp[:, 0:E], 0.0)
                        nc.vector.tensor_copy(exclp[:, E:cw], tots[:, 0:cw - E])
                        # baseb[t] = carry base (prev halves) + exclusive prefix
                        baseb = spool.tile([1, cw], f32, tag="baseb", name=f"baseb{hb}", bufs=2)
                        bb3 = baseb[:].rearrange("o (t e) -> o t e", e=E)
                        nc.vector.tensor_tensor(
                            out=bb3,
                            in0=exclp[:].rearrange("o (t e) -> o t e", e=E),
                            in1=base[:].rearrange("o e -> o () e").to_broadcast([1, hw, E]),
                            op=OP.add,
                        )
                        # update global base with this half's grand total (last inclusive)
                        nc.vector.tensor_tensor(
                            out=base[:], in0=base[:], in1=tots[:, cw - E:cw], op=OP.add,
                        )
                        # rank psum = per-tile cumsum + base broadcast (one accum group)
                        ps_rk = psLT.tile([P, cw], f32, tag="ps_lt", name=f"ps_rk{hb}")
                        nc.tensor.matmul(
                            ps_rk[:], lhsT=U128[:], rhs=oh_all[:, c0:c0 + cw],
                            start=True, stop=False,
                        )
                        nc.tensor.matmul(
                            ps_rk[:], lhsT=ones_r[:], rhs=baseb[:],
                            start=False, stop=True,
                        )
                        sel = spool.tile([P, cw], f32, tag="selh", name=f"selh{hb}", bufs=2)
                        nc.vector.tensor_tensor(
                            out=sel[:], in0=ps_rk[:], in1=oh_all[:, c0:c0 + cw], op=OP.mult,
                        )
                        rankh = spool.tile([P, hw], f32, tag="rankh", name=f"rankh{hb}", bufs=2)
                        nc.vector.tensor_reduce(
                            rankh[:], sel[:].rearrange("p (t e) -> p t e", e=E),
                            axis=AX.X, op=OP.add,
                        )
                        nc.vector.tensor_scalar(
                            out=rankh[:], in0=rankh[:], scalar1=-1.0, scalar2=None, op0=OP.add
                        )
                        # keys = eidx*W + rank, clamped to trash row if rank >= W
                        keyh = spool.tile([P, hw], f32, tag="keyh", name=f"keyh{hb}", bufs=2)
                        nc.vector.tensor_scalar(
                            out=keyh[:], in0=eidx_all[:, ht0:ht0 + hw],
                            scalar1=float(W), scalar2=None, op0=OP.mult,
                        )
                        nc.vector.tensor_tensor(out=keyh[:], in0=keyh[:], in1=rankh[:], op=OP.add)
                        okh = spool.tile([P, hw], f32, tag="okh", name=f"okh{hb}", bufs=2)
                        nc.vector.tensor_scalar(
                            out=okh[:], in0=rankh[:], scalar1=float(W), scalar2=None, op0=OP.is_lt
                        )
                        nc.vector.tensor_tensor(out=keyh[:], in0=keyh[:], in1=okh[:], op=OP.mult)
                        nc.vector.tensor_scalar(
                            out=okh[:], in0=okh[:], scalar1=-float(E * W), scalar2=float(E * W),
                            op0=OP.mult, op1=OP.add,
                        )
                        nc.vector.tensor_tensor(out=keyh[:], in0=keyh[:], in1=okh[:], op=OP.add)
                        keyih = spool.tile([P, hw], i32, tag="keyih", name=f"keyih{hb}", bufs=2)
                        nc.vector.tensor_copy(keyih[:], keyh[:])
                        comboh = spool.tile([P, hw * 2], f32, tag="comboh", name=f"comboh{hb}", bufs=2)
                        ch3 = comboh[:].rearrange("p (t c) -> p t c", c=2)
                        nc.vector.tensor_copy(
                            ch3[:, :, 0:1],
                            pmax_all[:, ht0:ht0 + hw].rearrange("p t -> p t ()"),
                        )
                        nc.vector.tensor_copy(
                            ch3[:, :, 1:2],
                            tokf[:, ht0:ht0 + hw].rearrange("p t -> p t ()"),
                        )
                        for tt in range(hw):
                            nc.gpsimd.indirect_dma_start(
                                out=table2[:],
                                out_offset=bass.IndirectOffsetOnAxis(
                                    ap=keyih[:, tt:tt + 1], axis=0),
                                in_=comboh[:, 2 * tt:2 * tt + 2],
                                in_offset=None,
                            )

                # ---- counts / starts / z-loss tail / aux ----
                nc.sync.dma_start(out=counts_d[:], in_=base[:])
                c1 = spool.tile([1, E], f32, tag="c1")
                nc.vector.tensor_copy(c1[:], base[:])
                nc.vector.tensor_tensor(out=c1[:, 1:E], in0=base[:, 1:E], in1=base[:, 0:E - 1], op=OP.add)
                c2 = spool.tile([1, E], f32, tag="c2")
                nc.vector.tensor_copy(c2[:], c1[:])
                nc.vector.tensor_tensor(out=c2[:, 2:E], in0=c1[:, 2:E], in1=c1[:, 0:E - 2], op=OP.add)
                c3 = spool.tile([1, E], f32, tag="c3")
                nc.vector.tensor_copy(c3[:], c2[:])
                nc.vector.tensor_tensor(out=c3[:, 4:E], in0=c2[:, 4:E], in1=c2[:, 0:E - 4], op=OP.add)
                excl = spool.tile([1, E], f32, tag="excl")
                nc.vector.tensor_tensor(out=excl[:], in0=c3[:], in1=base[:], op=OP.subtract)

                # z-loss: lse = rowmax + ln(sumex), batched over all 64 tiles
                lns = spool.tile([P, NT], f32, tag="lns")
                nc.scalar.activation(lns[:], sumex_all[:], AF.Ln)
                nc.vector.tensor_tensor(out=lns[:], in0=lns[:], in1=rowmax_all[:], op=OP.add)
                nc.vector.tensor_tensor(out=lns[:], in0=lns[:], in1=lns[:], op=OP.mult)
                zrow = spool.tile([P, 1], f32, tag="zrow")
                nc.vector.tensor_reduce(zrow[:], lns[:], axis=AX.X, op=OP.add)
                # fold pacc4 -> [P, E]
                pacc = spool.tile([P, E], f32, tag="pacc")
                nc.vector.tensor_tensor(out=pacc[:], in0=pacc4[:, 0:E], in1=pacc4[:, E:2 * E], op=OP.add)
                nc.vector.tensor_tensor(out=pacc[:], in0=pacc[:], in1=pacc4[:, 2 * E:3 * E], op=OP.add)
                nc.vector.tensor_tensor(out=pacc[:], in0=pacc[:], in1=pacc4[:, 3 * E:4 * E], op=OP.add)

                ps_z = psS.tile([1, 1], f32, tag="ps_small", name="ps_z")
                nc.tensor.matmul(ps_z[:], lhsT=zrow[:], rhs=ones_c[:], start=True, stop=True)
                ps_p = psS.tile([1, E], f32, tag="ps_small", name="ps_p")
                nc.tensor.matmul(ps_p[:], lhsT=ones_c[:], rhs=pacc[:], start=True, stop=True)
                fp = spool.tile([1, E], f32, tag="fp")
                nc.vector.tensor_tensor(out=fp[:], in0=ps_p[:], in1=base[:], op=OP.mult)
                auxv = spool.tile([1, 1], f32, tag="auxv")
                nc.vector.tensor_reduce(auxv[:], fp[:], axis=AX.X, op=OP.add)
                nc.vector.tensor_scalar(
                    out=auxv[:], in0=auxv[:],
                    scalar1=AUX_LOSS_COEF * E / (float(N) * float(N)), scalar2=None,
                    op0=OP.mult,
                )
                zv = spool.tile([1, 1], f32, tag="zv")
                nc.vector.tensor_scalar(
                    out=zv[:], in0=ps_z[:], scalar1=Z_LOSS_COEF / float(N), scalar2=None,
                    op0=OP.mult,
                )
                nc.vector.tensor_tensor(out=auxv[:], in0=auxv[:], in1=zv[:], op=OP.add)
                nc.sync.dma_start(out=aux_d[:], in_=auxv[:])

                # broadcasts for phase B
                ps_sb2 = psS.tile([P, E], f32, tag="ps_small", name="ps_sb2")
                nc.tensor.matmul(ps_sb2[:], lhsT=ones_r[:], rhs=excl[:], start=True, stop=True)
                nc.vector.tensor_copy(startsBC[:], ps_sb2[:])
                ps_s2 = psS.tile([P, 1], f32, tag="ps_small", name="ps_s2")
                nc.tensor.matmul(ps_s2[:], lhsT=ones_r[:], rhs=ew_sb[:], start=True, stop=True)
                nc.vector.tensor_copy(ewBC[:], ps_s2[:])

            # ================= PHASE B1 + mm1 =================
            with (
                tc.tile_pool(name="psT", bufs=2, space="PSUM") as psT,
                tc.tile_pool(name="psM1", bufs=4, space="PSUM") as psM1,
                tc.tile_pool(name="w1p", bufs=1) as w1p,
            ):
                w1_sb = [w1p.tile([P, H], bf16, tag=f"w1_{k}", name=f"w1_{k}") for k in range(8)]
                for k in range(8):
                    nc.sync.dma_start(out=w1_sb[k][:], in_=w1_d[k * P:(k + 1) * P, :])
                for c in range(8):
                    s_f = tokf[:, c:c + 1]  # slot ids c*128+p
                    k1f = spool.tile([P, 1], f32, tag="k1f")
                    nc.vector.tensor_tensor(out=k1f[:], in0=ewBC[:], in1=s_f, op=OP.add)
                    nc.vector.tensor_scalar(
                        out=k1f[:], in0=k1f[:], scalar1=float(E * W - 1), scalar2=None,
                        op0=OP.min,
                    )
                    k1i = spool.tile([P, 1], i32, tag="k1i")
                    nc.vector.tensor_copy(k1i[:], k1f[:])
                    g1 = spool.tile([P, 2], f32, tag="g1")
                    nc.gpsimd.indirect_dma_start(
                        out=g1[:], out_offset=None,
                        in_=table2[:],
                        in_offset=bass.IndirectOffsetOnAxis(ap=k1i[:, 0:1], axis=0),
                    )
                    ti = spool.tile([P, 1], i32, tag="ti")
                    nc.vector.tensor_copy(ti[:], g1[:, 1:2])
                    nc.sync.dma_start(out=glist_d[c * P:(c + 1) * P, :], in_=ti[:])
                    xg = xpool.tile([P, D], f32, tag="xg")
                    nc.gpsimd.indirect_dma_start(
                        out=xg[:], out_offset=None,
                        in_=x_pad[:],
                        in_offset=bass.IndirectOffsetOnAxis(ap=ti[:, 0:1], axis=0),
                    )
                    # m lookup: sorted position p = token id -> bucket -> table2 col0
                    pf = spool.tile([P, 1], f32, tag="pf")
                    nc.vector.tensor_copy(pf[:], g1[:, 1:2])
                    cmp = spool.tile([P, E], f32, tag="cmp")
                    nc.vector.tensor_tensor(
                        out=cmp[:], in0=pf[:].to_broadcast([P, E]), in1=startsBC[:],
                        op=OP.is_ge,
                    )
                    ehat = spool.tile([P, 1], f32, tag="ehat")
                    nc.vector.tensor_reduce(ehat[:], cmp[:], axis=AX.X, op=OP.add)
                    nc.vector.tensor_scalar(
                        out=ehat[:], in0=ehat[:], scalar1=-1.0, scalar2=None, op0=OP.add
                    )
                    oh8 = spool.tile([P, E], f32, tag="oh8")
                    nc.vector.tensor_scalar(
                        out=oh8[:], in0=iota8f[:], scalar1=ehat[:], scalar2=None,
                        op0=OP.is_equal,
                    )
                    sts = spool.tile([P, E], f32, tag="sts")
                    nc.vector.tensor_tensor(out=sts[:], in0=startsBC[:], in1=oh8[:], op=OP.mult)
                    stsel = spool.tile([P, 1], f32, tag="stsel")
                    nc.vector.tensor_reduce(stsel[:], sts[:], axis=AX.X, op=OP.add)
                    rr = spool.tile([P, 1], f32, tag="rr")
                    nc.vector.tensor_tensor(out=rr[:], in0=pf[:], in1=stsel[:], op=OP.subtract)
                    k2f = spool.tile([P, 1], f32, tag="k2f")
                    nc.vector.tensor_scalar(
                        out=k2f[:], in0=ehat[:], scalar1=float(W), scalar2=None, op0=OP.mult
                    )
                    nc.vector.tensor_tensor(out=k2f[:], in0=k2f[:], in1=rr[:], op=OP.add)
                    nc.vector.tensor_scalar(
                        out=k2f[:], in0=k2f[:], scalar1=float(E * W + 1), scalar2=None,
                        op0=OP.min,
                    )
                    k2i = spool.tile([P, 1], i32, tag="k2i")
                    nc.vector.tensor_copy(k2i[:], k2f[:])
                    g2 = spool.tile([P, 2], f32, tag="g2")
                    nc.gpsimd.indirect_dma_start(
                        out=g2[:], out_offset=None,
                        in_=table2[:],
                        in_offset=bass.IndirectOffsetOnAxis(ap=k2i[:, 0:1], axis=0),
                    )
                    nc.vector.tensor_copy(m_all[:, c:c + 1], g2[:, 0:1])

                    # transpose gathered rows into bf16 xT strips
                    for k in range(8):
                        pst = psT.tile([P, P], f32)
                        nc.tensor.transpose(pst[:], xg[:, k * P:(k + 1) * P], ident[:])
                        nc.vector.tensor_copy(xT_bf[k][:, c * P:(c + 1) * P], pst[:])

                # ---- mm1 + gelu: hT[j] = gelu(x @ W1 + b1), stationary reused over n
                for j in range(H // P):
                    psm = [
                        psM1.tile([P, 512], f32, tag="ps_mm1", name=f"psm_{j}_{n}")
                        for n in range(2)
                    ]
                    for k in range(8):
                        for n in range(2):
                            nc.tensor.matmul(
                                psm[n][:],
                                lhsT=w1_sb[k][:, j * P:(j + 1) * P],
                                rhs=xT_bf[k][:, n * 512:(n + 1) * 512],
                                start=(k == 0), stop=(k == 7),
                            )
                    for n in range(2):
                        nc.scalar.activation(
                            hT[:, j * CAP + n * 512: j * CAP + (n + 1) * 512],
                            psm[n][:], AF.Gelu, bias=b1_sb[:, j:j + 1],
                        )

            # ================= PHASE B3: mm2 (+b2, *m) =================
            with (
                tc.tile_pool(name="psM2", bufs=1, space="PSUM") as psM2,
                tc.tile_pool(name="w2p", bufs=3) as w2pool,
                tc.tile_pool(name="outp", bufs=2) as opool,
            ):
                for half in range(2):
                    psos = []
                    for tb in range(4):
                        for db in range(2):
                            psos.append(psM2.tile([P, 512], f32, tag=f"ps_o{tb}{db}", name=f"ps_o{half}{tb}{db}"))
                    for k2 in range(H // P):
                        w2s = w2pool.tile([P, D], bf16, tag="w2s")
                        nc.sync.dma_start(out=w2s[:], in_=w2_d[k2 * P:(k2 + 1) * P, :])
                        for tb in range(4):
                            tokblk = half * 4 + tb
                            for db in range(2):
                                nc.tensor.matmul(
                                    psos[tb * 2 + db][:],
                                    lhsT=hT[:, k2 * CAP + tokblk * P:
                                            k2 * CAP + (tokblk + 1) * P],
                                    rhs=w2s[:, db * 512:(db + 1) * 512],
                                    start=(k2 == 0), stop=False,
                                )
                    # b2 add closes the accumulation group
                    for tb in range(4):
                        tokblk = half * 4 + tb
                        for db in range(2):
                            nc.tensor.matmul(
                                psos[tb * 2 + db][:],
                                lhsT=ones_r[:],
                                rhs=b2_sb[:, db * 512:(db + 1) * 512],
                                start=False, stop=True,
                            )
                        o_sb = opool.tile([P, D], f32, tag="o_sb")
                        for db in range(2):
                            nc.vector.tensor_scalar(
                                out=o_sb[:, db * 512:(db + 1) * 512],
                                in0=psos[tb * 2 + db][:],
                                scalar1=m_all[:, tokblk:tokblk + 1], scalar2=None,
                                op0=OP.mult,
                            )
                        nc.sync.dma_start(
                            out=o_d[tokblk * P:(tokblk + 1) * P, :], in_=o_sb[:]
                        )
    nc.finalize()
    return nc


def _get_nc():
    global _nc_cache
    if _nc_cache is None:
        _nc_cache = build_nc()
    return _nc_cache


def run(x, Wg, W1, b1, W2, b2, trace=False):
    from concourse.bass_utils import run_bass_kernel_spmd

    nc = _get_nc()
    xf = np.ascontiguousarray(np.asarray(x, np.float32).reshape(N, D))
    x_pad = np.concatenate([xf, np.zeros((1, D), np.float32)], 0)
    xT = np.ascontiguousarray(xf.T)
    Wg = np.ascontiguousarray(np.asarray(Wg, np.float32))
    W1b = np.asarray(W1, np.float32).astype(ml_dtypes.bfloat16)
    W2b = np.asarray(W2, np.float32).astype(ml_dtypes.bfloat16)
    b1f = np.asarray(b1, np.float32)
    b2f = np.asarray(b2, np.float32)

    in_maps = []
    for e in range(E):
        in_maps.append({
            "x": x_pad,
            "xt": xT,
            "wg": Wg,
            "w1": np.ascontiguousarray(W1b[e]),
            "b1": np.ascontiguousarray(b1f[e].reshape(H, 1)),
            "w2": np.ascontiguousarray(W2b[e]),
            "b2": np.ascontiguousarray(b2f[e].reshape(D, 1)),
            "ew": np.full((1, 1), float(e * W), np.float32),
        })
    res = run_bass_kernel_spmd(nc, in_maps, core_ids=list(range(E)), trace=trace)

    out = np.zeros((N, D), np.float32)
    for e in range(E):
        r = res.results[e]
        cnt = min(int(round(float(r["counts"][0, e]))), CAP)
        idx = r["glist"][:cnt, 0].astype(np.int64)
        out[idx] = r["o"][:cnt]
    aux = np.float32(res.results[0]["aux"][0, 0])
    return (out.reshape(B, T, D), aux), res


def kernel(x, Wg, W1, b1, W2, b2):
    (out, aux), _ = run(x, Wg, W1, b1, W2, b2, trace=False)
    return out, aux


# revision 28
# speedup vs baseline: 1.4103x; 1.0734x over previous
"""MoE (top-1 routing, E=8 experts) Trainium2 Bass kernel.

Sharding: expert-parallel over 8 NeuronCores. Every core runs a replicated
fp32 router over all N=8192 tokens (reads host-pretransposed xT), builds the
global dispatch table via counting-sort (one triangular-matmul cumsum per
4-tile block + indirect DMA scatter), then computes ONLY its own expert's
capacity-limited MLP in bf16 (fp32 accumulate). Host combine is a pure
index-based scatter of the per-expert compact outputs using device-computed
token indices.

Self-contained: hardcodes shapes from the problem spec.
"""

import numpy as np
import ml_dtypes

B, T, D, H, E = 4, 2048, 1024, 4096, 8
N = B * T            # 8192 tokens
CAP = 1024           # capacity = max(4, ceil(N/E))
W = 2048             # table bucket width (max supported tokens/expert)
P = 128
NT = N // P          # 64 token tiles
NB = NT // 4         # 16 blocks of 4 tiles (512 tokens)
AUX_LOSS_COEF = 0.01
Z_LOSS_COEF = 0.001

_nc_cache = None


def build_nc():
    import concourse.bacc as bacc
    import concourse.tile as tile
    import concourse.bass as bass
    import concourse.mybir as mybir
    from concourse.masks import make_identity, make_upper_triangular

    f32 = mybir.dt.float32
    bf16 = mybir.dt.bfloat16
    i32 = mybir.dt.int32
    AF = mybir.ActivationFunctionType
    OP = mybir.AluOpType
    AX = mybir.AxisListType

    nc = bacc.Bacc(None, target_bir_lowering=False, num_swdge_queues=2)

    # ---- I/O ----
    x_pad = nc.dram_tensor("x", [N + 1, D], f32, kind="ExternalInput")
    xT_d = nc.dram_tensor("xt", [D, N], f32, kind="ExternalInput")
    wg_d = nc.dram_tensor("wg", [D, E], f32, kind="ExternalInput")
    w1_d = nc.dram_tensor("w1", [D, H], bf16, kind="ExternalInput")
    b1_d = nc.dram_tensor("b1", [H, 1], f32, kind="ExternalInput")
    w2_d = nc.dram_tensor("w2", [H, D], bf16, kind="ExternalInput")
    b2_d = nc.dram_tensor("b2", [D, 1], f32, kind="ExternalInput")
    ew_d = nc.dram_tensor("ew", [1, 1], f32, kind="ExternalInput")     # e*W

    o_d = nc.dram_tensor("o", [CAP, D], f32, kind="ExternalOutput")
    glist_d = nc.dram_tensor("glist", [CAP, 1], i32, kind="ExternalOutput")
    counts_d = nc.dram_tensor("counts", [1, E], f32, kind="ExternalOutput")
    aux_d = nc.dram_tensor("aux", [1, 1], f32, kind="ExternalOutput")

    table2 = nc.dram_tensor("table2", [E * W + 2, 2], f32, kind="Internal")
    table2b = nc.dram_tensor("table2b", [E * W + 2, 2], f32, kind="Internal")

    with tile.TileContext(nc) as tc:
        with (
            tc.tile_pool(name="const", bufs=1) as cpool,
            tc.tile_pool(name="resident", bufs=1) as rpool,
            tc.tile_pool(name="xin", bufs=2) as xpool,
            tc.tile_pool(name="small", bufs=4) as spool,
        ):
            # ---------------- constants ----------------
            ident = cpool.tile([P, P], f32)
            make_identity(nc, ident[:])
            ident8 = cpool.tile([8, 8], f32)
            make_identity(nc, ident8[:])
            U128 = cpool.tile([P, P], f32)
            make_upper_triangular(nc, U128[:], val=1.0, diag=True)
            ones_r = cpool.tile([1, P], f32)   # row of ones (K=1 bcast matmuls)
            nc.gpsimd.memset(ones_r[:], 1.0)
            ones_c = cpool.tile([P, 1], f32)   # column of ones (partition reduce)
            nc.gpsimd.memset(ones_c[:], 1.0)
            iota8_i = cpool.tile([P, E], i32)
            nc.gpsimd.iota(iota8_i[:], pattern=[[1, E]], base=0, channel_multiplier=0)
            iota8f = cpool.tile([P, E], f32)
            nc.vector.tensor_copy(iota8f[:], iota8_i[:])
            tok_i = cpool.tile([P, NT], i32)   # tok_i[p, t] = t*128 + p
            nc.gpsimd.iota(tok_i[:], pattern=[[P, NT]], base=0, channel_multiplier=1)
            tokf = cpool.tile([P, NT], f32)
            nc.vector.tensor_copy(tokf[:], tok_i[:])

            wg_sb = cpool.tile([P, E * 8], f32)   # wg strips: col k*8+e
            nc.sync.dma_start(
                out=wg_sb[:].rearrange("p (k e) -> p k e", e=E),
                in_=wg_d[:].rearrange("(k p) e -> p k e", p=P),
            )
            b1_sb = cpool.tile([P, H // P], f32)  # b1[j*128+p] -> [p, j]
            nc.sync.dma_start(
                out=b1_sb[:].rearrange("p j -> p j ()"),
                in_=b1_d[:].rearrange("(j p) o -> p j o", p=P),
            )
            b2_sb = cpool.tile([1, D], f32)
            nc.sync.dma_start(out=b2_sb[:], in_=b2_d[:].rearrange("d o -> o d"))
            ew_sb = cpool.tile([1, 1], f32)
            nc.sync.dma_start(out=ew_sb[:], in_=ew_d[:])

            # accumulators
            pacc4 = rpool.tile([P, 4 * E], f32, tag="pacc4")
            nc.gpsimd.memset(pacc4[:], 0.0)
            base = rpool.tile([1, E], f32, tag="base")
            nc.gpsimd.memset(base[:], 0.0)
            rowmax_all = rpool.tile([P, NT], f32, tag="rowmax_all")
            sumex_all = rpool.tile([P, NT], f32, tag="sumex_all")
            oh_all = rpool.tile([P, NT * E], f32, tag="oh_all")
            pmax_all = rpool.tile([P, NT], f32, tag="pmax_all")
            eidx_all = rpool.tile([P, NT], f32, tag="eidx_all")

            # table2 init to sentinel 8192.0 (one DMA from an SBUF tile)
            sent = rpool.tile([P, E * W * 2 // P], f32, tag="sent")
            nc.gpsimd.memset(sent[:], float(N))
            for tb2 in (table2, table2b):
                nc.sync.dma_start(
                    out=tb2[0:E * W, :].rearrange("(a r) c -> a (r c)", a=P),
                    in_=sent[:],
                )
                nc.sync.dma_start(out=tb2[E * W:E * W + 2, :], in_=sent[0:2, 0:2])

            # resident MLP buffers
            xT_bf = [rpool.tile([P, CAP], bf16, tag=f"xT_{k}", name=f"xT_{k}") for k in range(8)]
            # hT layout: [128 h-partitions, (j, tok) free]: col j*CAP + tok
            hT = rpool.tile([P, (H // P) * CAP], bf16, tag="hT")
            m_all = rpool.tile([P, 8], f32, tag="m_all")
            startsBC = rpool.tile([P, E], f32, tag="startsBC")
            ewBC = rpool.tile([P, 1], f32, tag="ewBC")

            # ================= PHASE A: replicated router =================
            with (
                tc.tile_pool(name="psLT", bufs=2, space="PSUM") as psLT,
                tc.tile_pool(name="psL", bufs=4, space="PSUM") as psL,
                tc.tile_pool(name="psS", bufs=2, space="PSUM") as psS,
                tc.tile_pool(name="xsp", bufs=2) as xsp,
            ):
                for b in range(NB):
                    # load xT strip-block [1024 d, 512 tok] as 8 strips
                    xs = []
                    for k in range(8):
                        s = xsp.tile([P, 512], f32, tag=f"xs{k}", name=f"xs_{b}_{k}")
                        nc.sync.dma_start(
                            out=s[:], in_=xT_d[k * P:(k + 1) * P, b * 512:(b + 1) * 512]
                        )
                        xs.append(s)
                    # logitsT [8, 512] accumulated over k (wg stationary)
                    ps_lt = psLT.tile([8, 512], f32, tag="ps_lt")
                    for k in range(8):
                        nc.tensor.matmul(
                            ps_lt[:], lhsT=wg_sb[:, k * 8:k * 8 + E], rhs=xs[k][:],
                            start=(k == 0), stop=(k == 7),
                        )
                    lt_sb = spool.tile([8, 512], f32, tag="lt_sb")
                    nc.vector.tensor_copy(lt_sb[:], ps_lt[:])
                    # transpose back into [128 tok, (t,8)] layout
                    logits4 = spool.tile([P, 4 * E], f32, tag="logits4")
                    for t in range(4):
                        ps_l = psL.tile([P, E], f32, tag="ps_l")
                        nc.tensor.transpose(ps_l[:], lt_sb[:, t * P:(t + 1) * P], ident8[:])
                        nc.vector.tensor_copy(logits4[:, t * E:(t + 1) * E], ps_l[:])

                    l3 = logits4[:].rearrange("p (t e) -> p t e", e=E)
                    rm4 = rowmax_all[:, b * 4:(b + 1) * 4]
                    nc.vector.tensor_reduce(rm4, l3, axis=AX.X, op=OP.max)
                    rm4b = rowmax_all[:].rearrange("p t -> p t ()")[
                        :, b * 4:(b + 1) * 4, :
                    ].to_broadcast([P, 4, E])
                    sh4 = spool.tile([P, 4 * E], f32, tag="sh4")
                    sh43 = sh4[:].rearrange("p (t e) -> p t e", e=E)
                    nc.vector.tensor_tensor(out=sh43, in0=l3, in1=rm4b, op=OP.subtract)
                    ex4 = spool.tile([P, 4 * E], f32, tag="ex4")
                    nc.scalar.activation(ex4[:], sh4[:], AF.Exp)
                    ex43 = ex4[:].rearrange("p (t e) -> p t e", e=E)
                    se4 = sumex_all[:, b * 4:(b + 1) * 4]
                    nc.vector.tensor_reduce(se4, ex43, axis=AX.X, op=OP.add)
                    rec4 = spool.tile([P, 4], f32, tag="rec4")
                    nc.vector.reciprocal(rec4[:], se4)
                    rec4b = rec4[:].rearrange("p t -> p t ()").to_broadcast([P, 4, E])
                    probs4 = spool.tile([P, 4 * E], f32, tag="probs4")
                    p43 = probs4[:].rearrange("p (t e) -> p t e", e=E)
                    nc.vector.tensor_tensor(out=p43, in0=ex43, in1=rec4b, op=OP.mult)
                    nc.vector.tensor_tensor(out=pacc4[:], in0=pacc4[:], in1=probs4[:], op=OP.add)
                    nc.vector.tensor_reduce(pmax_all[:, b * 4:(b + 1) * 4], p43, axis=AX.X, op=OP.max)

                    # first-argmax per tile: min over masked expert iota
                    iota48 = iota8f[:].rearrange("p e -> p () e").to_broadcast([P, 4, E])
                    eq4 = spool.tile([P, 4 * E], f32, tag="eq4")
                    eq43 = eq4[:].rearrange("p (t e) -> p t e", e=E)
                    nc.vector.tensor_tensor(out=eq43, in0=l3, in1=rm4b, op=OP.is_equal)
                    m14 = spool.tile([P, 4 * E], f32, tag="m14")
                    m143 = m14[:].rearrange("p (t e) -> p t e", e=E)
                    nc.vector.tensor_tensor(out=m143, in0=iota48, in1=eq43, op=OP.mult)
                    m24 = spool.tile([P, 4 * E], f32, tag="m24")
                    nc.vector.tensor_scalar(
                        out=m24[:], in0=eq4[:], scalar1=-9.0, scalar2=9.0,
                        op0=OP.mult, op1=OP.add,
                    )
                    nc.vector.tensor_tensor(out=m14[:], in0=m14[:], in1=m24[:], op=OP.add)
                    eidx4 = eidx_all[:, b * 4:(b + 1) * 4]
                    nc.vector.tensor_reduce(eidx4, m143, axis=AX.X, op=OP.min)
                    eidx4b = eidx_all[:].rearrange("p t -> p t ()")[
                        :, b * 4:(b + 1) * 4, :
                    ].to_broadcast([P, 4, E])
                    oh43 = oh_all[:].rearrange("p (t e) -> p t e", e=E)[
                        :, b * 4:(b + 1) * 4, :
                    ]
                    nc.vector.tensor_tensor(out=oh43, in0=iota48, in1=eidx4b, op=OP.is_equal)

                    # pmax into resident store (for scatter payload later)
                    # (pmax4 already written via pmax_all slice above)

                    if b % (NB // 4) == (NB // 4) - 1:
                        hb = b // (NB // 4)          # which quarter just finished
                        hw = NT // 4                 # 16 tiles per quarter
                        ht0 = hb * hw                # first tile of half
                        c0 = ht0 * E                 # first oh column
                        cw = hw * E                  # 256 columns
                        # tile totals for this half: [1, (t,e)]
                        ps_tt = psS.tile([1, cw], f32, tag="ps_small", name=f"ps_tt{hb}")
                        nc.tensor.matmul(
                            ps_tt[:], lhsT=ones_c[:], rhs=oh_all[:, c0:c0 + cw],
                            start=True, stop=True,
                        )
                        tots = spool.tile([1, cw], f32, tag="tots", name=f"tots{hb}", bufs=2)
                        nc.vector.tensor_copy(tots[:], ps_tt[:])
                        # inclusive prefix over tiles (shift-adds), then exclusive
                        for sh in [1, 2, 4, 8]:
                            nc.vector.tensor_tensor(
                                out=tots[:, sh * E:cw], in0=tots[:, sh * E:cw],
                                in1=tots[:, 0:cw - sh * E], op=OP.add,
                            )
                        # exclusive prefix = inclusive shifted right one tile
                        exclp = spool.tile([1, cw], f32, tag="exclp", name=f"exclp{hb}", bufs=2)
                        nc.vector.memset(exclp[:, 0:E], 0.0)
                        nc.vector.tensor_copy(exclp[:, E:cw], tots[:, 0:cw - E])
                        # baseb[t] = carry base (prev halves) + exclusive prefix
                        baseb = spool.tile([1, cw], f32, tag="baseb", name=f"baseb{hb}", bufs=2)
                        bb3 = baseb[:].rearrange("o (t e) -> o t e", e=E)
                        nc.vector.tensor_tensor(
                            out=bb3,
                            in0=exclp[:].rearrange("o (t e) -> o t e", e=E),
                            in1=base[:].rearrange("o e -> o () e").to_broadcast([1, hw, E]),
                            op=OP.add,
                        )
                        # update global base with this half's grand total (last inclusive)
                        nc.vector.tensor_tensor(
                            out=base[:], in0=base[:], in1=tots[:, cw - E:cw], op=OP.add,
                        )
                        # rank psum = per-tile cumsum + base broadcast (one accum group)
                        ps_rk = psLT.tile([P, cw], f32, tag="ps_lt", name=f"ps_rk{hb}")
                        nc.tensor.matmul(
                            ps_rk[:], lhsT=U128[:], rhs=oh_all[:, c0:c0 + cw],
                            start=True, stop=False,
                        )
                        nc.tensor.matmul(
                            ps_rk[:], lhsT=ones_r[:], rhs=baseb[:],
                            start=False, stop=True,
                        )
                        sel = spool.tile([P, cw], f32, tag="selh", name=f"selh{hb}", bufs=2)
                        nc.vector.tensor_tensor(
                            out=sel[:], in0=ps_rk[:], in1=oh_all[:, c0:c0 + cw], op=OP.mult,
                        )
                        rankh = spool.tile([P, hw], f32, tag="rankh", name=f"rankh{hb}", bufs=2)
                        nc.vector.tensor_reduce(
                            rankh[:], sel[:].rearrange("p (t e) -> p t e", e=E),
                            axis=AX.X, op=OP.add,
                        )
                        nc.vector.tensor_scalar(
                            out=rankh[:], in0=rankh[:], scalar1=-1.0, scalar2=None, op0=OP.add
                        )
                        # keys = eidx*W + rank, clamped to trash row if rank >= W
                        keyh = spool.tile([P, hw], f32, tag="keyh", name=f"keyh{hb}", bufs=2)
                        nc.vector.tensor_scalar(
                            out=keyh[:], in0=eidx_all[:, ht0:ht0 + hw],
                            scalar1=float(W), scalar2=None, op0=OP.mult,
                        )
                        nc.vector.tensor_tensor(out=keyh[:], in0=keyh[:], in1=rankh[:], op=OP.add)
                        okh = spool.tile([P, hw], f32, tag="okh", name=f"okh{hb}", bufs=2)
                        nc.vector.tensor_scalar(
                            out=okh[:], in0=rankh[:], scalar1=float(W), scalar2=None, op0=OP.is_lt
                        )
                        nc.vector.tensor_tensor(out=keyh[:], in0=keyh[:], in1=okh[:], op=OP.mult)
                        nc.vector.tensor_scalar(
                            out=okh[:], in0=okh[:], scalar1=-float(E * W), scalar2=float(E * W),
                            op0=OP.mult, op1=OP.add,
                        )
                        nc.vector.tensor_tensor(out=keyh[:], in0=keyh[:], in1=okh[:], op=OP.add)
                        keyih = spool.tile([P, hw], i32, tag="keyih", name=f"keyih{hb}", bufs=2)
                        nc.vector.tensor_copy(keyih[:], keyh[:])
                        comboh = spool.tile([P, hw * 2], f32, tag="comboh", name=f"comboh{hb}", bufs=2)
                        ch3 = comboh[:].rearrange("p (t c) -> p t c", c=2)
                        nc.vector.tensor_copy(
                            ch3[:, :, 0:1],
                            pmax_all[:, ht0:ht0 + hw].rearrange("p t -> p t ()"),
                        )
                        nc.vector.tensor_copy(
                            ch3[:, :, 1:2],
                            tokf[:, ht0:ht0 + hw].rearrange("p t -> p t ()"),
                        )
                        for tt in range(hw):
                            tgt = table2 if (ht0 + tt) % 2 == 0 else table2b
                            nc.gpsimd.indirect_dma_start(
                                out=tgt[:],
                                out_offset=bass.IndirectOffsetOnAxis(
                                    ap=keyih[:, tt:tt + 1], axis=0),
                                in_=comboh[:, 2 * tt:2 * tt + 2],
                                in_offset=None,
                            )

                # ---- counts / starts / z-loss tail / aux ----
                nc.sync.dma_start(out=counts_d[:], in_=base[:])
                c1 = spool.tile([1, E], f32, tag="c1")
                nc.vector.tensor_copy(c1[:], base[:])
                nc.vector.tensor_tensor(out=c1[:, 1:E], in0=base[:, 1:E], in1=base[:, 0:E - 1], op=OP.add)
                c2 = spool.tile([1, E], f32, tag="c2")
                nc.vector.tensor_copy(c2[:], c1[:])
                nc.vector.tensor_tensor(out=c2[:, 2:E], in0=c1[:, 2:E], in1=c1[:, 0:E - 2], op=OP.add)
                c3 = spool.tile([1, E], f32, tag="c3")
                nc.vector.tensor_copy(c3[:], c2[:])
                nc.vector.tensor_tensor(out=c3[:, 4:E], in0=c2[:, 4:E], in1=c2[:, 0:E - 4], op=OP.add)
                excl = spool.tile([1, E], f32, tag="excl")
                nc.vector.tensor_tensor(out=excl[:], in0=c3[:], in1=base[:], op=OP.subtract)

                # z-loss: lse = rowmax + ln(sumex), batched over all 64 tiles
                lns = spool.tile([P, NT], f32, tag="lns")
                nc.scalar.activation(lns[:], sumex_all[:], AF.Ln)
                nc.vector.tensor_tensor(out=lns[:], in0=lns[:], in1=rowmax_all[:], op=OP.add)
                nc.vector.tensor_tensor(out=lns[:], in0=lns[:], in1=lns[:], op=OP.mult)
                zrow = spool.tile([P, 1], f32, tag="zrow")
                nc.vector.tensor_reduce(zrow[:], lns[:], axis=AX.X, op=OP.add)
                # fold pacc4 -> [P, E]
                pacc = spool.tile([P, E], f32, tag="pacc")
                nc.vector.tensor_tensor(out=pacc[:], in0=pacc4[:, 0:E], in1=pacc4[:, E:2 * E], op=OP.add)
                nc.vector.tensor_tensor(out=pacc[:], in0=pacc[:], in1=pacc4[:, 2 * E:3 * E], op=OP.add)
                nc.vector.tensor_tensor(out=pacc[:], in0=pacc[:], in1=pacc4[:, 3 * E:4 * E], op=OP.add)

                ps_z = psS.tile([1, 1], f32, tag="ps_small", name="ps_z")
                nc.tensor.matmul(ps_z[:], lhsT=zrow[:], rhs=ones_c[:], start=True, stop=True)
                ps_p = psS.tile([1, E], f32, tag="ps_small", name="ps_p")
                nc.tensor.matmul(ps_p[:], lhsT=ones_c[:], rhs=pacc[:], start=True, stop=True)
                fp = spool.tile([1, E], f32, tag="fp")
                nc.vector.tensor_tensor(out=fp[:], in0=ps_p[:], in1=base[:], op=OP.mult)
                auxv = spool.tile([1, 1], f32, tag="auxv")
                nc.vector.tensor_reduce(auxv[:], fp[:], axis=AX.X, op=OP.add)
                nc.vector.tensor_scalar(
                    out=auxv[:], in0=auxv[:],
                    scalar1=AUX_LOSS_COEF * E / (float(N) * float(N)), scalar2=None,
                    op0=OP.mult,
                )
                zv = spool.tile([1, 1], f32, tag="zv")
                nc.vector.tensor_scalar(
                    out=zv[:], in0=ps_z[:], scalar1=Z_LOSS_COEF / float(N), scalar2=None,
                    op0=OP.mult,
                )
                nc.vector.tensor_tensor(out=auxv[:], in0=auxv[:], in1=zv[:], op=OP.add)
                nc.sync.dma_start(out=aux_d[:], in_=auxv[:])

                # broadcasts for phase B
                ps_sb2 = psS.tile([P, E], f32, tag="ps_small", name="ps_sb2")
                nc.tensor.matmul(ps_sb2[:], lhsT=ones_r[:], rhs=excl[:], start=True, stop=True)
                nc.vector.tensor_copy(startsBC[:], ps_sb2[:])
                ps_s2 = psS.tile([P, 1], f32, tag="ps_small", name="ps_s2")
                nc.tensor.matmul(ps_s2[:], lhsT=ones_r[:], rhs=ew_sb[:], start=True, stop=True)
                nc.vector.tensor_copy(ewBC[:], ps_s2[:])

            # ================= PHASE B1 + mm1 =================
            with (
                tc.tile_pool(name="psT", bufs=2, space="PSUM") as psT,
                tc.tile_pool(name="psM1", bufs=4, space="PSUM") as psM1,
                tc.tile_pool(name="w1p", bufs=1) as w1p,
            ):
                w1_sb = [w1p.tile([P, H], bf16, tag=f"w1_{k}", name=f"w1_{k}") for k in range(8)]
                for k in range(8):
                    nc.sync.dma_start(out=w1_sb[k][:], in_=w1_d[k * P:(k + 1) * P, :])
                for c in range(8):
                    s_f = tokf[:, c:c + 1]  # slot ids c*128+p
                    k1f = spool.tile([P, 1], f32, tag="k1f")
                    nc.vector.tensor_tensor(out=k1f[:], in0=ewBC[:], in1=s_f, op=OP.add)
                    nc.vector.tensor_scalar(
                        out=k1f[:], in0=k1f[:], scalar1=float(E * W - 1), scalar2=None,
                        op0=OP.min,
                    )
                    k1i = spool.tile([P, 1], i32, tag="k1i")
                    nc.vector.tensor_copy(k1i[:], k1f[:])
                    g1 = spool.tile([P, 2], f32, tag="g1")
                    nc.gpsimd.indirect_dma_start(
                        out=g1[:], out_offset=None,
                        in_=table2[:],
                        in_offset=bass.IndirectOffsetOnAxis(ap=k1i[:, 0:1], axis=0),
                    )
                    g1b = spool.tile([P, 2], f32, tag="g1b")
                    nc.gpsimd.indirect_dma_start(
                        out=g1b[:], out_offset=None,
                        in_=table2b[:],
                        in_offset=bass.IndirectOffsetOnAxis(ap=k1i[:, 0:1], axis=0),
                    )
                    sentm = spool.tile([P, 2], i32, tag="sentm")
                    nc.vector.tensor_scalar(
                        out=sentm[:], in0=g1[:], scalar1=float(N), scalar2=None,
                        op0=OP.is_equal,
                    )
                    nc.vector.copy_predicated(g1[:], sentm[:], g1b[:])
                    ti = spool.tile([P, 1], i32, tag="ti")
                    nc.vector.tensor_copy(ti[:], g1[:, 1:2])
                    nc.sync.dma_start(out=glist_d[c * P:(c + 1) * P, :], in_=ti[:])
                    xg = xpool.tile([P, D], f32, tag="xg")
                    nc.gpsimd.indirect_dma_start(
                        out=xg[:], out_offset=None,
                        in_=x_pad[:],
                        in_offset=bass.IndirectOffsetOnAxis(ap=ti[:, 0:1], axis=0),
                    )
                    # m lookup: sorted position p = token id -> bucket -> table2 col0
                    pf = spool.tile([P, 1], f32, tag="pf")
                    nc.vector.tensor_copy(pf[:], g1[:, 1:2])
                    cmp = spool.tile([P, E], f32, tag="cmp")
                    nc.vector.tensor_tensor(
                        out=cmp[:], in0=pf[:].to_broadcast([P, E]), in1=startsBC[:],
                        op=OP.is_ge,
                    )
                    ehat = spool.tile([P, 1], f32, tag="ehat")
                    nc.vector.tensor_reduce(ehat[:], cmp[:], axis=AX.X, op=OP.add)
                    nc.vector.tensor_scalar(
                        out=ehat[:], in0=ehat[:], scalar1=-1.0, scalar2=None, op0=OP.add
                    )
                    oh8 = spool.tile([P, E], f32, tag="oh8")
                    nc.vector.tensor_scalar(
                        out=oh8[:], in0=iota8f[:], scalar1=ehat[:], scalar2=None,
                        op0=OP.is_equal,
                    )
                    sts = spool.tile([P, E], f32, tag="sts")
                    nc.vector.tensor_tensor(out=sts[:], in0=startsBC[:], in1=oh8[:], op=OP.mult)
                    stsel = spool.tile([P, 1], f32, tag="stsel")
                    nc.vector.tensor_reduce(stsel[:], sts[:], axis=AX.X, op=OP.add)
                    rr = spool.tile([P, 1], f32, tag="rr")
                    nc.vector.tensor_tensor(out=rr[:], in0=pf[:], in1=stsel[:], op=OP.subtract)
                    k2f = spool.tile([P, 1], f32, tag="k2f")
                    nc.vector.tensor_scalar(
                        out=k2f[:], in0=ehat[:], scalar1=float(W), scalar2=None, op0=OP.mult
                    )
                    nc.vector.tensor_tensor(out=k2f[:], in0=k2f[:], in1=rr[:], op=OP.add)
                    nc.vector.tensor_scalar(
                        out=k2f[:], in0=k2f[:], scalar1=float(E * W + 1), scalar2=None,
                        op0=OP.min,
                    )
                    k2i = spool.tile([P, 1], i32, tag="k2i")
                    nc.vector.tensor_copy(k2i[:], k2f[:])
                    g2 = spool.tile([P, 2], f32, tag="g2")
                    nc.gpsimd.indirect_dma_start(
                        out=g2[:], out_offset=None,
                        in_=table2[:],
                        in_offset=bass.IndirectOffsetOnAxis(ap=k2i[:, 0:1], axis=0),
                    )
                    g2b = spool.tile([P, 2], f32, tag="g2b")
                    nc.gpsimd.indirect_dma_start(
                        out=g2b[:], out_offset=None,
                        in_=table2b[:],
                        in_offset=bass.IndirectOffsetOnAxis(ap=k2i[:, 0:1], axis=0),
                    )
                    sentm2 = spool.tile([P, 2], i32, tag="sentm2")
                    nc.vector.tensor_scalar(
                        out=sentm2[:], in0=g2[:], scalar1=float(N), scalar2=None,
                        op0=OP.is_equal,
                    )
                    nc.vector.copy_predicated(g2[:], sentm2[:], g2b[:])
                    nc.vector.tensor_copy(m_all[:, c:c + 1], g2[:, 0:1])

                    # transpose gathered rows into bf16 xT strips
                    for k in range(8):
                        pst = psT.tile([P, P], f32)
                        nc.tensor.transpose(pst[:], xg[:, k * P:(k + 1) * P], ident[:])
                        nc.vector.tensor_copy(xT_bf[k][:, c * P:(c + 1) * P], pst[:])

                # ---- mm1 + gelu: hT[j] = gelu(x @ W1 + b1), stationary reused over n
                for j in range(H // P):
                    psm = [
                        psM1.tile([P, 512], f32, tag="ps_mm1", name=f"psm_{j}_{n}")
                        for n in range(2)
                    ]
                    for k in range(8):
                        for n in range(2):
                            nc.tensor.matmul(
                                psm[n][:],
                                lhsT=w1_sb[k][:, j * P:(j + 1) * P],
                                rhs=xT_bf[k][:, n * 512:(n + 1) * 512],
                                start=(k == 0), stop=(k == 7),
                            )
                    for n in range(2):
                        nc.scalar.activation(
                            hT[:, j * CAP + n * 512: j * CAP + (n + 1) * 512],
                            psm[n][:], AF.Gelu, bias=b1_sb[:, j:j + 1],
                        )

            # ================= PHASE B3: mm2 (+b2, *m) =================
            with (
                tc.tile_pool(name="psM2", bufs=1, space="PSUM") as psM2,
                tc.tile_pool(name="w2p", bufs=3) as w2pool,
                tc.tile_pool(name="outp", bufs=2) as opool,
            ):
                for half in range(2):
                    psos = []
                    for tb in range(4):
                        for db in range(2):
                            psos.append(psM2.tile([P, 512], f32, tag=f"ps_o{tb}{db}", name=f"ps_o{half}{tb}{db}"))
                    for k2 in range(H // P):
                        w2s = w2pool.tile([P, D], bf16, tag="w2s")
                        nc.sync.dma_start(out=w2s[:], in_=w2_d[k2 * P:(k2 + 1) * P, :])
                        for tb in range(4):
                            tokblk = half * 4 + tb
                            for db in range(2):
                                nc.tensor.matmul(
                                    psos[tb * 2 + db][:],
                                    lhsT=hT[:, k2 * CAP + tokblk * P:
                                            k2 * CAP + (tokblk + 1) * P],
                                    rhs=w2s[:, db * 512:(db + 1) * 512],
                                    start=(k2 == 0), stop=False,
                                )
                    # b2 add closes the accumulation group
                    for tb in range(4):
                        tokblk = half * 4 + tb
                        for db in range(2):
                            nc.tensor.matmul(
                                psos[tb * 2 + db][:],
                                lhsT=ones_r[:],
                                rhs=b2_sb[:, db * 512:(db + 1) * 512],
                                start=False, stop=True,
                            )
                        o_sb = opool.tile([P, D], f32, tag="o_sb")
                        for db in range(2):
                            nc.vector.tensor_scalar(
                                out=o_sb[:, db * 512:(db + 1) * 512],
                                in0=psos[tb * 2 + db][:],
                                scalar1=m_all[:, tokblk:tokblk + 1], scalar2=None,
                                op0=OP.mult,
                            )
                        nc.sync.dma_start(
                            out=o_d[tokblk * P:(tokblk + 1) * P, :], in_=o_sb[:]
                        )
    nc.finalize()
    return nc


def _get_nc():
    global _nc_cache
    if _nc_cache is None:
        _nc_cache = build_nc()
    return _nc_cache


def run(x, Wg, W1, b1, W2, b2, trace=False):
    from concourse.bass_utils import run_bass_kernel_spmd

    nc = _get_nc()
    xf = np.ascontiguousarray(np.asarray(x, np.float32).reshape(N, D))
    x_pad = np.concatenate([xf, np.zeros((1, D), np.float32)], 0)
    xT = np.ascontiguousarray(xf.T)
    Wg = np.ascontiguousarray(np.asarray(Wg, np.float32))
    W1b = np.asarray(W1, np.float32).astype(ml_dtypes.bfloat16)
    W2b = np.asarray(W2, np.float32).astype(ml_dtypes.bfloat16)
    b1f = np.asarray(b1, np.float32)
    b2f = np.asarray(b2, np.float32)

    in_maps = []
    for e in range(E):
        in_maps.append({
            "x": x_pad,
            "xt": xT,
            "wg": Wg,
            "w1": np.ascontiguousarray(W1b[e]),
            "b1": np.ascontiguousarray(b1f[e].reshape(H, 1)),
            "w2": np.ascontiguousarray(W2b[e]),
            "b2": np.ascontiguousarray(b2f[e].reshape(D, 1)),
            "ew": np.full((1, 1), float(e * W), np.float32),
        })
    res = run_bass_kernel_spmd(nc, in_maps, core_ids=list(range(E)), trace=trace)

    out = np.zeros((N, D), np.float32)
    for e in range(E):
        r = res.results[e]
        cnt = min(int(round(float(r["counts"][0, e]))), CAP)
        idx = r["glist"][:cnt, 0].astype(np.int64)
        out[idx] = r["o"][:cnt]
    aux = np.float32(res.results[0]["aux"][0, 0])
    return (out.reshape(B, T, D), aux), res


def kernel(x, Wg, W1, b1, W2, b2):
    (out, aux), _ = run(x, Wg, W1, b1, W2, b2, trace=False)
    return out, aux


# revision 29
# speedup vs baseline: 1.5909x; 1.1281x over previous
"""MoE (top-1 routing, E=8 experts) Trainium2 Bass kernel.

Sharding: expert-parallel over 8 NeuronCores. Every core runs a replicated
fp32 router over all N=8192 tokens (reads host-pretransposed xT), builds the
global dispatch table via counting-sort (one triangular-matmul cumsum per
4-tile block + indirect DMA scatter), then computes ONLY its own expert's
capacity-limited MLP in bf16 (fp32 accumulate). Host combine is a pure
index-based scatter of the per-expert compact outputs using device-computed
token indices.

Self-contained: hardcodes shapes from the problem spec.
"""

import numpy as np
import ml_dtypes

B, T, D, H, E = 4, 2048, 1024, 4096, 8
N = B * T            # 8192 tokens
CAP = 1024           # capacity = max(4, ceil(N/E))
W = 2048             # table bucket width (max supported tokens/expert)
P = 128
NT = N // P          # 64 token tiles
NB = NT // 4         # 16 blocks of 4 tiles (512 tokens)
AUX_LOSS_COEF = 0.01
Z_LOSS_COEF = 0.001

_nc_cache = None


def build_nc():
    import concourse.bacc as bacc
    import concourse.tile as tile
    import concourse.bass as bass
    import concourse.mybir as mybir
    from concourse.masks import make_identity, make_upper_triangular

    f32 = mybir.dt.float32
    bf16 = mybir.dt.bfloat16
    i32 = mybir.dt.int32
    AF = mybir.ActivationFunctionType
    OP = mybir.AluOpType
    AX = mybir.AxisListType

    nc = bacc.Bacc(None, target_bir_lowering=False, num_swdge_queues=2)

    # ---- I/O ----
    x_pad = nc.dram_tensor("x", [N + 1, D], f32, kind="ExternalInput")
    xT_d = nc.dram_tensor("xt", [D, N], f32, kind="ExternalInput")
    wg_d = nc.dram_tensor("wg", [D, E], f32, kind="ExternalInput")
    w1_d = nc.dram_tensor("w1", [D, H], bf16, kind="ExternalInput")
    b1_d = nc.dram_tensor("b1", [H, 1], f32, kind="ExternalInput")
    w2_d = nc.dram_tensor("w2", [H, D], bf16, kind="ExternalInput")
    b2_d = nc.dram_tensor("b2", [D, 1], f32, kind="ExternalInput")
    ew_d = nc.dram_tensor("ew", [1, 1], f32, kind="ExternalInput")     # e*W

    o_d = nc.dram_tensor("o", [CAP, D], f32, kind="ExternalOutput")
    glist_d = nc.dram_tensor("glist", [CAP, 1], i32, kind="ExternalOutput")
    counts_d = nc.dram_tensor("counts", [1, E], f32, kind="ExternalOutput")
    aux_d = nc.dram_tensor("aux", [1, 1], f32, kind="ExternalOutput")

    table2 = nc.dram_tensor("table2", [E * W + 2, 2], f32, kind="Internal")
    table2b = nc.dram_tensor("table2b", [E * W + 2, 2], f32, kind="Internal")

    with tile.TileContext(nc) as tc:
        with (
            tc.tile_pool(name="const", bufs=1) as cpool,
            tc.tile_pool(name="resident", bufs=1) as rpool,
            tc.tile_pool(name="xin", bufs=2) as xpool,
            tc.tile_pool(name="small", bufs=4) as spool,
        ):
            # ---------------- constants ----------------
            ident = cpool.tile([P, P], f32)
            make_identity(nc, ident[:])
            ident8 = cpool.tile([8, 8], f32)
            make_identity(nc, ident8[:])
            U128 = cpool.tile([P, P], f32)
            make_upper_triangular(nc, U128[:], val=1.0, diag=True)
            ones_r = cpool.tile([1, P], f32)   # row of ones (K=1 bcast matmuls)
            nc.gpsimd.memset(ones_r[:], 1.0)
            ones_c = cpool.tile([P, 1], f32)   # column of ones (partition reduce)
            nc.gpsimd.memset(ones_c[:], 1.0)
            iota8_i = cpool.tile([P, E], i32)
            nc.gpsimd.iota(iota8_i[:], pattern=[[1, E]], base=0, channel_multiplier=0)
            iota8f = cpool.tile([P, E], f32)
            nc.vector.tensor_copy(iota8f[:], iota8_i[:])
            tok_i = cpool.tile([P, NT], i32)   # tok_i[p, t] = t*128 + p
            nc.gpsimd.iota(tok_i[:], pattern=[[P, NT]], base=0, channel_multiplier=1)
            tokf = cpool.tile([P, NT], f32)
            nc.vector.tensor_copy(tokf[:], tok_i[:])

            wg_sb = cpool.tile([P, E * 8], f32)   # wg strips: col k*8+e
            nc.sync.dma_start(
                out=wg_sb[:].rearrange("p (k e) -> p k e", e=E),
                in_=wg_d[:].rearrange("(k p) e -> p k e", p=P),
            )
            b1_sb = cpool.tile([P, H // P], f32)  # b1[j*128+p] -> [p, j]
            nc.sync.dma_start(
                out=b1_sb[:].rearrange("p j -> p j ()"),
                in_=b1_d[:].rearrange("(j p) o -> p j o", p=P),
            )
            b2_sb = cpool.tile([1, D], f32)
            nc.sync.dma_start(out=b2_sb[:], in_=b2_d[:].rearrange("d o -> o d"))
            ew_sb = cpool.tile([1, 1], f32)
            nc.sync.dma_start(out=ew_sb[:], in_=ew_d[:])

            # accumulators
            pacc4 = rpool.tile([P, 4 * E], f32, tag="pacc4")
            nc.gpsimd.memset(pacc4[:], 0.0)
            base = rpool.tile([1, E], f32, tag="base")
            nc.gpsimd.memset(base[:], 0.0)
            rowmax_all = rpool.tile([P, NT], f32, tag="rowmax_all")
            sumex_all = rpool.tile([P, NT], f32, tag="sumex_all")
            oh_all = rpool.tile([P, NT * E], f32, tag="oh_all")
            pmax_all = rpool.tile([P, NT], f32, tag="pmax_all")
            eidx_all = rpool.tile([P, NT], f32, tag="eidx_all")

            # table2 init to sentinel 8192.0 (one DMA from an SBUF tile)
            sent = rpool.tile([P, E * W * 2 // P], f32, tag="sent")
            nc.gpsimd.memset(sent[:], float(N))
            for tb2 in (table2, table2b):
                nc.sync.dma_start(
                    out=tb2[0:E * W, :].rearrange("(a r) c -> a (r c)", a=P),
                    in_=sent[:],
                )
                nc.sync.dma_start(out=tb2[E * W:E * W + 2, :], in_=sent[0:2, 0:2])

            # resident MLP buffers
            xT_bf = [rpool.tile([P, CAP], bf16, tag=f"xT_{k}", name=f"xT_{k}") for k in range(8)]
            # hT layout: [128 h-partitions, (j, tok) free]: col j*CAP + tok
            hT = rpool.tile([P, (H // P) * CAP], bf16, tag="hT")
            m_all = rpool.tile([P, 8], f32, tag="m_all")
            tf_all = rpool.tile([P, 8], f32, tag="tf_all")
            startsBC = rpool.tile([P, E], f32, tag="startsBC")
            ewBC = rpool.tile([P, 1], f32, tag="ewBC")

            # ================= PHASE A: replicated router =================
            with (
                tc.tile_pool(name="psLT", bufs=2, space="PSUM") as psLT,
                tc.tile_pool(name="psL", bufs=4, space="PSUM") as psL,
                tc.tile_pool(name="psS", bufs=2, space="PSUM") as psS,
                tc.tile_pool(name="xsp", bufs=2) as xsp,
            ):
                for b in range(NB):
                    # load xT strip-block [1024 d, 512 tok] as 8 strips
                    xs = []
                    for k in range(8):
                        s = xsp.tile([P, 512], f32, tag=f"xs{k}", name=f"xs_{b}_{k}")
                        nc.sync.dma_start(
                            out=s[:], in_=xT_d[k * P:(k + 1) * P, b * 512:(b + 1) * 512]
                        )
                        xs.append(s)
                    # logitsT [8, 512] accumulated over k (wg stationary)
                    ps_lt = psLT.tile([8, 512], f32, tag="ps_lt")
                    for k in range(8):
                        nc.tensor.matmul(
                            ps_lt[:], lhsT=wg_sb[:, k * 8:k * 8 + E], rhs=xs[k][:],
                            start=(k == 0), stop=(k == 7),
                        )
                    lt_sb = spool.tile([8, 512], f32, tag="lt_sb")
                    nc.vector.tensor_copy(lt_sb[:], ps_lt[:])
                    # transpose back into [128 tok, (t,8)] layout
                    logits4 = spool.tile([P, 4 * E], f32, tag="logits4")
                    for t in range(4):
                        ps_l = psL.tile([P, E], f32, tag="ps_l")
                        nc.tensor.transpose(ps_l[:], lt_sb[:, t * P:(t + 1) * P], ident8[:])
                        nc.vector.tensor_copy(logits4[:, t * E:(t + 1) * E], ps_l[:])

                    l3 = logits4[:].rearrange("p (t e) -> p t e", e=E)
                    rm4 = rowmax_all[:, b * 4:(b + 1) * 4]
                    nc.vector.tensor_reduce(rm4, l3, axis=AX.X, op=OP.max)
                    rm4b = rowmax_all[:].rearrange("p t -> p t ()")[
                        :, b * 4:(b + 1) * 4, :
                    ].to_broadcast([P, 4, E])
                    sh4 = spool.tile([P, 4 * E], f32, tag="sh4")
                    sh43 = sh4[:].rearrange("p (t e) -> p t e", e=E)
                    nc.vector.tensor_tensor(out=sh43, in0=l3, in1=rm4b, op=OP.subtract)
                    ex4 = spool.tile([P, 4 * E], f32, tag="ex4")
                    nc.scalar.activation(ex4[:], sh4[:], AF.Exp)
                    ex43 = ex4[:].rearrange("p (t e) -> p t e", e=E)
                    se4 = sumex_all[:, b * 4:(b + 1) * 4]
                    nc.vector.tensor_reduce(se4, ex43, axis=AX.X, op=OP.add)
                    rec4 = spool.tile([P, 4], f32, tag="rec4")
                    nc.vector.reciprocal(rec4[:], se4)
                    rec4b = rec4[:].rearrange("p t -> p t ()").to_broadcast([P, 4, E])
                    probs4 = spool.tile([P, 4 * E], f32, tag="probs4")
                    p43 = probs4[:].rearrange("p (t e) -> p t e", e=E)
                    nc.vector.tensor_tensor(out=p43, in0=ex43, in1=rec4b, op=OP.mult)
                    nc.vector.tensor_tensor(out=pacc4[:], in0=pacc4[:], in1=probs4[:], op=OP.add)
                    nc.vector.tensor_reduce(pmax_all[:, b * 4:(b + 1) * 4], p43, axis=AX.X, op=OP.max)

                    # first-argmax per tile: min over masked expert iota
                    iota48 = iota8f[:].rearrange("p e -> p () e").to_broadcast([P, 4, E])
                    eq4 = spool.tile([P, 4 * E], f32, tag="eq4")
                    eq43 = eq4[:].rearrange("p (t e) -> p t e", e=E)
                    nc.vector.tensor_tensor(out=eq43, in0=l3, in1=rm4b, op=OP.is_equal)
                    m14 = spool.tile([P, 4 * E], f32, tag="m14")
                    m143 = m14[:].rearrange("p (t e) -> p t e", e=E)
                    nc.vector.tensor_tensor(out=m143, in0=iota48, in1=eq43, op=OP.mult)
                    m24 = spool.tile([P, 4 * E], f32, tag="m24")
                    nc.vector.tensor_scalar(
                        out=m24[:], in0=eq4[:], scalar1=-9.0, scalar2=9.0,
                        op0=OP.mult, op1=OP.add,
                    )
                    nc.vector.tensor_tensor(out=m14[:], in0=m14[:], in1=m24[:], op=OP.add)
                    eidx4 = eidx_all[:, b * 4:(b + 1) * 4]
                    nc.vector.tensor_reduce(eidx4, m143, axis=AX.X, op=OP.min)
                    eidx4b = eidx_all[:].rearrange("p t -> p t ()")[
                        :, b * 4:(b + 1) * 4, :
                    ].to_broadcast([P, 4, E])
                    oh43 = oh_all[:].rearrange("p (t e) -> p t e", e=E)[
                        :, b * 4:(b + 1) * 4, :
                    ]
                    nc.vector.tensor_tensor(out=oh43, in0=iota48, in1=eidx4b, op=OP.is_equal)

                    # pmax into resident store (for scatter payload later)
                    # (pmax4 already written via pmax_all slice above)

                    if b % (NB // 4) == (NB // 4) - 1:
                        hb = b // (NB // 4)          # which quarter just finished
                        hw = NT // 4                 # 16 tiles per quarter
                        ht0 = hb * hw                # first tile of half
                        c0 = ht0 * E                 # first oh column
                        cw = hw * E                  # 256 columns
                        # tile totals for this half: [1, (t,e)]
                        ps_tt = psS.tile([1, cw], f32, tag="ps_small", name=f"ps_tt{hb}")
                        nc.tensor.matmul(
                            ps_tt[:], lhsT=ones_c[:], rhs=oh_all[:, c0:c0 + cw],
                            start=True, stop=True,
                        )
                        tots = spool.tile([1, cw], f32, tag="tots", name=f"tots{hb}", bufs=2)
                        nc.vector.tensor_copy(tots[:], ps_tt[:])
                        # inclusive prefix over tiles (shift-adds), then exclusive
                        for sh in [1, 2, 4, 8]:
                            nc.vector.tensor_tensor(
                                out=tots[:, sh * E:cw], in0=tots[:, sh * E:cw],
                                in1=tots[:, 0:cw - sh * E], op=OP.add,
                            )
                        # exclusive prefix = inclusive shifted right one tile
                        exclp = spool.tile([1, cw], f32, tag="exclp", name=f"exclp{hb}", bufs=2)
                        nc.vector.memset(exclp[:, 0:E], 0.0)
                        nc.vector.tensor_copy(exclp[:, E:cw], tots[:, 0:cw - E])
                        # baseb[t] = carry base (prev halves) + exclusive prefix
                        baseb = spool.tile([1, cw], f32, tag="baseb", name=f"baseb{hb}", bufs=2)
                        bb3 = baseb[:].rearrange("o (t e) -> o t e", e=E)
                        nc.vector.tensor_tensor(
                            out=bb3,
                            in0=exclp[:].rearrange("o (t e) -> o t e", e=E),
                            in1=base[:].rearrange("o e -> o () e").to_broadcast([1, hw, E]),
                            op=OP.add,
                        )
                        # update global base with this half's grand total (last inclusive)
                        nc.vector.tensor_tensor(
                            out=base[:], in0=base[:], in1=tots[:, cw - E:cw], op=OP.add,
                        )
                        # rank psum = per-tile cumsum + base broadcast (one accum group)
                        ps_rk = psLT.tile([P, cw], f32, tag="ps_lt", name=f"ps_rk{hb}")
                        nc.tensor.matmul(
                            ps_rk[:], lhsT=U128[:], rhs=oh_all[:, c0:c0 + cw],
                            start=True, stop=False,
                        )
                        nc.tensor.matmul(
                            ps_rk[:], lhsT=ones_r[:], rhs=baseb[:],
                            start=False, stop=True,
                        )
                        sel = spool.tile([P, cw], f32, tag="selh", name=f"selh{hb}", bufs=2)
                        nc.vector.tensor_tensor(
                            out=sel[:], in0=ps_rk[:], in1=oh_all[:, c0:c0 + cw], op=OP.mult,
                        )
                        rankh = spool.tile([P, hw], f32, tag="rankh", name=f"rankh{hb}", bufs=2)
                        nc.vector.tensor_reduce(
                            rankh[:], sel[:].rearrange("p (t e) -> p t e", e=E),
                            axis=AX.X, op=OP.add,
                        )
                        nc.vector.tensor_scalar(
                            out=rankh[:], in0=rankh[:], scalar1=-1.0, scalar2=None, op0=OP.add
                        )
                        # keys = eidx*W + rank, clamped to trash row if rank >= W
                        keyh = spool.tile([P, hw], f32, tag="keyh", name=f"keyh{hb}", bufs=2)
                        nc.vector.tensor_scalar(
                            out=keyh[:], in0=eidx_all[:, ht0:ht0 + hw],
                            scalar1=float(W), scalar2=None, op0=OP.mult,
                        )
                        nc.vector.tensor_tensor(out=keyh[:], in0=keyh[:], in1=rankh[:], op=OP.add)
                        okh = spool.tile([P, hw], f32, tag="okh", name=f"okh{hb}", bufs=2)
                        nc.vector.tensor_scalar(
                            out=okh[:], in0=rankh[:], scalar1=float(W), scalar2=None, op0=OP.is_lt
                        )
                        nc.vector.tensor_tensor(out=keyh[:], in0=keyh[:], in1=okh[:], op=OP.mult)
                        nc.vector.tensor_scalar(
                            out=okh[:], in0=okh[:], scalar1=-float(E * W), scalar2=float(E * W),
                            op0=OP.mult, op1=OP.add,
                        )
                        nc.vector.tensor_tensor(out=keyh[:], in0=keyh[:], in1=okh[:], op=OP.add)
                        keyih = spool.tile([P, hw], i32, tag="keyih", name=f"keyih{hb}", bufs=2)
                        nc.vector.tensor_copy(keyih[:], keyh[:])
                        comboh = spool.tile([P, hw * 2], f32, tag="comboh", name=f"comboh{hb}", bufs=2)
                        ch3 = comboh[:].rearrange("p (t c) -> p t c", c=2)
                        nc.vector.tensor_copy(
                            ch3[:, :, 0:1],
                            pmax_all[:, ht0:ht0 + hw].rearrange("p t -> p t ()"),
                        )
                        nc.vector.tensor_copy(
                            ch3[:, :, 1:2],
                            tokf[:, ht0:ht0 + hw].rearrange("p t -> p t ()"),
                        )
                        for tt in range(hw):
                            tgt = table2 if (ht0 + tt) % 2 == 0 else table2b
                            nc.gpsimd.indirect_dma_start(
                                out=tgt[:],
                                out_offset=bass.IndirectOffsetOnAxis(
                                    ap=keyih[:, tt:tt + 1], axis=0),
                                in_=comboh[:, 2 * tt:2 * tt + 2],
                                in_offset=None,
                            )

                # ---- counts / starts / z-loss tail / aux ----
                nc.sync.dma_start(out=counts_d[:], in_=base[:])
                c1 = spool.tile([1, E], f32, tag="c1")
                nc.vector.tensor_copy(c1[:], base[:])
                nc.vector.tensor_tensor(out=c1[:, 1:E], in0=base[:, 1:E], in1=base[:, 0:E - 1], op=OP.add)
                c2 = spool.tile([1, E], f32, tag="c2")
                nc.vector.tensor_copy(c2[:], c1[:])
                nc.vector.tensor_tensor(out=c2[:, 2:E], in0=c1[:, 2:E], in1=c1[:, 0:E - 2], op=OP.add)
                c3 = spool.tile([1, E], f32, tag="c3")
                nc.vector.tensor_copy(c3[:], c2[:])
                nc.vector.tensor_tensor(out=c3[:, 4:E], in0=c2[:, 4:E], in1=c2[:, 0:E - 4], op=OP.add)
                excl = spool.tile([1, E], f32, tag="excl")
                nc.vector.tensor_tensor(out=excl[:], in0=c3[:], in1=base[:], op=OP.subtract)

                # z-loss: lse = rowmax + ln(sumex), batched over all 64 tiles
                lns = spool.tile([P, NT], f32, tag="lns")
                nc.scalar.activation(lns[:], sumex_all[:], AF.Ln)
                nc.vector.tensor_tensor(out=lns[:], in0=lns[:], in1=rowmax_all[:], op=OP.add)
                nc.vector.tensor_tensor(out=lns[:], in0=lns[:], in1=lns[:], op=OP.mult)
                zrow = spool.tile([P, 1], f32, tag="zrow")
                nc.vector.tensor_reduce(zrow[:], lns[:], axis=AX.X, op=OP.add)
                # fold pacc4 -> [P, E]
                pacc = spool.tile([P, E], f32, tag="pacc")
                nc.vector.tensor_tensor(out=pacc[:], in0=pacc4[:, 0:E], in1=pacc4[:, E:2 * E], op=OP.add)
                nc.vector.tensor_tensor(out=pacc[:], in0=pacc[:], in1=pacc4[:, 2 * E:3 * E], op=OP.add)
                nc.vector.tensor_tensor(out=pacc[:], in0=pacc[:], in1=pacc4[:, 3 * E:4 * E], op=OP.add)

                ps_z = psS.tile([1, 1], f32, tag="ps_small", name="ps_z")
                nc.tensor.matmul(ps_z[:], lhsT=zrow[:], rhs=ones_c[:], start=True, stop=True)
                ps_p = psS.tile([1, E], f32, tag="ps_small", name="ps_p")
                nc.tensor.matmul(ps_p[:], lhsT=ones_c[:], rhs=pacc[:], start=True, stop=True)
                fp = spool.tile([1, E], f32, tag="fp")
                nc.vector.tensor_tensor(out=fp[:], in0=ps_p[:], in1=base[:], op=OP.mult)
                auxv = spool.tile([1, 1], f32, tag="auxv")
                nc.vector.tensor_reduce(auxv[:], fp[:], axis=AX.X, op=OP.add)
                nc.vector.tensor_scalar(
                    out=auxv[:], in0=auxv[:],
                    scalar1=AUX_LOSS_COEF * E / (float(N) * float(N)), scalar2=None,
                    op0=OP.mult,
                )
                zv = spool.tile([1, 1], f32, tag="zv")
                nc.vector.tensor_scalar(
                    out=zv[:], in0=ps_z[:], scalar1=Z_LOSS_COEF / float(N), scalar2=None,
                    op0=OP.mult,
                )
                nc.vector.tensor_tensor(out=auxv[:], in0=auxv[:], in1=zv[:], op=OP.add)
                nc.sync.dma_start(out=aux_d[:], in_=auxv[:])

                # broadcasts for phase B
                ps_sb2 = psS.tile([P, E], f32, tag="ps_small", name="ps_sb2")
                nc.tensor.matmul(ps_sb2[:], lhsT=ones_r[:], rhs=excl[:], start=True, stop=True)
                nc.vector.tensor_copy(startsBC[:], ps_sb2[:])
                ps_s2 = psS.tile([P, 1], f32, tag="ps_small", name="ps_s2")
                nc.tensor.matmul(ps_s2[:], lhsT=ones_r[:], rhs=ew_sb[:], start=True, stop=True)
                nc.vector.tensor_copy(ewBC[:], ps_s2[:])

            # ================= PHASE B1 + mm1 =================
            with (
                tc.tile_pool(name="psT", bufs=2, space="PSUM") as psT,
                tc.tile_pool(name="psM1", bufs=4, space="PSUM") as psM1,
                tc.tile_pool(name="w1p", bufs=1) as w1p,
            ):
                w1_sb = [w1p.tile([P, H], bf16, tag=f"w1_{k}", name=f"w1_{k}") for k in range(8)]
                for k in range(8):
                    nc.sync.dma_start(out=w1_sb[k][:], in_=w1_d[k * P:(k + 1) * P, :])
                for c in range(8):
                    s_f = tokf[:, c:c + 1]  # slot ids c*128+p
                    k1f = spool.tile([P, 1], f32, tag="k1f")
                    nc.vector.tensor_tensor(out=k1f[:], in0=ewBC[:], in1=s_f, op=OP.add)
                    nc.vector.tensor_scalar(
                        out=k1f[:], in0=k1f[:], scalar1=float(E * W - 1), scalar2=None,
                        op0=OP.min,
                    )
                    k1i = spool.tile([P, 1], i32, tag="k1i")
                    nc.vector.tensor_copy(k1i[:], k1f[:])
                    g1 = spool.tile([P, 2], f32, tag="g1")
                    nc.gpsimd.indirect_dma_start(
                        out=g1[:], out_offset=None,
                        in_=table2[:],
                        in_offset=bass.IndirectOffsetOnAxis(ap=k1i[:, 0:1], axis=0),
                    )
                    g1b = spool.tile([P, 2], f32, tag="g1b")
                    nc.gpsimd.indirect_dma_start(
                        out=g1b[:], out_offset=None,
                        in_=table2b[:],
                        in_offset=bass.IndirectOffsetOnAxis(ap=k1i[:, 0:1], axis=0),
                    )
                    sentm = spool.tile([P, 2], i32, tag="sentm")
                    nc.vector.tensor_scalar(
                        out=sentm[:], in0=g1[:], scalar1=float(N), scalar2=None,
                        op0=OP.is_equal,
                    )
                    nc.vector.copy_predicated(g1[:], sentm[:], g1b[:])
                    ti = spool.tile([P, 1], i32, tag="ti")
                    nc.vector.tensor_copy(ti[:], g1[:, 1:2])
                    nc.vector.tensor_copy(tf_all[:, c:c + 1], g1[:, 1:2])
                    nc.sync.dma_start(out=glist_d[c * P:(c + 1) * P, :], in_=ti[:])
                    xg = xpool.tile([P, D], f32, tag="xg")
                    nc.gpsimd.indirect_dma_start(
                        out=xg[:], out_offset=None,
                        in_=x_pad[:],
                        in_offset=bass.IndirectOffsetOnAxis(ap=ti[:, 0:1], axis=0),
                    )
                    # transpose gathered rows into bf16 xT strips
                    for k in range(8):
                        pst = psT.tile([P, P], f32)
                        nc.tensor.transpose(pst[:], xg[:, k * P:(k + 1) * P], ident[:])
                        nc.vector.tensor_copy(xT_bf[k][:, c * P:(c + 1) * P], pst[:])

                # m-lookup chain, off the mm1 critical path (feeds only mm2 scaling)
                for c in range(8):
                    # sorted position p = token id -> bucket -> table col0
                    pf = spool.tile([P, 1], f32, tag="pf")
                    nc.vector.tensor_copy(pf[:], tf_all[:, c:c + 1])
                    cmp = spool.tile([P, E], f32, tag="cmp")
                    nc.vector.tensor_tensor(
                        out=cmp[:], in0=pf[:].to_broadcast([P, E]), in1=startsBC[:],
                        op=OP.is_ge,
                    )
                    ehat = spool.tile([P, 1], f32, tag="ehat")
                    nc.vector.tensor_reduce(ehat[:], cmp[:], axis=AX.X, op=OP.add)
                    nc.vector.tensor_scalar(
                        out=ehat[:], in0=ehat[:], scalar1=-1.0, scalar2=None, op0=OP.add
                    )
                    oh8 = spool.tile([P, E], f32, tag="oh8")
                    nc.vector.tensor_scalar(
                        out=oh8[:], in0=iota8f[:], scalar1=ehat[:], scalar2=None,
                        op0=OP.is_equal,
                    )
                    sts = spool.tile([P, E], f32, tag="sts")
                    nc.vector.tensor_tensor(out=sts[:], in0=startsBC[:], in1=oh8[:], op=OP.mult)
                    stsel = spool.tile([P, 1], f32, tag="stsel")
                    nc.vector.tensor_reduce(stsel[:], sts[:], axis=AX.X, op=OP.add)
                    rr = spool.tile([P, 1], f32, tag="rr")
                    nc.vector.tensor_tensor(out=rr[:], in0=pf[:], in1=stsel[:], op=OP.subtract)
                    k2f = spool.tile([P, 1], f32, tag="k2f")
                    nc.vector.tensor_scalar(
                        out=k2f[:], in0=ehat[:], scalar1=float(W), scalar2=None, op0=OP.mult
                    )
                    nc.vector.tensor_tensor(out=k2f[:], in0=k2f[:], in1=rr[:], op=OP.add)
                    nc.vector.tensor_scalar(
                        out=k2f[:], in0=k2f[:], scalar1=float(E * W + 1), scalar2=None,
                        op0=OP.min,
                    )
                    k2i = spool.tile([P, 1], i32, tag="k2i")
                    nc.vector.tensor_copy(k2i[:], k2f[:])
                    g2 = spool.tile([P, 2], f32, tag="g2")
                    nc.gpsimd.indirect_dma_start(
                        out=g2[:], out_offset=None,
                        in_=table2[:],
                        in_offset=bass.IndirectOffsetOnAxis(ap=k2i[:, 0:1], axis=0),
                    )
                    g2b = spool.tile([P, 2], f32, tag="g2b")
                    nc.gpsimd.indirect_dma_start(
                        out=g2b[:], out_offset=None,
                        in_=table2b[:],
                        in_offset=bass.IndirectOffsetOnAxis(ap=k2i[:, 0:1], axis=0),
                    )
                    sentm2 = spool.tile([P, 2], i32, tag="sentm2")
                    nc.vector.tensor_scalar(
                        out=sentm2[:], in0=g2[:], scalar1=float(N), scalar2=None,
                        op0=OP.is_equal,
                    )
                    nc.vector.copy_predicated(g2[:], sentm2[:], g2b[:])
                    nc.vector.tensor_copy(m_all[:, c:c + 1], g2[:, 0:1])

                # ---- mm1 + gelu: hT[j] = gelu(x @ W1 + b1), stationary reused over n
                for j in range(H // P):
                    psm = [
                        psM1.tile([P, 512], f32, tag="ps_mm1", name=f"psm_{j}_{n}")
                        for n in range(2)
                    ]
                    for k in range(8):
                        for n in range(2):
                            nc.tensor.matmul(
                                psm[n][:],
                                lhsT=w1_sb[k][:, j * P:(j + 1) * P],
                                rhs=xT_bf[k][:, n * 512:(n + 1) * 512],
                                start=(k == 0), stop=(k == 7),
                            )
                    for n in range(2):
                        nc.scalar.activation(
                            hT[:, j * CAP + n * 512: j * CAP + (n + 1) * 512],
                            psm[n][:], AF.Gelu, bias=b1_sb[:, j:j + 1],
                        )

            # ================= PHASE B3: mm2 (+b2, *m) =================
            with (
                tc.tile_pool(name="psM2", bufs=1, space="PSUM") as psM2,
                tc.tile_pool(name="w2p", bufs=3) as w2pool,
                tc.tile_pool(name="outp", bufs=2) as opool,
            ):
                for half in range(2):
                    psos = []
                    for tb in range(4):
                        for db in range(2):
                            psos.append(psM2.tile([P, 512], f32, tag=f"ps_o{tb}{db}", name=f"ps_o{half}{tb}{db}"))
                    for k2 in range(H // P):
                        w2s = w2pool.tile([P, D], bf16, tag="w2s")
                        nc.sync.dma_start(out=w2s[:], in_=w2_d[k2 * P:(k2 + 1) * P, :])
                        for tb in range(4):
                            tokblk = half * 4 + tb
                            for db in range(2):
                                nc.tensor.matmul(
                                    psos[tb * 2 + db][:],
                                    lhsT=hT[:, k2 * CAP + tokblk * P:
                                            k2 * CAP + (tokblk + 1) * P],
                                    rhs=w2s[:, db * 512:(db + 1) * 512],
                                    start=(k2 == 0), stop=False,
                                )
                    # b2 add closes the accumulation group
                    for tb in range(4):
                        tokblk = half * 4 + tb
                        for db in range(2):
                            nc.tensor.matmul(
                                psos[tb * 2 + db][:],
                                lhsT=ones_r[:],
                                rhs=b2_sb[:, db * 512:(db + 1) * 512],
                                start=False, stop=True,
                            )
                        o_sb = opool.tile([P, D], f32, tag="o_sb")
                        for db in range(2):
                            nc.vector.tensor_scalar(
                                out=o_sb[:, db * 512:(db + 1) * 512],
                                in0=psos[tb * 2 + db][:],
                                scalar1=m_all[:, tokblk:tokblk + 1], scalar2=None,
                                op0=OP.mult,
                            )
                        nc.sync.dma_start(
                            out=o_d[tokblk * P:(tokblk + 1) * P, :], in_=o_sb[:]
                        )
    nc.finalize()
    return nc


def _get_nc():
    global _nc_cache
    if _nc_cache is None:
        _nc_cache = build_nc()
    return _nc_cache


def run(x, Wg, W1, b1, W2, b2, trace=False):
    from concourse.bass_utils import run_bass_kernel_spmd

    nc = _get_nc()
    xf = np.ascontiguousarray(np.asarray(x, np.float32).reshape(N, D))
    x_pad = np.concatenate([xf, np.zeros((1, D), np.float32)], 0)
    xT = np.ascontiguousarray(xf.T)
    Wg = np.ascontiguousarray(np.asarray(Wg, np.float32))
    W1b = np.asarray(W1, np.float32).astype(ml_dtypes.bfloat16)
    W2b = np.asarray(W2, np.float32).astype(ml_dtypes.bfloat16)
    b1f = np.asarray(b1, np.float32)
    b2f = np.asarray(b2, np.float32)

    in_maps = []
    for e in range(E):
        in_maps.append({
            "x": x_pad,
            "xt": xT,
            "wg": Wg,
            "w1": np.ascontiguousarray(W1b[e]),
            "b1": np.ascontiguousarray(b1f[e].reshape(H, 1)),
            "w2": np.ascontiguousarray(W2b[e]),
            "b2": np.ascontiguousarray(b2f[e].reshape(D, 1)),
            "ew": np.full((1, 1), float(e * W), np.float32),
        })
    res = run_bass_kernel_spmd(nc, in_maps, core_ids=list(range(E)), trace=trace)

    out = np.zeros((N, D), np.float32)
    for e in range(E):
        r = res.results[e]
        cnt = min(int(round(float(r["counts"][0, e]))), CAP)
        idx = r["glist"][:cnt, 0].astype(np.int64)
        out[idx] = r["o"][:cnt]
    aux = np.float32(res.results[0]["aux"][0, 0])
    return (out.reshape(B, T, D), aux), res


def kernel(x, Wg, W1, b1, W2, b2):
    (out, aux), _ = run(x, Wg, W1, b1, W2, b2, trace=False)
    return out, aux


# revision 30
# speedup vs baseline: 1.6479x; 1.0359x over previous
"""MoE (top-1 routing, E=8 experts) Trainium2 Bass kernel.

Sharding: expert-parallel over 8 NeuronCores. Every core runs a replicated
fp32 router over all N=8192 tokens (reads host-pretransposed xT), builds the
global dispatch table via counting-sort (one triangular-matmul cumsum per
4-tile block + indirect DMA scatter), then computes ONLY its own expert's
capacity-limited MLP in bf16 (fp32 accumulate). Host combine is a pure
index-based scatter of the per-expert compact outputs using device-computed
token indices.

Self-contained: hardcodes shapes from the problem spec.
"""

import numpy as np
import ml_dtypes

B, T, D, H, E = 4, 2048, 1024, 4096, 8
N = B * T            # 8192 tokens
CAP = 1024           # capacity = max(4, ceil(N/E))
W = 2048             # table bucket width (max supported tokens/expert)
P = 128
NT = N // P          # 64 token tiles
NB = NT // 4         # 16 blocks of 4 tiles (512 tokens)
AUX_LOSS_COEF = 0.01
Z_LOSS_COEF = 0.001

_nc_cache = None


def build_nc():
    import concourse.bacc as bacc
    import concourse.tile as tile
    import concourse.bass as bass
    import concourse.mybir as mybir
    from concourse.masks import make_identity, make_upper_triangular

    f32 = mybir.dt.float32
    bf16 = mybir.dt.bfloat16
    i32 = mybir.dt.int32
    AF = mybir.ActivationFunctionType
    OP = mybir.AluOpType
    AX = mybir.AxisListType

    nc = bacc.Bacc(None, target_bir_lowering=False, num_swdge_queues=2)

    # ---- I/O ----
    x_pad = nc.dram_tensor("x", [N + 1, D], f32, kind="ExternalInput")
    xT_d = nc.dram_tensor("xt", [D, N], f32, kind="ExternalInput")
    wg_d = nc.dram_tensor("wg", [D, E], f32, kind="ExternalInput")
    w1_d = nc.dram_tensor("w1", [D, H], bf16, kind="ExternalInput")
    b1_d = nc.dram_tensor("b1", [H, 1], f32, kind="ExternalInput")
    w2_d = nc.dram_tensor("w2", [H, D], bf16, kind="ExternalInput")
    b2_d = nc.dram_tensor("b2", [D, 1], f32, kind="ExternalInput")
    ew_d = nc.dram_tensor("ew", [1, 1], f32, kind="ExternalInput")     # e*W

    o_d = nc.dram_tensor("o", [CAP, D], f32, kind="ExternalOutput")
    glist_d = nc.dram_tensor("glist", [CAP, 1], i32, kind="ExternalOutput")
    counts_d = nc.dram_tensor("counts", [1, E], f32, kind="ExternalOutput")
    aux_d = nc.dram_tensor("aux", [1, 1], f32, kind="ExternalOutput")

    table2 = nc.dram_tensor("table2", [E * W + 2, 2], f32, kind="Internal")
    table2b = nc.dram_tensor("table2b", [E * W + 2, 2], f32, kind="Internal")
    table2c = nc.dram_tensor("table2c", [E * W + 2, 2], f32, kind="Internal")
    table2d = nc.dram_tensor("table2d", [E * W + 2, 2], f32, kind="Internal")

    with tile.TileContext(nc) as tc:
        with (
            tc.tile_pool(name="const", bufs=1) as cpool,
            tc.tile_pool(name="resident", bufs=1) as rpool,
            tc.tile_pool(name="xin", bufs=2) as xpool,
            tc.tile_pool(name="small", bufs=4) as spool,
        ):
            # ---------------- constants ----------------
            ident = cpool.tile([P, P], f32)
            make_identity(nc, ident[:])
            ident8 = cpool.tile([8, 8], f32)
            make_identity(nc, ident8[:])
            U128 = cpool.tile([P, P], f32)
            make_upper_triangular(nc, U128[:], val=1.0, diag=True)
            ones_r = cpool.tile([1, P], f32)   # row of ones (K=1 bcast matmuls)
            nc.gpsimd.memset(ones_r[:], 1.0)
            ones_c = cpool.tile([P, 1], f32)   # column of ones (partition reduce)
            nc.gpsimd.memset(ones_c[:], 1.0)
            iota8_i = cpool.tile([P, E], i32)
            nc.gpsimd.iota(iota8_i[:], pattern=[[1, E]], base=0, channel_multiplier=0)
            iota8f = cpool.tile([P, E], f32)
            nc.vector.tensor_copy(iota8f[:], iota8_i[:])
            tok_i = cpool.tile([P, NT], i32)   # tok_i[p, t] = t*128 + p
            nc.gpsimd.iota(tok_i[:], pattern=[[P, NT]], base=0, channel_multiplier=1)
            tokf = cpool.tile([P, NT], f32)
            nc.vector.tensor_copy(tokf[:], tok_i[:])

            wg_sb = cpool.tile([P, E * 8], f32)   # wg strips: col k*8+e
            nc.sync.dma_start(
                out=wg_sb[:].rearrange("p (k e) -> p k e", e=E),
                in_=wg_d[:].rearrange("(k p) e -> p k e", p=P),
            )
            b1_sb = cpool.tile([P, H // P], f32)  # b1[j*128+p] -> [p, j]
            nc.sync.dma_start(
                out=b1_sb[:].rearrange("p j -> p j ()"),
                in_=b1_d[:].rearrange("(j p) o -> p j o", p=P),
            )
            b2_sb = cpool.tile([1, D], f32)
            nc.sync.dma_start(out=b2_sb[:], in_=b2_d[:].rearrange("d o -> o d"))
            ew_sb = cpool.tile([1, 1], f32)
            nc.sync.dma_start(out=ew_sb[:], in_=ew_d[:])

            # accumulators
            pacc4 = rpool.tile([P, 4 * E], f32, tag="pacc4")
            nc.gpsimd.memset(pacc4[:], 0.0)
            base = rpool.tile([1, E], f32, tag="base")
            nc.gpsimd.memset(base[:], 0.0)
            rowmax_all = rpool.tile([P, NT], f32, tag="rowmax_all")
            sumex_all = rpool.tile([P, NT], f32, tag="sumex_all")
            oh_all = rpool.tile([P, NT * E], f32, tag="oh_all")
            pmax_all = rpool.tile([P, NT], f32, tag="pmax_all")
            eidx_all = rpool.tile([P, NT], f32, tag="eidx_all")

            # table2 init to sentinel 8192.0 (one DMA from an SBUF tile)
            sent = rpool.tile([P, E * W * 2 // P], f32, tag="sent")
            nc.gpsimd.memset(sent[:], float(N))
            for tb2 in (table2, table2b, table2c, table2d):
                nc.sync.dma_start(
                    out=tb2[0:E * W, :].rearrange("(a r) c -> a (r c)", a=P),
                    in_=sent[:],
                )
                nc.sync.dma_start(out=tb2[E * W:E * W + 2, :], in_=sent[0:2, 0:2])

            # resident MLP buffers
            xT_bf = [rpool.tile([P, CAP], bf16, tag=f"xT_{k}", name=f"xT_{k}") for k in range(8)]
            # hT layout: [128 h-partitions, (j, tok) free]: col j*CAP + tok
            hT = rpool.tile([P, (H // P) * CAP], bf16, tag="hT")
            m_all = rpool.tile([P, 8], f32, tag="m_all")
            tf_all = rpool.tile([P, 8], f32, tag="tf_all")
            startsBC = rpool.tile([P, E], f32, tag="startsBC")
            ewBC = rpool.tile([P, 1], f32, tag="ewBC")

            # ================= PHASE A: replicated router =================
            with (
                tc.tile_pool(name="psLT", bufs=2, space="PSUM") as psLT,
                tc.tile_pool(name="psL", bufs=4, space="PSUM") as psL,
                tc.tile_pool(name="psS", bufs=2, space="PSUM") as psS,
                tc.tile_pool(name="xsp", bufs=2) as xsp,
            ):
                for b in range(NB):
                    # load xT strip-block [1024 d, 512 tok] as 8 strips
                    xs = []
                    for k in range(8):
                        s = xsp.tile([P, 512], f32, tag=f"xs{k}", name=f"xs_{b}_{k}")
                        nc.sync.dma_start(
                            out=s[:], in_=xT_d[k * P:(k + 1) * P, b * 512:(b + 1) * 512]
                        )
                        xs.append(s)
                    # logitsT [8, 512] accumulated over k (wg stationary)
                    ps_lt = psLT.tile([8, 512], f32, tag="ps_lt")
                    for k in range(8):
                        nc.tensor.matmul(
                            ps_lt[:], lhsT=wg_sb[:, k * 8:k * 8 + E], rhs=xs[k][:],
                            start=(k == 0), stop=(k == 7),
                        )
                    lt_sb = spool.tile([8, 512], f32, tag="lt_sb")
                    nc.vector.tensor_copy(lt_sb[:], ps_lt[:])
                    # transpose back into [128 tok, (t,8)] layout
                    logits4 = spool.tile([P, 4 * E], f32, tag="logits4")
                    for t in range(4):
                        ps_l = psL.tile([P, E], f32, tag="ps_l")
                        nc.tensor.transpose(ps_l[:], lt_sb[:, t * P:(t + 1) * P], ident8[:])
                        nc.vector.tensor_copy(logits4[:, t * E:(t + 1) * E], ps_l[:])

                    l3 = logits4[:].rearrange("p (t e) -> p t e", e=E)
                    rm4 = rowmax_all[:, b * 4:(b + 1) * 4]
                    nc.vector.tensor_reduce(rm4, l3, axis=AX.X, op=OP.max)
                    rm4b = rowmax_all[:].rearrange("p t -> p t ()")[
                        :, b * 4:(b + 1) * 4, :
                    ].to_broadcast([P, 4, E])
                    sh4 = spool.tile([P, 4 * E], f32, tag="sh4")
                    sh43 = sh4[:].rearrange("p (t e) -> p t e", e=E)
                    nc.vector.tensor_tensor(out=sh43, in0=l3, in1=rm4b, op=OP.subtract)
                    ex4 = spool.tile([P, 4 * E], f32, tag="ex4")
                    nc.scalar.activation(ex4[:], sh4[:], AF.Exp)
                    ex43 = ex4[:].rearrange("p (t e) -> p t e", e=E)
                    se4 = sumex_all[:, b * 4:(b + 1) * 4]
                    nc.vector.tensor_reduce(se4, ex43, axis=AX.X, op=OP.add)
                    rec4 = spool.tile([P, 4], f32, tag="rec4")
                    nc.vector.reciprocal(rec4[:], se4)
                    rec4b = rec4[:].rearrange("p t -> p t ()").to_broadcast([P, 4, E])
                    probs4 = spool.tile([P, 4 * E], f32, tag="probs4")
                    p43 = probs4[:].rearrange("p (t e) -> p t e", e=E)
                    nc.vector.tensor_tensor(out=p43, in0=ex43, in1=rec4b, op=OP.mult)
                    nc.vector.tensor_tensor(out=pacc4[:], in0=pacc4[:], in1=probs4[:], op=OP.add)
                    nc.vector.tensor_reduce(pmax_all[:, b * 4:(b + 1) * 4], p43, axis=AX.X, op=OP.max)

                    # first-argmax per tile: min over masked expert iota
                    iota48 = iota8f[:].rearrange("p e -> p () e").to_broadcast([P, 4, E])
                    eq4 = spool.tile([P, 4 * E], f32, tag="eq4")
                    eq43 = eq4[:].rearrange("p (t e) -> p t e", e=E)
                    nc.vector.tensor_tensor(out=eq43, in0=l3, in1=rm4b, op=OP.is_equal)
                    m14 = spool.tile([P, 4 * E], f32, tag="m14")
                    m143 = m14[:].rearrange("p (t e) -> p t e", e=E)
                    nc.vector.tensor_tensor(out=m143, in0=iota48, in1=eq43, op=OP.mult)
                    m24 = spool.tile([P, 4 * E], f32, tag="m24")
                    nc.vector.tensor_scalar(
                        out=m24[:], in0=eq4[:], scalar1=-9.0, scalar2=9.0,
                        op0=OP.mult, op1=OP.add,
                    )
                    nc.vector.tensor_tensor(out=m14[:], in0=m14[:], in1=m24[:], op=OP.add)
                    eidx4 = eidx_all[:, b * 4:(b + 1) * 4]
                    nc.vector.tensor_reduce(eidx4, m143, axis=AX.X, op=OP.min)
                    eidx4b = eidx_all[:].rearrange("p t -> p t ()")[
                        :, b * 4:(b + 1) * 4, :
                    ].to_broadcast([P, 4, E])
                    oh43 = oh_all[:].rearrange("p (t e) -> p t e", e=E)[
                        :, b * 4:(b + 1) * 4, :
                    ]
                    nc.vector.tensor_tensor(out=oh43, in0=iota48, in1=eidx4b, op=OP.is_equal)

                    # pmax into resident store (for scatter payload later)
                    # (pmax4 already written via pmax_all slice above)

                    if b % (NB // 4) == (NB // 4) - 1:
                        hb = b // (NB // 4)          # which quarter just finished
                        hw = NT // 4                 # 16 tiles per quarter
                        ht0 = hb * hw                # first tile of half
                        c0 = ht0 * E                 # first oh column
                        cw = hw * E                  # 256 columns
                        # tile totals for this half: [1, (t,e)]
                        ps_tt = psS.tile([1, cw], f32, tag="ps_small", name=f"ps_tt{hb}")
                        nc.tensor.matmul(
                            ps_tt[:], lhsT=ones_c[:], rhs=oh_all[:, c0:c0 + cw],
                            start=True, stop=True,
                        )
                        tots = spool.tile([1, cw], f32, tag="tots", name=f"tots{hb}", bufs=2)
                        nc.vector.tensor_copy(tots[:], ps_tt[:])
                        # inclusive prefix over tiles (shift-adds), then exclusive
                        for sh in [1, 2, 4, 8]:
                            nc.vector.tensor_tensor(
                                out=tots[:, sh * E:cw], in0=tots[:, sh * E:cw],
                                in1=tots[:, 0:cw - sh * E], op=OP.add,
                            )
                        # exclusive prefix = inclusive shifted right one tile
                        exclp = spool.tile([1, cw], f32, tag="exclp", name=f"exclp{hb}", bufs=2)
                        nc.vector.memset(exclp[:, 0:E], 0.0)
                        nc.vector.tensor_copy(exclp[:, E:cw], tots[:, 0:cw - E])
                        # baseb[t] = carry base (prev halves) + exclusive prefix
                        baseb = spool.tile([1, cw], f32, tag="baseb", name=f"baseb{hb}", bufs=2)
                        bb3 = baseb[:].rearrange("o (t e) -> o t e", e=E)
                        nc.vector.tensor_tensor(
                            out=bb3,
                            in0=exclp[:].rearrange("o (t e) -> o t e", e=E),
                            in1=base[:].rearrange("o e -> o () e").to_broadcast([1, hw, E]),
                            op=OP.add,
                        )
                        # update global base with this half's grand total (last inclusive)
                        nc.vector.tensor_tensor(
                            out=base[:], in0=base[:], in1=tots[:, cw - E:cw], op=OP.add,
                        )
                        # rank psum = per-tile cumsum + base broadcast (one accum group)
                        ps_rk = psLT.tile([P, cw], f32, tag="ps_lt", name=f"ps_rk{hb}")
                        nc.tensor.matmul(
                            ps_rk[:], lhsT=U128[:], rhs=oh_all[:, c0:c0 + cw],
                            start=True, stop=False,
                        )
                        nc.tensor.matmul(
                            ps_rk[:], lhsT=ones_r[:], rhs=baseb[:],
                            start=False, stop=True,
                        )
                        sel = spool.tile([P, cw], f32, tag="selh", name=f"selh{hb}", bufs=2)
                        nc.vector.tensor_tensor(
                            out=sel[:], in0=ps_rk[:], in1=oh_all[:, c0:c0 + cw], op=OP.mult,
                        )
                        rankh = spool.tile([P, hw], f32, tag="rankh", name=f"rankh{hb}", bufs=2)
                        nc.vector.tensor_reduce(
                            rankh[:], sel[:].rearrange("p (t e) -> p t e", e=E),
                            axis=AX.X, op=OP.add,
                        )
                        nc.vector.tensor_scalar(
                            out=rankh[:], in0=rankh[:], scalar1=-1.0, scalar2=None, op0=OP.add
                        )
                        # keys = eidx*W + rank, clamped to trash row if rank >= W
                        keyh = spool.tile([P, hw], f32, tag="keyh", name=f"keyh{hb}", bufs=2)
                        nc.vector.tensor_scalar(
                            out=keyh[:], in0=eidx_all[:, ht0:ht0 + hw],
                            scalar1=float(W), scalar2=None, op0=OP.mult,
                        )
                        nc.vector.tensor_tensor(out=keyh[:], in0=keyh[:], in1=rankh[:], op=OP.add)
                        okh = spool.tile([P, hw], f32, tag="okh", name=f"okh{hb}", bufs=2)
                        nc.vector.tensor_scalar(
                            out=okh[:], in0=rankh[:], scalar1=float(W), scalar2=None, op0=OP.is_lt
                        )
                        nc.vector.tensor_tensor(out=keyh[:], in0=keyh[:], in1=okh[:], op=OP.mult)
                        nc.vector.tensor_scalar(
                            out=okh[:], in0=okh[:], scalar1=-float(E * W), scalar2=float(E * W),
                            op0=OP.mult, op1=OP.add,
                        )
                        nc.vector.tensor_tensor(out=keyh[:], in0=keyh[:], in1=okh[:], op=OP.add)
                        keyih = spool.tile([P, hw], i32, tag="keyih", name=f"keyih{hb}", bufs=2)
                        nc.vector.tensor_copy(keyih[:], keyh[:])
                        comboh = spool.tile([P, hw * 2], f32, tag="comboh", name=f"comboh{hb}", bufs=2)
                        ch3 = comboh[:].rearrange("p (t c) -> p t c", c=2)
                        nc.vector.tensor_copy(
                            ch3[:, :, 0:1],
                            pmax_all[:, ht0:ht0 + hw].rearrange("p t -> p t ()"),
                        )
                        nc.vector.tensor_copy(
                            ch3[:, :, 1:2],
                            tokf[:, ht0:ht0 + hw].rearrange("p t -> p t ()"),
                        )
                        for tt in range(hw):
                            tgt = (table2, table2b, table2c, table2d)[(ht0 + tt) % 4]
                            nc.gpsimd.indirect_dma_start(
                                out=tgt[:],
                                out_offset=bass.IndirectOffsetOnAxis(
                                    ap=keyih[:, tt:tt + 1], axis=0),
                                in_=comboh[:, 2 * tt:2 * tt + 2],
                                in_offset=None,
                            )

                # ---- counts / starts / z-loss tail / aux ----
                nc.sync.dma_start(out=counts_d[:], in_=base[:])
                c1 = spool.tile([1, E], f32, tag="c1")
                nc.vector.tensor_copy(c1[:], base[:])
                nc.vector.tensor_tensor(out=c1[:, 1:E], in0=base[:, 1:E], in1=base[:, 0:E - 1], op=OP.add)
                c2 = spool.tile([1, E], f32, tag="c2")
                nc.vector.tensor_copy(c2[:], c1[:])
                nc.vector.tensor_tensor(out=c2[:, 2:E], in0=c1[:, 2:E], in1=c1[:, 0:E - 2], op=OP.add)
                c3 = spool.tile([1, E], f32, tag="c3")
                nc.vector.tensor_copy(c3[:], c2[:])
                nc.vector.tensor_tensor(out=c3[:, 4:E], in0=c2[:, 4:E], in1=c2[:, 0:E - 4], op=OP.add)
                excl = spool.tile([1, E], f32, tag="excl")
                nc.vector.tensor_tensor(out=excl[:], in0=c3[:], in1=base[:], op=OP.subtract)

                # z-loss: lse = rowmax + ln(sumex), batched over all 64 tiles
                lns = spool.tile([P, NT], f32, tag="lns")
                nc.scalar.activation(lns[:], sumex_all[:], AF.Ln)
                nc.vector.tensor_tensor(out=lns[:], in0=lns[:], in1=rowmax_all[:], op=OP.add)
                nc.vector.tensor_tensor(out=lns[:], in0=lns[:], in1=lns[:], op=OP.mult)
                zrow = spool.tile([P, 1], f32, tag="zrow")
                nc.vector.tensor_reduce(zrow[:], lns[:], axis=AX.X, op=OP.add)
                # fold pacc4 -> [P, E]
                pacc = spool.tile([P, E], f32, tag="pacc")
                nc.vector.tensor_tensor(out=pacc[:], in0=pacc4[:, 0:E], in1=pacc4[:, E:2 * E], op=OP.add)
                nc.vector.tensor_tensor(out=pacc[:], in0=pacc[:], in1=pacc4[:, 2 * E:3 * E], op=OP.add)
                nc.vector.tensor_tensor(out=pacc[:], in0=pacc[:], in1=pacc4[:, 3 * E:4 * E], op=OP.add)

                ps_z = psS.tile([1, 1], f32, tag="ps_small", name="ps_z")
                nc.tensor.matmul(ps_z[:], lhsT=zrow[:], rhs=ones_c[:], start=True, stop=True)
                ps_p = psS.tile([1, E], f32, tag="ps_small", name="ps_p")
                nc.tensor.matmul(ps_p[:], lhsT=ones_c[:], rhs=pacc[:], start=True, stop=True)
                fp = spool.tile([1, E], f32, tag="fp")
                nc.vector.tensor_tensor(out=fp[:], in0=ps_p[:], in1=base[:], op=OP.mult)
                auxv = spool.tile([1, 1], f32, tag="auxv")
                nc.vector.tensor_reduce(auxv[:], fp[:], axis=AX.X, op=OP.add)
                nc.vector.tensor_scalar(
                    out=auxv[:], in0=auxv[:],
                    scalar1=AUX_LOSS_COEF * E / (float(N) * float(N)), scalar2=None,
                    op0=OP.mult,
                )
                zv = spool.tile([1, 1], f32, tag="zv")
                nc.vector.tensor_scalar(
                    out=zv[:], in0=ps_z[:], scalar1=Z_LOSS_COEF / float(N), scalar2=None,
                    op0=OP.mult,
                )
                nc.vector.tensor_tensor(out=auxv[:], in0=auxv[:], in1=zv[:], op=OP.add)
                nc.sync.dma_start(out=aux_d[:], in_=auxv[:])

                # broadcasts for phase B
                ps_sb2 = psS.tile([P, E], f32, tag="ps_small", name="ps_sb2")
                nc.tensor.matmul(ps_sb2[:], lhsT=ones_r[:], rhs=excl[:], start=True, stop=True)
                nc.vector.tensor_copy(startsBC[:], ps_sb2[:])
                ps_s2 = psS.tile([P, 1], f32, tag="ps_small", name="ps_s2")
                nc.tensor.matmul(ps_s2[:], lhsT=ones_r[:], rhs=ew_sb[:], start=True, stop=True)
                nc.vector.tensor_copy(ewBC[:], ps_s2[:])

            # ================= PHASE B1 + mm1 =================
            with (
                tc.tile_pool(name="psT", bufs=2, space="PSUM") as psT,
                tc.tile_pool(name="psM1", bufs=4, space="PSUM") as psM1,
                tc.tile_pool(name="w1p", bufs=1) as w1p,
            ):
                w1_sb = [w1p.tile([P, H], bf16, tag=f"w1_{k}", name=f"w1_{k}") for k in range(8)]
                for k in range(8):
                    nc.sync.dma_start(out=w1_sb[k][:], in_=w1_d[k * P:(k + 1) * P, :])
                for c in range(8):
                    s_f = tokf[:, c:c + 1]  # slot ids c*128+p
                    k1f = spool.tile([P, 1], f32, tag="k1f")
                    nc.vector.tensor_tensor(out=k1f[:], in0=ewBC[:], in1=s_f, op=OP.add)
                    nc.vector.tensor_scalar(
                        out=k1f[:], in0=k1f[:], scalar1=float(E * W - 1), scalar2=None,
                        op0=OP.min,
                    )
                    k1i = spool.tile([P, 1], i32, tag="k1i")
                    nc.vector.tensor_copy(k1i[:], k1f[:])
                    g1 = spool.tile([P, 2], f32, tag="g1")
                    nc.gpsimd.indirect_dma_start(
                        out=g1[:], out_offset=None,
                        in_=table2[:],
                        in_offset=bass.IndirectOffsetOnAxis(ap=k1i[:, 0:1], axis=0),
                    )
                    for nm, tbx in (("b", table2b), ("c", table2c), ("d", table2d)):
                        g1b = spool.tile([P, 2], f32, tag=f"g1{nm}", name=f"g1{nm}_{c}")
                        nc.gpsimd.indirect_dma_start(
                            out=g1b[:], out_offset=None,
                            in_=tbx[:],
                            in_offset=bass.IndirectOffsetOnAxis(ap=k1i[:, 0:1], axis=0),
                        )
                        sentm = spool.tile([P, 2], i32, tag="sentm")
                        nc.vector.tensor_scalar(
                            out=sentm[:], in0=g1[:], scalar1=float(N), scalar2=None,
                            op0=OP.is_equal,
                        )
                        nc.vector.copy_predicated(g1[:], sentm[:], g1b[:])
                    ti = spool.tile([P, 1], i32, tag="ti")
                    nc.vector.tensor_copy(ti[:], g1[:, 1:2])
                    nc.vector.tensor_copy(tf_all[:, c:c + 1], g1[:, 1:2])
                    nc.sync.dma_start(out=glist_d[c * P:(c + 1) * P, :], in_=ti[:])
                    xg = xpool.tile([P, D], f32, tag="xg")
                    nc.gpsimd.indirect_dma_start(
                        out=xg[:], out_offset=None,
                        in_=x_pad[:],
                        in_offset=bass.IndirectOffsetOnAxis(ap=ti[:, 0:1], axis=0),
                    )
                    # transpose gathered rows into bf16 xT strips
                    for k in range(8):
                        pst = psT.tile([P, P], f32)
                        nc.tensor.transpose(pst[:], xg[:, k * P:(k + 1) * P], ident[:])
                        nc.vector.tensor_copy(xT_bf[k][:, c * P:(c + 1) * P], pst[:])

                # m-lookup chain, off the mm1 critical path (feeds only mm2 scaling)
                for c in range(8):
                    # sorted position p = token id -> bucket -> table col0
                    pf = spool.tile([P, 1], f32, tag="pf")
                    nc.vector.tensor_copy(pf[:], tf_all[:, c:c + 1])
                    cmp = spool.tile([P, E], f32, tag="cmp")
                    nc.vector.tensor_tensor(
                        out=cmp[:], in0=pf[:].to_broadcast([P, E]), in1=startsBC[:],
                        op=OP.is_ge,
                    )
                    ehat = spool.tile([P, 1], f32, tag="ehat")
                    nc.vector.tensor_reduce(ehat[:], cmp[:], axis=AX.X, op=OP.add)
                    nc.vector.tensor_scalar(
                        out=ehat[:], in0=ehat[:], scalar1=-1.0, scalar2=None, op0=OP.add
                    )
                    oh8 = spool.tile([P, E], f32, tag="oh8")
                    nc.vector.tensor_scalar(
                        out=oh8[:], in0=iota8f[:], scalar1=ehat[:], scalar2=None,
                        op0=OP.is_equal,
                    )
                    sts = spool.tile([P, E], f32, tag="sts")
                    nc.vector.tensor_tensor(out=sts[:], in0=startsBC[:], in1=oh8[:], op=OP.mult)
                    stsel = spool.tile([P, 1], f32, tag="stsel")
                    nc.vector.tensor_reduce(stsel[:], sts[:], axis=AX.X, op=OP.add)
                    rr = spool.tile([P, 1], f32, tag="rr")
                    nc.vector.tensor_tensor(out=rr[:], in0=pf[:], in1=stsel[:], op=OP.subtract)
                    k2f = spool.tile([P, 1], f32, tag="k2f")
                    nc.vector.tensor_scalar(
                        out=k2f[:], in0=ehat[:], scalar1=float(W), scalar2=None, op0=OP.mult
                    )
                    nc.vector.tensor_tensor(out=k2f[:], in0=k2f[:], in1=rr[:], op=OP.add)
                    nc.vector.tensor_scalar(
                        out=k2f[:], in0=k2f[:], scalar1=float(E * W + 1), scalar2=None,
                        op0=OP.min,
                    )
                    k2i = spool.tile([P, 1], i32, tag="k2i")
                    nc.vector.tensor_copy(k2i[:], k2f[:])
                    g2 = spool.tile([P, 2], f32, tag="g2")
                    nc.gpsimd.indirect_dma_start(
                        out=g2[:], out_offset=None,
                        in_=table2[:],
                        in_offset=bass.IndirectOffsetOnAxis(ap=k2i[:, 0:1], axis=0),
                    )
                    for nm, tbx in (("b", table2b), ("c", table2c), ("d", table2d)):
                        g2b = spool.tile([P, 2], f32, tag=f"g2{nm}", name=f"g2{nm}_{c}")
                        nc.gpsimd.indirect_dma_start(
                            out=g2b[:], out_offset=None,
                            in_=tbx[:],
                            in_offset=bass.IndirectOffsetOnAxis(ap=k2i[:, 0:1], axis=0),
                        )
                        sentm2 = spool.tile([P, 2], i32, tag="sentm2")
                        nc.vector.tensor_scalar(
                            out=sentm2[:], in0=g2[:], scalar1=float(N), scalar2=None,
                            op0=OP.is_equal,
                        )
                        nc.vector.copy_predicated(g2[:], sentm2[:], g2b[:])
                    nc.vector.tensor_copy(m_all[:, c:c + 1], g2[:, 0:1])

                # ---- mm1 + gelu: hT[j] = gelu(x @ W1 + b1), stationary reused over n
                for j in range(H // P):
                    psm = [
                        psM1.tile([P, 512], f32, tag="ps_mm1", name=f"psm_{j}_{n}")
                        for n in range(2)
                    ]
                    for k in range(8):
                        for n in range(2):
                            nc.tensor.matmul(
                                psm[n][:],
                                lhsT=w1_sb[k][:, j * P:(j + 1) * P],
                                rhs=xT_bf[k][:, n * 512:(n + 1) * 512],
                                start=(k == 0), stop=(k == 7),
                            )
                    for n in range(2):
                        nc.scalar.activation(
                            hT[:, j * CAP + n * 512: j * CAP + (n + 1) * 512],
                            psm[n][:], AF.Gelu, bias=b1_sb[:, j:j + 1],
                        )

            # ================= PHASE B3: mm2 (+b2, *m) =================
            with (
                tc.tile_pool(name="psM2", bufs=1, space="PSUM") as psM2,
                tc.tile_pool(name="w2p", bufs=3) as w2pool,
                tc.tile_pool(name="outp", bufs=2) as opool,
            ):
                for half in range(2):
                    psos = []
                    for tb in range(4):
                        for db in range(2):
                            psos.append(psM2.tile([P, 512], f32, tag=f"ps_o{tb}{db}", name=f"ps_o{half}{tb}{db}"))
                    for k2 in range(H // P):
                        w2s = w2pool.tile([P, D], bf16, tag="w2s")
                        nc.sync.dma_start(out=w2s[:], in_=w2_d[k2 * P:(k2 + 1) * P, :])
                        for tb in range(4):
                            tokblk = half * 4 + tb
                            for db in range(2):
                                nc.tensor.matmul(
                                    psos[tb * 2 + db][:],
                                    lhsT=hT[:, k2 * CAP + tokblk * P:
                                            k2 * CAP + (tokblk + 1) * P],
                                    rhs=w2s[:, db * 512:(db + 1) * 512],
                                    start=(k2 == 0), stop=False,
                                )
                    # b2 add closes the accumulation group
                    for tb in range(4):
                        tokblk = half * 4 + tb
                        for db in range(2):
                            nc.tensor.matmul(
                                psos[tb * 2 + db][:],
                                lhsT=ones_r[:],
                                rhs=b2_sb[:, db * 512:(db + 1) * 512],
                                start=False, stop=True,
                            )
                        o_sb = opool.tile([P, D], f32, tag="o_sb")
                        for db in range(2):
                            nc.vector.tensor_scalar(
                                out=o_sb[:, db * 512:(db + 1) * 512],
                                in0=psos[tb * 2 + db][:],
                                scalar1=m_all[:, tokblk:tokblk + 1], scalar2=None,
                                op0=OP.mult,
                            )
                        nc.sync.dma_start(
                            out=o_d[tokblk * P:(tokblk + 1) * P, :], in_=o_sb[:]
                        )
    nc.finalize()
    return nc


def _get_nc():
    global _nc_cache
    if _nc_cache is None:
        _nc_cache = build_nc()
    return _nc_cache


def run(x, Wg, W1, b1, W2, b2, trace=False):
    from concourse.bass_utils import run_bass_kernel_spmd

    nc = _get_nc()
    xf = np.ascontiguousarray(np.asarray(x, np.float32).reshape(N, D))
    x_pad = np.concatenate([xf, np.zeros((1, D), np.float32)], 0)
    xT = np.ascontiguousarray(xf.T)
    Wg = np.ascontiguousarray(np.asarray(Wg, np.float32))
    W1b = np.asarray(W1, np.float32).astype(ml_dtypes.bfloat16)
    W2b = np.asarray(W2, np.float32).astype(ml_dtypes.bfloat16)
    b1f = np.asarray(b1, np.float32)
    b2f = np.asarray(b2, np.float32)

    in_maps = []
    for e in range(E):
        in_maps.append({
            "x": x_pad,
            "xt": xT,
            "wg": Wg,
            "w1": np.ascontiguousarray(W1b[e]),
            "b1": np.ascontiguousarray(b1f[e].reshape(H, 1)),
            "w2": np.ascontiguousarray(W2b[e]),
            "b2": np.ascontiguousarray(b2f[e].reshape(D, 1)),
            "ew": np.full((1, 1), float(e * W), np.float32),
        })
    res = run_bass_kernel_spmd(nc, in_maps, core_ids=list(range(E)), trace=trace)

    out = np.zeros((N, D), np.float32)
    for e in range(E):
        r = res.results[e]
        cnt = min(int(round(float(r["counts"][0, e]))), CAP)
        idx = r["glist"][:cnt, 0].astype(np.int64)
        out[idx] = r["o"][:cnt]
    aux = np.float32(res.results[0]["aux"][0, 0])
    return (out.reshape(B, T, D), aux), res


def kernel(x, Wg, W1, b1, W2, b2):
    (out, aux), _ = run(x, Wg, W1, b1, W2, b2, trace=False)
    return out, aux


# revision 31
# speedup vs baseline: 1.6932x; 1.0275x over previous
"""MoE (top-1 routing, E=8 experts) Trainium2 Bass kernel.

Sharding: expert-parallel over 8 NeuronCores. Every core runs a replicated
fp32 router over all N=8192 tokens (reads host-pretransposed xT), builds the
global dispatch table via counting-sort (one triangular-matmul cumsum per
4-tile block + indirect DMA scatter), then computes ONLY its own expert's
capacity-limited MLP in bf16 (fp32 accumulate). Host combine is a pure
index-based scatter of the per-expert compact outputs using device-computed
token indices.

Self-contained: hardcodes shapes from the problem spec.
"""

import numpy as np
import ml_dtypes

B, T, D, H, E = 4, 2048, 1024, 4096, 8
N = B * T            # 8192 tokens
CAP = 1024           # capacity = max(4, ceil(N/E))
W = 2048             # table bucket width (max supported tokens/expert)
P = 128
NT = N // P          # 64 token tiles
NB = NT // 4         # 16 blocks of 4 tiles (512 tokens)
AUX_LOSS_COEF = 0.01
Z_LOSS_COEF = 0.001

_nc_cache = None


def build_nc():
    import concourse.bacc as bacc
    import concourse.tile as tile
    import concourse.bass as bass
    import concourse.mybir as mybir
    from concourse.masks import make_identity, make_upper_triangular

    f32 = mybir.dt.float32
    bf16 = mybir.dt.bfloat16
    i32 = mybir.dt.int32
    AF = mybir.ActivationFunctionType
    OP = mybir.AluOpType
    AX = mybir.AxisListType

    nc = bacc.Bacc(None, target_bir_lowering=False, num_swdge_queues=2)

    # ---- I/O ----
    x_pad = nc.dram_tensor("x", [N + 1, D], f32, kind="ExternalInput")
    xT_d = nc.dram_tensor("xt", [D, N], f32, kind="ExternalInput")
    wg_d = nc.dram_tensor("wg", [D, E], f32, kind="ExternalInput")
    w1_d = nc.dram_tensor("w1", [D, H], bf16, kind="ExternalInput")
    b1_d = nc.dram_tensor("b1", [H, 1], f32, kind="ExternalInput")
    w2_d = nc.dram_tensor("w2", [H, D], bf16, kind="ExternalInput")
    b2_d = nc.dram_tensor("b2", [D, 1], f32, kind="ExternalInput")
    ew_d = nc.dram_tensor("ew", [1, 1], f32, kind="ExternalInput")     # e*W

    o_d = nc.dram_tensor("o", [CAP, D], f32, kind="ExternalOutput")
    glist_d = nc.dram_tensor("glist", [CAP, 1], i32, kind="ExternalOutput")
    counts_d = nc.dram_tensor("counts", [1, E], f32, kind="ExternalOutput")
    aux_d = nc.dram_tensor("aux", [1, 1], f32, kind="ExternalOutput")

    table2 = nc.dram_tensor("table2", [E * W + 2, 2], f32, kind="Internal")
    table2b = nc.dram_tensor("table2b", [E * W + 2, 2], f32, kind="Internal")
    table2c = nc.dram_tensor("table2c", [E * W + 2, 2], f32, kind="Internal")
    table2d = nc.dram_tensor("table2d", [E * W + 2, 2], f32, kind="Internal")

    with tile.TileContext(nc) as tc:
        with (
            tc.tile_pool(name="const", bufs=1) as cpool,
            tc.tile_pool(name="resident", bufs=1) as rpool,
            tc.tile_pool(name="xin", bufs=2) as xpool,
            tc.tile_pool(name="small", bufs=4) as spool,
        ):
            # ---------------- constants ----------------
            ident = cpool.tile([P, P], f32)
            make_identity(nc, ident[:])
            ident8 = cpool.tile([8, 8], f32)
            make_identity(nc, ident8[:])
            U128 = cpool.tile([P, P], f32)
            make_upper_triangular(nc, U128[:], val=1.0, diag=True)
            ones_r = cpool.tile([1, P], f32)   # row of ones (K=1 bcast matmuls)
            nc.gpsimd.memset(ones_r[:], 1.0)
            ones_c = cpool.tile([P, 1], f32)   # column of ones (partition reduce)
            nc.gpsimd.memset(ones_c[:], 1.0)
            iota8_i = cpool.tile([P, E], i32)
            nc.gpsimd.iota(iota8_i[:], pattern=[[1, E]], base=0, channel_multiplier=0)
            iota8f = cpool.tile([P, E], f32)
            nc.vector.tensor_copy(iota8f[:], iota8_i[:])
            tok_i = cpool.tile([P, NT], i32)   # tok_i[p, t] = t*128 + p
            nc.gpsimd.iota(tok_i[:], pattern=[[P, NT]], base=0, channel_multiplier=1)
            tokf = cpool.tile([P, NT], f32)
            nc.vector.tensor_copy(tokf[:], tok_i[:])

            wg_sb = cpool.tile([P, E * 8], f32)   # wg strips: col k*8+e
            nc.sync.dma_start(
                out=wg_sb[:].rearrange("p (k e) -> p k e", e=E),
                in_=wg_d[:].rearrange("(k p) e -> p k e", p=P),
            )
            b1_sb = cpool.tile([P, H // P], f32)  # b1[j*128+p] -> [p, j]
            nc.sync.dma_start(
                out=b1_sb[:].rearrange("p j -> p j ()"),
                in_=b1_d[:].rearrange("(j p) o -> p j o", p=P),
            )
            b2_sb = cpool.tile([1, D], f32)
            nc.sync.dma_start(out=b2_sb[:], in_=b2_d[:].rearrange("d o -> o d"))
            ew_sb = cpool.tile([1, 1], f32)
            nc.sync.dma_start(out=ew_sb[:], in_=ew_d[:])

            # accumulators
            pacc4 = rpool.tile([P, 4 * E], f32, tag="pacc4")
            nc.gpsimd.memset(pacc4[:], 0.0)
            base = rpool.tile([1, E], f32, tag="base")
            nc.gpsimd.memset(base[:], 0.0)
            rowmax_all = rpool.tile([P, NT], f32, tag="rowmax_all")
            sumex_all = rpool.tile([P, NT], f32, tag="sumex_all")
            oh_all = rpool.tile([P, NT * E], f32, tag="oh_all")
            pmax_all = rpool.tile([P, NT], f32, tag="pmax_all")
            eidx_all = rpool.tile([P, NT], f32, tag="eidx_all")

            # table2 init to sentinel 8192.0 (one DMA from an SBUF tile)
            sent = rpool.tile([P, E * W * 2 // P], f32, tag="sent")
            nc.gpsimd.memset(sent[:], float(N))
            for tb2 in (table2, table2b, table2c, table2d):
                nc.sync.dma_start(
                    out=tb2[0:E * W, :].rearrange("(a r) c -> a (r c)", a=P),
                    in_=sent[:],
                )
                nc.sync.dma_start(out=tb2[E * W:E * W + 2, :], in_=sent[0:2, 0:2])

            # resident MLP buffers
            xT_bf = [rpool.tile([P, CAP], bf16, tag=f"xT_{k}", name=f"xT_{k}") for k in range(8)]
            # hT layout: [128 h-partitions, (j, tok) free]: col j*CAP + tok
            hT = rpool.tile([P, (H // P) * CAP], bf16, tag="hT")
            m_all = rpool.tile([P, 8], f32, tag="m_all")
            tf_all = rpool.tile([P, 8], f32, tag="tf_all")
            startsBC = rpool.tile([P, E], f32, tag="startsBC")
            ewBC = rpool.tile([P, 1], f32, tag="ewBC")

            # ================= PHASE A: replicated router =================
            with (
                tc.tile_pool(name="psLT", bufs=2, space="PSUM") as psLT,
                tc.tile_pool(name="psL", bufs=4, space="PSUM") as psL,
                tc.tile_pool(name="psS", bufs=2, space="PSUM") as psS,
                tc.tile_pool(name="xsp", bufs=2) as xsp,
            ):
                for b in range(NB):
                    # load xT strip-block [1024 d, 512 tok] as 8 strips
                    xs = []
                    for k in range(8):
                        s = xsp.tile([P, 512], f32, tag=f"xs{k}", name=f"xs_{b}_{k}")
                        nc.sync.dma_start(
                            out=s[:], in_=xT_d[k * P:(k + 1) * P, b * 512:(b + 1) * 512]
                        )
                        xs.append(s)
                    # logitsT [8, 512] accumulated over k (wg stationary)
                    ps_lt = psLT.tile([8, 512], f32, tag="ps_lt")
                    for k in range(8):
                        nc.tensor.matmul(
                            ps_lt[:], lhsT=wg_sb[:, k * 8:k * 8 + E], rhs=xs[k][:],
                            start=(k == 0), stop=(k == 7),
                        )
                    lt_sb = spool.tile([8, 512], f32, tag="lt_sb")
                    nc.vector.tensor_copy(lt_sb[:], ps_lt[:])
                    # transpose back into [128 tok, (t,8)] layout
                    logits4 = spool.tile([P, 4 * E], f32, tag="logits4")
                    for t in range(4):
                        ps_l = psL.tile([P, E], f32, tag="ps_l")
                        nc.tensor.transpose(ps_l[:], lt_sb[:, t * P:(t + 1) * P], ident8[:])
                        nc.vector.tensor_copy(logits4[:, t * E:(t + 1) * E], ps_l[:])

                    l3 = logits4[:].rearrange("p (t e) -> p t e", e=E)
                    rm4 = rowmax_all[:, b * 4:(b + 1) * 4]
                    nc.vector.tensor_reduce(rm4, l3, axis=AX.X, op=OP.max)
                    rm4b = rowmax_all[:].rearrange("p t -> p t ()")[
                        :, b * 4:(b + 1) * 4, :
                    ].to_broadcast([P, 4, E])
                    sh4 = spool.tile([P, 4 * E], f32, tag="sh4")
                    sh43 = sh4[:].rearrange("p (t e) -> p t e", e=E)
                    nc.vector.tensor_tensor(out=sh43, in0=l3, in1=rm4b, op=OP.subtract)
                    ex4 = spool.tile([P, 4 * E], f32, tag="ex4")
                    nc.scalar.activation(ex4[:], sh4[:], AF.Exp)
                    ex43 = ex4[:].rearrange("p (t e) -> p t e", e=E)
                    se4 = sumex_all[:, b * 4:(b + 1) * 4]
                    nc.vector.tensor_reduce(se4, ex43, axis=AX.X, op=OP.add)
                    rec4 = spool.tile([P, 4], f32, tag="rec4")
                    nc.vector.reciprocal(rec4[:], se4)
                    rec4b = rec4[:].rearrange("p t -> p t ()").to_broadcast([P, 4, E])
                    probs4 = spool.tile([P, 4 * E], f32, tag="probs4")
                    p43 = probs4[:].rearrange("p (t e) -> p t e", e=E)
                    nc.vector.tensor_tensor(out=p43, in0=ex43, in1=rec4b, op=OP.mult)
                    nc.vector.tensor_tensor(out=pacc4[:], in0=pacc4[:], in1=probs4[:], op=OP.add)
                    nc.vector.tensor_reduce(pmax_all[:, b * 4:(b + 1) * 4], p43, axis=AX.X, op=OP.max)

                    # first-argmax per tile: min over masked expert iota
                    iota48 = iota8f[:].rearrange("p e -> p () e").to_broadcast([P, 4, E])
                    eq4 = spool.tile([P, 4 * E], f32, tag="eq4")
                    eq43 = eq4[:].rearrange("p (t e) -> p t e", e=E)
                    nc.vector.tensor_tensor(out=eq43, in0=l3, in1=rm4b, op=OP.is_equal)
                    m14 = spool.tile([P, 4 * E], f32, tag="m14")
                    m143 = m14[:].rearrange("p (t e) -> p t e", e=E)
                    nc.vector.tensor_tensor(out=m143, in0=iota48, in1=eq43, op=OP.mult)
                    m24 = spool.tile([P, 4 * E], f32, tag="m24")
                    nc.vector.tensor_scalar(
                        out=m24[:], in0=eq4[:], scalar1=-9.0, scalar2=9.0,
                        op0=OP.mult, op1=OP.add,
                    )
                    nc.vector.tensor_tensor(out=m14[:], in0=m14[:], in1=m24[:], op=OP.add)
                    eidx4 = eidx_all[:, b * 4:(b + 1) * 4]
                    nc.vector.tensor_reduce(eidx4, m143, axis=AX.X, op=OP.min)
                    eidx4b = eidx_all[:].rearrange("p t -> p t ()")[
                        :, b * 4:(b + 1) * 4, :
                    ].to_broadcast([P, 4, E])
                    oh43 = oh_all[:].rearrange("p (t e) -> p t e", e=E)[
                        :, b * 4:(b + 1) * 4, :
                    ]
                    nc.vector.tensor_tensor(out=oh43, in0=iota48, in1=eidx4b, op=OP.is_equal)

                    # pmax into resident store (for scatter payload later)
                    # (pmax4 already written via pmax_all slice above)

                    if b % (NB // 4) == (NB // 4) - 1:
                        hb = b // (NB // 4)          # which quarter just finished
                        hw = NT // 4                 # 16 tiles per quarter
                        ht0 = hb * hw                # first tile of half
                        c0 = ht0 * E                 # first oh column
                        cw = hw * E                  # 256 columns
                        # tile totals for this half: [1, (t,e)]
                        ps_tt = psS.tile([1, cw], f32, tag="ps_small", name=f"ps_tt{hb}")
                        nc.tensor.matmul(
                            ps_tt[:], lhsT=ones_c[:], rhs=oh_all[:, c0:c0 + cw],
                            start=True, stop=True,
                        )
                        tots = spool.tile([1, cw], f32, tag="tots", name=f"tots{hb}", bufs=2)
                        nc.vector.tensor_copy(tots[:], ps_tt[:])
                        # inclusive prefix over tiles (shift-adds), then exclusive
                        for sh in [1, 2, 4, 8]:
                            nc.vector.tensor_tensor(
                                out=tots[:, sh * E:cw], in0=tots[:, sh * E:cw],
                                in1=tots[:, 0:cw - sh * E], op=OP.add,
                            )
                        # exclusive prefix = inclusive shifted right one tile
                        exclp = spool.tile([1, cw], f32, tag="exclp", name=f"exclp{hb}", bufs=2)
                        nc.vector.memset(exclp[:, 0:E], 0.0)
                        nc.vector.tensor_copy(exclp[:, E:cw], tots[:, 0:cw - E])
                        # baseb[t] = carry base (prev halves) + exclusive prefix
                        baseb = spool.tile([1, cw], f32, tag="baseb", name=f"baseb{hb}", bufs=2)
                        bb3 = baseb[:].rearrange("o (t e) -> o t e", e=E)
                        nc.vector.tensor_tensor(
                            out=bb3,
                            in0=exclp[:].rearrange("o (t e) -> o t e", e=E),
                            in1=base[:].rearrange("o e -> o () e").to_broadcast([1, hw, E]),
                            op=OP.add,
                        )
                        # update global base with this half's grand total (last inclusive)
                        nc.vector.tensor_tensor(
                            out=base[:], in0=base[:], in1=tots[:, cw - E:cw], op=OP.add,
                        )
                        # rank psum = per-tile cumsum + base broadcast (one accum group)
                        ps_rk = psLT.tile([P, cw], f32, tag="ps_lt", name=f"ps_rk{hb}")
                        nc.tensor.matmul(
                            ps_rk[:], lhsT=U128[:], rhs=oh_all[:, c0:c0 + cw],
                            start=True, stop=False,
                        )
                        nc.tensor.matmul(
                            ps_rk[:], lhsT=ones_r[:], rhs=baseb[:],
                            start=False, stop=True,
                        )
                        sel = spool.tile([P, cw], f32, tag="selh", name=f"selh{hb}", bufs=2)
                        nc.vector.tensor_tensor(
                            out=sel[:], in0=ps_rk[:], in1=oh_all[:, c0:c0 + cw], op=OP.mult,
                        )
                        rankh = spool.tile([P, hw], f32, tag="rankh", name=f"rankh{hb}", bufs=2)
                        nc.vector.tensor_reduce(
                            rankh[:], sel[:].rearrange("p (t e) -> p t e", e=E),
                            axis=AX.X, op=OP.add,
                        )
                        nc.vector.tensor_scalar(
                            out=rankh[:], in0=rankh[:], scalar1=-1.0, scalar2=None, op0=OP.add
                        )
                        # keys = eidx*W + rank, clamped to trash row if rank >= W
                        keyh = spool.tile([P, hw], f32, tag="keyh", name=f"keyh{hb}", bufs=2)
                        nc.vector.tensor_scalar(
                            out=keyh[:], in0=eidx_all[:, ht0:ht0 + hw],
                            scalar1=float(W), scalar2=None, op0=OP.mult,
                        )
                        nc.vector.tensor_tensor(out=keyh[:], in0=keyh[:], in1=rankh[:], op=OP.add)
                        okh = spool.tile([P, hw], f32, tag="okh", name=f"okh{hb}", bufs=2)
                        nc.vector.tensor_scalar(
                            out=okh[:], in0=rankh[:], scalar1=float(W), scalar2=None, op0=OP.is_lt
                        )
                        nc.vector.tensor_tensor(out=keyh[:], in0=keyh[:], in1=okh[:], op=OP.mult)
                        nc.vector.tensor_scalar(
                            out=okh[:], in0=okh[:], scalar1=-float(E * W), scalar2=float(E * W),
                            op0=OP.mult, op1=OP.add,
                        )
                        nc.vector.tensor_tensor(out=keyh[:], in0=keyh[:], in1=okh[:], op=OP.add)
                        keyih = spool.tile([P, hw], i32, tag="keyih", name=f"keyih{hb}", bufs=2)
                        nc.vector.tensor_copy(keyih[:], keyh[:])
                        comboh = spool.tile([P, hw * 2], f32, tag="comboh", name=f"comboh{hb}", bufs=2)
                        ch3 = comboh[:].rearrange("p (t c) -> p t c", c=2)
                        nc.vector.tensor_copy(
                            ch3[:, :, 0:1],
                            pmax_all[:, ht0:ht0 + hw].rearrange("p t -> p t ()"),
                        )
                        nc.vector.tensor_copy(
                            ch3[:, :, 1:2],
                            tokf[:, ht0:ht0 + hw].rearrange("p t -> p t ()"),
                        )
                        for tt in range(hw):
                            tgt = (table2, table2b, table2c, table2d)[(ht0 + tt) % 4]
                            nc.gpsimd.indirect_dma_start(
                                out=tgt[:],
                                out_offset=bass.IndirectOffsetOnAxis(
                                    ap=keyih[:, tt:tt + 1], axis=0),
                                in_=comboh[:, 2 * tt:2 * tt + 2],
                                in_offset=None,
                            )

                # ---- counts / starts / z-loss tail / aux ----
                nc.sync.dma_start(out=counts_d[:], in_=base[:])
                c1 = spool.tile([1, E], f32, tag="c1")
                nc.vector.tensor_copy(c1[:], base[:])
                nc.vector.tensor_tensor(out=c1[:, 1:E], in0=base[:, 1:E], in1=base[:, 0:E - 1], op=OP.add)
                c2 = spool.tile([1, E], f32, tag="c2")
                nc.vector.tensor_copy(c2[:], c1[:])
                nc.vector.tensor_tensor(out=c2[:, 2:E], in0=c1[:, 2:E], in1=c1[:, 0:E - 2], op=OP.add)
                c3 = spool.tile([1, E], f32, tag="c3")
                nc.vector.tensor_copy(c3[:], c2[:])
                nc.vector.tensor_tensor(out=c3[:, 4:E], in0=c2[:, 4:E], in1=c2[:, 0:E - 4], op=OP.add)
                excl = spool.tile([1, E], f32, tag="excl")
                nc.vector.tensor_tensor(out=excl[:], in0=c3[:], in1=base[:], op=OP.subtract)

                # z-loss: lse = rowmax + ln(sumex), batched over all 64 tiles
                lns = spool.tile([P, NT], f32, tag="lns")
                nc.scalar.activation(lns[:], sumex_all[:], AF.Ln)
                nc.vector.tensor_tensor(out=lns[:], in0=lns[:], in1=rowmax_all[:], op=OP.add)
                nc.vector.tensor_tensor(out=lns[:], in0=lns[:], in1=lns[:], op=OP.mult)
                zrow = spool.tile([P, 1], f32, tag="zrow")
                nc.vector.tensor_reduce(zrow[:], lns[:], axis=AX.X, op=OP.add)
                # fold pacc4 -> [P, E]
                pacc = spool.tile([P, E], f32, tag="pacc")
                nc.vector.tensor_tensor(out=pacc[:], in0=pacc4[:, 0:E], in1=pacc4[:, E:2 * E], op=OP.add)
                nc.vector.tensor_tensor(out=pacc[:], in0=pacc[:], in1=pacc4[:, 2 * E:3 * E], op=OP.add)
                nc.vector.tensor_tensor(out=pacc[:], in0=pacc[:], in1=pacc4[:, 3 * E:4 * E], op=OP.add)

                ps_z = psS.tile([1, 1], f32, tag="ps_small", name="ps_z")
                nc.tensor.matmul(ps_z[:], lhsT=zrow[:], rhs=ones_c[:], start=True, stop=True)
                ps_p = psS.tile([1, E], f32, tag="ps_small", name="ps_p")
                nc.tensor.matmul(ps_p[:], lhsT=ones_c[:], rhs=pacc[:], start=True, stop=True)
                fp = spool.tile([1, E], f32, tag="fp")
                nc.vector.tensor_tensor(out=fp[:], in0=ps_p[:], in1=base[:], op=OP.mult)
                auxv = spool.tile([1, 1], f32, tag="auxv")
                nc.vector.tensor_reduce(auxv[:], fp[:], axis=AX.X, op=OP.add)
                nc.vector.tensor_scalar(
                    out=auxv[:], in0=auxv[:],
                    scalar1=AUX_LOSS_COEF * E / (float(N) * float(N)), scalar2=None,
                    op0=OP.mult,
                )
                zv = spool.tile([1, 1], f32, tag="zv")
                nc.vector.tensor_scalar(
                    out=zv[:], in0=ps_z[:], scalar1=Z_LOSS_COEF / float(N), scalar2=None,
                    op0=OP.mult,
                )
                nc.vector.tensor_tensor(out=auxv[:], in0=auxv[:], in1=zv[:], op=OP.add)
                nc.sync.dma_start(out=aux_d[:], in_=auxv[:])

                # broadcasts for phase B
                ps_sb2 = psS.tile([P, E], f32, tag="ps_small", name="ps_sb2")
                nc.tensor.matmul(ps_sb2[:], lhsT=ones_r[:], rhs=excl[:], start=True, stop=True)
                nc.vector.tensor_copy(startsBC[:], ps_sb2[:])
                ps_s2 = psS.tile([P, 1], f32, tag="ps_small", name="ps_s2")
                nc.tensor.matmul(ps_s2[:], lhsT=ones_r[:], rhs=ew_sb[:], start=True, stop=True)
                nc.vector.tensor_copy(ewBC[:], ps_s2[:])

            # ================= PHASE B1 + mm1 =================
            with (
                tc.tile_pool(name="psT", bufs=2, space="PSUM") as psT,
                tc.tile_pool(name="psM1", bufs=6, space="PSUM") as psM1,
                tc.tile_pool(name="w1p", bufs=1) as w1p,
            ):
                w1_sb = [w1p.tile([P, H], bf16, tag=f"w1_{k}", name=f"w1_{k}") for k in range(8)]
                for k in range(8):
                    nc.sync.dma_start(out=w1_sb[k][:], in_=w1_d[k * P:(k + 1) * P, :])
                for c in range(8):
                    s_f = tokf[:, c:c + 1]  # slot ids c*128+p
                    k1f = spool.tile([P, 1], f32, tag="k1f")
                    nc.vector.tensor_tensor(out=k1f[:], in0=ewBC[:], in1=s_f, op=OP.add)
                    nc.vector.tensor_scalar(
                        out=k1f[:], in0=k1f[:], scalar1=float(E * W - 1), scalar2=None,
                        op0=OP.min,
                    )
                    k1i = spool.tile([P, 1], i32, tag="k1i")
                    nc.vector.tensor_copy(k1i[:], k1f[:])
                    g1 = spool.tile([P, 2], f32, tag="g1")
                    nc.gpsimd.indirect_dma_start(
                        out=g1[:], out_offset=None,
                        in_=table2[:],
                        in_offset=bass.IndirectOffsetOnAxis(ap=k1i[:, 0:1], axis=0),
                    )
                    for nm, tbx in (("b", table2b), ("c", table2c), ("d", table2d)):
                        g1b = spool.tile([P, 2], f32, tag=f"g1{nm}", name=f"g1{nm}_{c}")
                        nc.gpsimd.indirect_dma_start(
                            out=g1b[:], out_offset=None,
                            in_=tbx[:],
                            in_offset=bass.IndirectOffsetOnAxis(ap=k1i[:, 0:1], axis=0),
                        )
                        sentm = spool.tile([P, 2], i32, tag="sentm")
                        nc.vector.tensor_scalar(
                            out=sentm[:], in0=g1[:], scalar1=float(N), scalar2=None,
                            op0=OP.is_equal,
                        )
                        nc.vector.copy_predicated(g1[:], sentm[:], g1b[:])
                    ti = spool.tile([P, 1], i32, tag="ti")
                    nc.vector.tensor_copy(ti[:], g1[:, 1:2])
                    nc.vector.tensor_copy(tf_all[:, c:c + 1], g1[:, 1:2])
                    nc.sync.dma_start(out=glist_d[c * P:(c + 1) * P, :], in_=ti[:])
                    xg = xpool.tile([P, D], f32, tag="xg")
                    nc.gpsimd.indirect_dma_start(
                        out=xg[:], out_offset=None,
                        in_=x_pad[:],
                        in_offset=bass.IndirectOffsetOnAxis(ap=ti[:, 0:1], axis=0),
                    )
                    # transpose gathered rows into bf16 xT strips
                    for k in range(8):
                        pst = psT.tile([P, P], f32)
                        nc.tensor.transpose(pst[:], xg[:, k * P:(k + 1) * P], ident[:])
                        nc.vector.tensor_copy(xT_bf[k][:, c * P:(c + 1) * P], pst[:])

                # m-lookup chain, off the mm1 critical path (feeds only mm2 scaling)
                for c in range(8):
                    # sorted position p = token id -> bucket -> table col0
                    pf = spool.tile([P, 1], f32, tag="pf")
                    nc.vector.tensor_copy(pf[:], tf_all[:, c:c + 1])
                    cmp = spool.tile([P, E], f32, tag="cmp")
                    nc.vector.tensor_tensor(
                        out=cmp[:], in0=pf[:].to_broadcast([P, E]), in1=startsBC[:],
                        op=OP.is_ge,
                    )
                    ehat = spool.tile([P, 1], f32, tag="ehat")
                    nc.vector.tensor_reduce(ehat[:], cmp[:], axis=AX.X, op=OP.add)
                    nc.vector.tensor_scalar(
                        out=ehat[:], in0=ehat[:], scalar1=-1.0, scalar2=None, op0=OP.add
                    )
                    oh8 = spool.tile([P, E], f32, tag="oh8")
                    nc.vector.tensor_scalar(
                        out=oh8[:], in0=iota8f[:], scalar1=ehat[:], scalar2=None,
                        op0=OP.is_equal,
                    )
                    sts = spool.tile([P, E], f32, tag="sts")
                    nc.vector.tensor_tensor(out=sts[:], in0=startsBC[:], in1=oh8[:], op=OP.mult)
                    stsel = spool.tile([P, 1], f32, tag="stsel")
                    nc.vector.tensor_reduce(stsel[:], sts[:], axis=AX.X, op=OP.add)
                    rr = spool.tile([P, 1], f32, tag="rr")
                    nc.vector.tensor_tensor(out=rr[:], in0=pf[:], in1=stsel[:], op=OP.subtract)
                    k2f = spool.tile([P, 1], f32, tag="k2f")
                    nc.vector.tensor_scalar(
                        out=k2f[:], in0=ehat[:], scalar1=float(W), scalar2=None, op0=OP.mult
                    )
                    nc.vector.tensor_tensor(out=k2f[:], in0=k2f[:], in1=rr[:], op=OP.add)
                    nc.vector.tensor_scalar(
                        out=k2f[:], in0=k2f[:], scalar1=float(E * W + 1), scalar2=None,
                        op0=OP.min,
                    )
                    k2i = spool.tile([P, 1], i32, tag="k2i")
                    nc.vector.tensor_copy(k2i[:], k2f[:])
                    g2 = spool.tile([P, 2], f32, tag="g2")
                    nc.gpsimd.indirect_dma_start(
                        out=g2[:], out_offset=None,
                        in_=table2[:],
                        in_offset=bass.IndirectOffsetOnAxis(ap=k2i[:, 0:1], axis=0),
                    )
                    for nm, tbx in (("b", table2b), ("c", table2c), ("d", table2d)):
                        g2b = spool.tile([P, 2], f32, tag=f"g2{nm}", name=f"g2{nm}_{c}")
                        nc.gpsimd.indirect_dma_start(
                            out=g2b[:], out_offset=None,
                            in_=tbx[:],
                            in_offset=bass.IndirectOffsetOnAxis(ap=k2i[:, 0:1], axis=0),
                        )
                        sentm2 = spool.tile([P, 2], i32, tag="sentm2")
                        nc.vector.tensor_scalar(
                            out=sentm2[:], in0=g2[:], scalar1=float(N), scalar2=None,
                            op0=OP.is_equal,
                        )
                        nc.vector.copy_predicated(g2[:], sentm2[:], g2b[:])
                    nc.vector.tensor_copy(m_all[:, c:c + 1], g2[:, 0:1])

                # ---- mm1 + gelu: hT[j] = gelu(x @ W1 + b1), stationary reused over n
                for j in range(H // P):
                    psm = [
                        psM1.tile([P, 512], f32, tag="ps_mm1", name=f"psm_{j}_{n}")
                        for n in range(2)
                    ]
                    for k in range(8):
                        for n in range(2):
                            nc.tensor.matmul(
                                psm[n][:],
                                lhsT=w1_sb[k][:, j * P:(j + 1) * P],
                                rhs=xT_bf[k][:, n * 512:(n + 1) * 512],
                                start=(k == 0), stop=(k == 7),
                            )
                    for n in range(2):
                        nc.scalar.activation(
                            hT[:, j * CAP + n * 512: j * CAP + (n + 1) * 512],
                            psm[n][:], AF.Gelu, bias=b1_sb[:, j:j + 1],
                        )

            # ================= PHASE B3: mm2 (+b2, *m) =================
            with (
                tc.tile_pool(name="psM2", bufs=1, space="PSUM") as psM2,
                tc.tile_pool(name="w2p", bufs=3) as w2pool,
                tc.tile_pool(name="outp", bufs=2) as opool,
            ):
                for half in range(2):
                    psos = []
                    for tb in range(4):
                        for db in range(2):
                            psos.append(psM2.tile([P, 512], f32, tag=f"ps_o{tb}{db}", name=f"ps_o{half}{tb}{db}"))
                    for k2 in range(H // P):
                        w2s = w2pool.tile([P, D], bf16, tag="w2s")
                        nc.sync.dma_start(out=w2s[:], in_=w2_d[k2 * P:(k2 + 1) * P, :])
                        for tb in range(4):
                            tokblk = half * 4 + tb
                            for db in range(2):
                                nc.tensor.matmul(
                                    psos[tb * 2 + db][:],
                                    lhsT=hT[:, k2 * CAP + tokblk * P:
                                            k2 * CAP + (tokblk + 1) * P],
                                    rhs=w2s[:, db * 512:(db + 1) * 512],
                                    start=(k2 == 0), stop=False,
                                )
                    # b2 add closes the accumulation group
                    for tb in range(4):
                        tokblk = half * 4 + tb
                        for db in range(2):
                            nc.tensor.matmul(
                                psos[tb * 2 + db][:],
                                lhsT=ones_r[:],
                                rhs=b2_sb[:, db * 512:(db + 1) * 512],
                                start=False, stop=True,
                            )
                        o_sb = opool.tile([P, D], f32, tag="o_sb")
                        for db in range(2):
                            nc.vector.tensor_scalar(
                                out=o_sb[:, db * 512:(db + 1) * 512],
                                in0=psos[tb * 2 + db][:],
                                scalar1=m_all[:, tokblk:tokblk + 1], scalar2=None,
                                op0=OP.mult,
                            )
                        nc.sync.dma_start(
                            out=o_d[tokblk * P:(tokblk + 1) * P, :], in_=o_sb[:]
                        )
    nc.finalize()
    return nc


def _get_nc():
    global _nc_cache
    if _nc_cache is None:
        _nc_cache = build_nc()
    return _nc_cache


def run(x, Wg, W1, b1, W2, b2, trace=False):
    from concourse.bass_utils import run_bass_kernel_spmd

    nc = _get_nc()
    xf = np.ascontiguousarray(np.asarray(x, np.float32).reshape(N, D))
    x_pad = np.concatenate([xf, np.zeros((1, D), np.float32)], 0)
    xT = np.ascontiguousarray(xf.T)
    Wg = np.ascontiguousarray(np.asarray(Wg, np.float32))
    W1b = np.asarray(W1, np.float32).astype(ml_dtypes.bfloat16)
    W2b = np.asarray(W2, np.float32).astype(ml_dtypes.bfloat16)
    b1f = np.asarray(b1, np.float32)
    b2f = np.asarray(b2, np.float32)

    in_maps = []
    for e in range(E):
        in_maps.append({
            "x": x_pad,
            "xt": xT,
            "wg": Wg,
            "w1": np.ascontiguousarray(W1b[e]),
            "b1": np.ascontiguousarray(b1f[e].reshape(H, 1)),
            "w2": np.ascontiguousarray(W2b[e]),
            "b2": np.ascontiguousarray(b2f[e].reshape(D, 1)),
            "ew": np.full((1, 1), float(e * W), np.float32),
        })
    res = run_bass_kernel_spmd(nc, in_maps, core_ids=list(range(E)), trace=trace)

    out = np.zeros((N, D), np.float32)
    for e in range(E):
        r = res.results[e]
        cnt = min(int(round(float(r["counts"][0, e]))), CAP)
        idx = r["glist"][:cnt, 0].astype(np.int64)
        out[idx] = r["o"][:cnt]
    aux = np.float32(res.results[0]["aux"][0, 0])
    return (out.reshape(B, T, D), aux), res


def kernel(x, Wg, W1, b1, W2, b2):
    (out, aux), _ = run(x, Wg, W1, b1, W2, b2, trace=False)
    return out, aux
